# revision 1
# baseline (speedup 1.0000x reference)
"""Trainium2 Bass kernel for nn_AttentionLayer (B=4, S=2048, D=1024, H=16).

Self-contained: builds and compiles an SPMD Bass/Tile program once, then
runs it across 8 NeuronCores via run_bass_kernel_spmd.

Sharding (no collectives): core c handles batch b = c // 2 and query-token
half c % 2 (1024 query tokens). Each core receives pre-transposed bf16
activations (x^T slices) plus bf16 weights, computes its [1024, 1024]
slice of the final layernorm output in fp32, and the host reassembles.

Per-core pipeline (all matmuls bf16 with fp32 PSUM accumulation):
- K / V projections as dense up-front TensorE phases (V in natural token-
  major layout with a per-head ones column so each head's attn@V matmul
  also produces its softmax denominator row).
- Attention processes heads sequentially: scores^T = Kh @ Qh^T into
  double-buffered PSUM, exp on ScalarE (scale=1/8 folded into the
  activation), attn@V accumulation; Q^T/residual projections are emitted
  one matmul per kb-step to fill TensorE slack inside the ACT-bound loop.
- Softmax normalization is deferred: denominators go to DRAM; per pair a
  broadcast-DMA + fast approximate reciprocal + one multiply normalizes
  the bf16 context off the critical path.
- FC + residual + layernorm finish per 128-token block.
"""

import numpy as np
import ml_dtypes


from contextlib import ExitStack

import concourse.bass as bass
import concourse.tile as tile
import concourse.mybir as mybir
from concourse import bacc

F32 = mybir.dt.float32
BF16 = mybir.dt.bfloat16
AF = mybir.ActivationFunctionType
ALU = mybir.AluOpType


def bcast_ap(ap: bass.AP, parts: int) -> bass.AP:
    """Partition-broadcast a [1, N]-shaped DRAM AP to [parts, N]."""
    return bass.AP(tensor=ap.tensor, offset=ap.offset,
                   ap=[[0, parts]] + list(ap.ap[-1:]))


def nsplits(total, cap=512):
    return [(i, min(cap, total - i)) for i in range(0, total, cap)]


def build(T=1024, S=2048, D=1024, H=16, DK=64, n_cores=8, eps=1e-5,
          trn_type="TRN2"):
    assert DK == 64 and H % 2 == 0 and D == H * DK
    DB = D // 128     # contraction chunks over d
    EB = D // 128     # e blocks (projection output chunks); == H//2
    TB = T // 128
    SB = S // 128
    PAIRS = H // 2
    VW = 65           # per-head vp stripe: 64 v columns + 1 ones column
    DEN_F = 2 * T // 128  # free size of the per-pair denominator tile

    nc = bacc.Bacc(trn_type, target_bir_lowering=False, debug=False,
                   num_devices=n_cores)

    qT = nc.dram_tensor("qT", [D, T], BF16, kind="ExternalInput").ap()
    kT = nc.dram_tensor("kT", [D, S], BF16, kind="ExternalInput").ap()
    vT = nc.dram_tensor("vT", [D, S], BF16, kind="ExternalInput").ap()
    Wq = nc.dram_tensor("Wq", [D, D], BF16, kind="ExternalInput").ap()
    Wk = nc.dram_tensor("Wk", [D, D], BF16, kind="ExternalInput").ap()
    Wv = nc.dram_tensor("Wv", [D, D], BF16, kind="ExternalInput").ap()
    Wfc = nc.dram_tensor("Wfc", [D, D], BF16, kind="ExternalInput").ap()
    bq = nc.dram_tensor("bq", [D], F32, kind="ExternalInput").ap()
    bk = nc.dram_tensor("bk", [D], F32, kind="ExternalInput").ap()
    bv = nc.dram_tensor("bv", [D], F32, kind="ExternalInput").ap()
    bfc = nc.dram_tensor("bfc", [D], F32, kind="ExternalInput").ap()
    gamma = nc.dram_tensor("gamma", [D], F32, kind="ExternalInput").ap()
    beta = nc.dram_tensor("beta", [D], F32, kind="ExternalInput").ap()
    out = nc.dram_tensor("out", [T, D], F32, kind="ExternalOutput").ap()

    den_dram = nc.dram_tensor("den_scratch", [H, T], F32).ap()
    qp_dram = nc.dram_tensor("qp_scratch", [T, D], F32).ap()
    rec_dram = nc.dram_tensor("rec_scratch", [H, T], F32).ap()

    with tile.TileContext(nc) as tc, ExitStack() as ctx:
        pconst = ctx.enter_context(tc.tile_pool(name="const", bufs=1))
        ppers = ctx.enter_context(tc.tile_pool(name="persist", bufs=1))

        # ---- tiny constants -------------------------------------------
        bqT = pconst.tile([128, EB], F32, tag="bqT", name="bqT")
        nc.sync.dma_start(out=bqT, in_=bq.rearrange("(e p) -> p e", p=128))
        bkT = pconst.tile([128, EB], F32, tag="bkT", name="bkT")
        nc.sync.dma_start(out=bkT, in_=bk.rearrange("(e p) -> p e", p=128))
        eps_t = pconst.tile([128, 1], F32, tag="eps", name="eps")
        nc.vector.memset(eps_t, eps)

        # ---- persistent outputs ---------------------------------------
        kpT_sb = [ppers.tile([128, S], BF16, tag=f"kpT{e}", name=f"kpT{e}")
                  for e in range(EB)]
        vp_sb = [ppers.tile([128, H * VW], BF16, tag=f"vp{s}", name=f"vp{s}")
                 for s in range(SB)]
        ctxT_sb = [ppers.tile([128, T], BF16, tag=f"ctxT{e}", name=f"ctxT{e}")
                   for e in range(EB)]

        pqx = ctx.enter_context(tc.tile_pool(name="qx", bufs=1))
        pwq = ctx.enter_context(tc.tile_pool(name="wq", bufs=1))
        qx_sb = [pqx.tile([128, T], BF16, tag=f"qx{d}", name=f"qx{d}")
                 for d in range(DB)]
        wq_sb = [pwq.tile([128, D], BF16, tag=f"wq{d}", name=f"wq{d}")
                 for d in range(DB)]

        # ================= K projection =================================
        # c-outer loop + per-half kx loads so the first matmul only waits
        # for wk + the first half of kT.
        with tc.tile_pool(name="wk", bufs=1) as pw, \
             tc.tile_pool(name="kx", bufs=1) as pkx, \
             tc.tile_pool(name="kps", bufs=3, space="PSUM") as pps:
            CK = min(S, 1024)
            NC_ = len(nsplits(S, CK))
            wk_sb = [pw.tile([128, D], BF16, tag=f"wk{d}", name=f"wk{d}")
                     for d in range(DB)]
            kx_sb = [[pkx.tile([128, CK], BF16, tag=f"kx{d}_{c}",
                               name=f"kx{d}_{c}") for c in range(NC_)]
                     for d in range(DB)]
            for d in range(DB):
                nc.sync.dma_start(out=wk_sb[d], in_=Wk[d * 128:(d + 1) * 128, :])
            for ci, (c0, cn) in enumerate(nsplits(S, CK)):
                for d in range(DB):
                    nc.sync.dma_start(out=kx_sb[d][ci][:, 0:cn],
                                      in_=kT[d * 128:(d + 1) * 128, c0:c0 + cn])
            # qx/wq load after kx (needed later, at attention start)
            for d in range(DB):
                nc.sync.dma_start(out=qx_sb[d], in_=qT[d * 128:(d + 1) * 128, :])
                nc.sync.dma_start(out=wq_sb[d], in_=Wq[d * 128:(d + 1) * 128, :])
            for ci, (c0, cn) in enumerate(nsplits(S, CK)):
                for e in range(EB):
                    ps = pps.tile([128, CK], F32, tag="kpT_ps", name="kpT_ps")
                    for d in range(DB):
                        for n0, nn in nsplits(cn):
                            nc.tensor.matmul(
                                ps[:, n0:n0 + nn],
                                lhsT=wk_sb[d][:, e * 128:(e + 1) * 128],
                                rhs=kx_sb[d][ci][:, n0:n0 + nn],
                                start=(d == 0), stop=(d == DB - 1))
                    nc.vector.tensor_scalar(
                        out=kpT_sb[e][:, c0:c0 + cn], in0=ps[:, 0:cn],
                        scalar1=bkT[:, e:e + 1], scalar2=None, op0=ALU.add)

        # ================= V projection (natural layout) ================
        with tc.tile_pool(name="wv", bufs=1) as pw, \
             tc.tile_pool(name="vx", bufs=1) as pvx, \
             tc.tile_pool(name="vbc", bufs=1) as pvbc, \
             tc.tile_pool(name="vps", bufs=3, space="PSUM") as pps:
            bv_bc = pvbc.tile([128, D], F32, tag="bv_bc", name="bv_bc")
            nc.gpsimd.dma_start(out=bv_bc, in_=bcast_ap(bv, 128))
            wv_sb = [pw.tile([128, D], BF16, tag=f"wv{d}", name=f"wv{d}")
                     for d in range(DB)]
            vx_sb = [pvx.tile([128, S], BF16, tag=f"vx{d}", name=f"vx{d}")
                     for d in range(DB)]
            for d in range(DB):
                nc.sync.dma_start(out=wv_sb[d], in_=Wv[d * 128:(d + 1) * 128, :])
                nc.sync.dma_start(out=vx_sb[d], in_=vT[d * 128:(d + 1) * 128, :])
            for s in range(SB):
                ps = pps.tile([128, D], F32, tag="vp_ps", name="vp_ps")
                for d in range(DB):
                    for n0, nn in nsplits(D):
                        nc.tensor.matmul(
                            ps[:, n0:n0 + nn],
                            lhsT=vx_sb[d][:, s * 128:(s + 1) * 128],
                            rhs=wv_sb[d][:, n0:n0 + nn],
                            start=(d == 0), stop=(d == DB - 1))
                vr = vp_sb[s].rearrange("p (h c) -> p h c", c=VW)
                nc.vector.tensor_add(
                    out=vr[:, :, 0:64],
                    in0=ps.rearrange("p (h c) -> p h c", c=DK),
                    in1=bv_bc.rearrange("p (h c) -> p h c", c=DK))
                nc.vector.memset(vr[:, :, 64:65], 1.0)

        # ================= attention (+ Q-proj, qp-nat interleaved) =====
        pwfc = ctx.enter_context(tc.tile_pool(name="wfc", bufs=1))
        wfc_sb = [pwfc.tile([128, D], BF16, tag=f"wfc{d}", name=f"wfc{d}")
                  for d in range(DB)]
        for d in range(DB):
            nc.gpsimd.dma_start(out=wfc_sb[d], in_=Wfc[d * 128:(d + 1) * 128, :])

        pbqfc = ctx.enter_context(tc.tile_pool(name="bqfcp", bufs=1))
        bqfc_bc = pbqfc.tile([128, D], F32, tag="bqfc", name="bqfc")
        nc.gpsimd.dma_start(out=bqfc_bc, in_=bcast_ap(bq, 128))
        tmp_bfc = pbqfc.tile([128, D], F32, tag="tmp_bfc", name="tmp_bfc")
        nc.gpsimd.dma_start(out=tmp_bfc, in_=bcast_ap(bfc, 128))
        nc.vector.tensor_add(out=bqfc_bc, in0=bqfc_bc, in1=tmp_bfc)

        with tc.tile_pool(name="scps", bufs=2, space="PSUM") as psc, \
             tc.tile_pool(name="cxps", bufs=1, space="PSUM") as pcx, \
             tc.tile_pool(name="pjps", bufs=1, space="PSUM") as ppj, \
             tc.tile_pool(name="qpT", bufs=3) as pqpt, \
             tc.tile_pool(name="attn", bufs=4) as patn, \
             tc.tile_pool(name="den", bufs=2) as pden, \
             tc.tile_pool(name="qpev", bufs=2) as pqpe, \
             tc.tile_pool(name="norm", bufs=2) as pnm, \
             tc.tile_pool(name="ctmp", bufs=2) as ptmp:
            qpT_tiles = {}

            def make_proj_thunks(jj):
                """Q^T proj + residual proj for pair jj as single-matmul
                thunks, consumed one per attention kb-step so TensorE slack
                is filled without stalling the exp stream."""
                state = {}
                thunks = []

                def qps_mm(d, n0, nn):
                    def f():
                        if 'qps' not in state:
                            state['qps'] = ppj.tile([128, T], F32, tag="pj",
                                                    name="pjq")
                        nc.tensor.matmul(
                            state['qps'][:, n0:n0 + nn],
                            lhsT=wq_sb[d][:, jj * 128:(jj + 1) * 128],
                            rhs=qx_sb[d][:, n0:n0 + nn],
                            start=(d == 0), stop=(d == DB - 1))
                    return f

                def qpt_evac():
                    qt = pqpt.tile([128, T], BF16, tag="qpT_t", name="qpT_t")
                    nc.vector.tensor_scalar(out=qt, in0=state['qps'],
                                            scalar1=bqT[:, jj:jj + 1],
                                            scalar2=None, op0=ALU.add)
                    qpT_tiles[jj] = qt

                def nps_mm(d, n0, nn):
                    def f():
                        if 'nps' not in state:
                            state['nps'] = ppj.tile([128, D], F32, tag="pj",
                                                    name="pjn")
                        nc.tensor.matmul(
                            state['nps'][:, n0:n0 + nn],
                            lhsT=qx_sb[d][:, jj * 128:(jj + 1) * 128],
                            rhs=wq_sb[d][:, n0:n0 + nn],
                            start=(d == 0), stop=(d == DB - 1))
                    return f

                def qp_evac():
                    ev = pqpe.tile([128, D], F32, tag="qp_ev", name="qp_ev")
                    nc.vector.tensor_add(out=ev, in0=state['nps'], in1=bqfc_bc)
                    nc.sync.dma_start(out=qp_dram[jj * 128:(jj + 1) * 128, :],
                                      in_=ev)

                for d in range(DB):
                    for n0, nn in nsplits(T):
                        thunks.append(qps_mm(d, n0, nn))
                thunks.append(qpt_evac)
                for d in range(DB):
                    for n0, nn in nsplits(D):
                        thunks.append(nps_mm(d, n0, nn))
                thunks.append(qp_evac)
                return thunks

            # pair 0's projections run during the V phase / attention ramp
            for th in make_proj_thunks(0):
                th()

            for j in range(PAIRS):
                qpT_t = qpT_tiles.pop(j)
                pending = make_proj_thunks(j + 1) if j + 1 < PAIRS else []
                for hh in range(2):
                    h = 2 * j + hh
                    pr = slice(hh * 64, hh * 64 + 64)
                    cx = pcx.tile([VW, T], F32, tag="cx", name="cx")
                    for kb in range(SB):
                        sc = psc.tile([128, T], F32, tag="sc", name="sc")
                        for n0, nn in nsplits(T):
                            nc.tensor.matmul(
                                sc[:, n0:n0 + nn],
                                lhsT=kpT_sb[j][pr, kb * 128:(kb + 1) * 128],
                                rhs=qpT_t[pr, n0:n0 + nn],
                                start=True, stop=True)
                        at = patn.tile([128, T], BF16, tag="at", name="at")
                        nc.scalar.activation(out=at, in_=sc, func=AF.Exp,
                                             scale=0.125)
                        vr = vp_sb[kb].rearrange("p (h c) -> p h c", c=VW)
                        for n0, nn in nsplits(T):
                            nc.tensor.matmul(
                                cx[:, n0:n0 + nn],
                                lhsT=vr[:, h, :],
                                rhs=at[:, n0:n0 + nn],
                                start=(kb == 0), stop=(kb == SB - 1))
                        if pending:
                            pending.pop(0)()
                    # evacuate unnormalized ctx + denominator
                    den = pden.tile([VW, T], F32, tag="den", name="den")
                    nc.vector.tensor_copy(out=den[64:65, :], in_=cx[64:65, :])
                    nc.gpsimd.dma_start(out=den_dram[h, :], in_=den[64:65, :])
                    if hh == 0:
                        nc.vector.tensor_copy(out=ctxT_sb[j][0:64, :],
                                              in_=cx[0:64, :])
                    else:
                        tmp = ptmp.tile([64, T], BF16, tag="ctmp", name="ctmp")
                        nc.vector.tensor_copy(out=tmp, in_=cx[0:64, :])
                        nc.sync.dma_start(out=ctxT_sb[j][64:128, :], in_=tmp)
                while pending:
                    pending.pop(0)()
                # normalize this pair's ctxT (cheap chain, off critical path)
                dbc = pnm.tile([128, T], F32, tag="dbc", name="dbc")
                nc.gpsimd.dma_start(
                    out=dbc[0:64, :],
                    in_=bcast_ap(den_dram[2 * j:2 * j + 1, :], 64))
                nc.gpsimd.dma_start(
                    out=dbc[64:128, :],
                    in_=bcast_ap(den_dram[2 * j + 1:2 * j + 2, :], 64))
                rbc = pnm.tile([128, T], F32, tag="rbc", name="rbc")
                nc.vector.reciprocal_approx_fast(out=rbc, in_=dbc)
                nc.vector.tensor_mul(out=ctxT_sb[j], in0=ctxT_sb[j], in1=rbc)

        # ================= FC + residual + layernorm ====================
        with tc.tile_pool(name="fcps", bufs=2, space="PSUM") as pfc, \
             tc.tile_pool(name="lnbc", bufs=1) as plnb, \
             tc.tile_pool(name="qpl", bufs=2) as pqp, \
             tc.tile_pool(name="xln", bufs=2) as px, \
             tc.tile_pool(name="stat", bufs=4) as pst:
            gamma_bc = plnb.tile([128, D], F32, tag="gamma_bc", name="gamma_bc")
            nc.gpsimd.dma_start(out=gamma_bc, in_=bcast_ap(gamma, 128))
            beta_bc = plnb.tile([128, D], F32, tag="beta_bc", name="beta_bc")
            nc.gpsimd.dma_start(out=beta_bc, in_=bcast_ap(beta, 128))

            for t in range(TB):
                qp_t = pqp.tile([128, D], F32, tag="qp_t", name="qp_t")
                nc.sync.dma_start(out=qp_t,
                                  in_=qp_dram[t * 128:(t + 1) * 128, :])
                fc = pfc.tile([128, D], F32, tag="fc", name="fc")
                for j in range(EB):
                    for n0, nn in nsplits(D):
                        nc.tensor.matmul(
                            fc[:, n0:n0 + nn],
                            lhsT=ctxT_sb[j][:, t * 128:(t + 1) * 128],
                            rhs=wfc_sb[j][:, n0:n0 + nn],
                            start=(j == 0), stop=(j == EB - 1))
                x = px.tile([128, D], F32, tag="x", name="x")
                nc.vector.tensor_add(out=x, in0=fc, in1=qp_t)
                ngr = max(D // 512, 1)
                gsz = min(D, 512)
                stats = pst.tile([128, ngr, 6], F32, tag="stats", name="stats")
                for g in range(ngr):
                    nc.vector.bn_stats(out=stats[:, g, :],
                                       in_=x[:, g * gsz:(g + 1) * gsz])
                mv = pst.tile([128, 2], F32, tag="mv", name="mv")
                nc.vector.bn_aggr(out=mv, in_=stats)
                rstd = pst.tile([128, 1], F32, tag="rstd", name="rstd")
                nc.scalar.activation(out=rstd, in_=mv[:, 1:2], func=AF.Sqrt,
                                     bias=eps_t, scale=1.0)
                nc.vector.reciprocal(out=rstd, in_=rstd)
                xn = px.tile([128, D], F32, tag="xn", name="xn")
                nc.vector.tensor_scalar(out=xn, in0=x, scalar1=mv[:, 0:1],
                                        scalar2=rstd, op0=ALU.subtract,
                                        op1=ALU.mult)
                nc.vector.tensor_mul(out=xn, in0=xn, in1=gamma_bc)
                nc.gpsimd.tensor_add(out=xn, in0=xn, in1=beta_bc)
                nc.sync.dma_start(out=out[t * 128:(t + 1) * 128, :], in_=xn)

    nc.compile()
    return nc


_B, _S, _D, _H, _DK = 4, 2048, 1024, 16, 64
_T = _S // 2
_NCORES = 8
_BF = ml_dtypes.bfloat16

_nc_cache = [None]


def _get_nc():
    if _nc_cache[0] is None:
        _nc_cache[0] = build(T=_T, S=_S, D=_D, H=_H, DK=_DK, n_cores=_NCORES)
    return _nc_cache[0]


def _execute(inputs, trace=False):
    from concourse.bass_utils import run_bass_kernel_spmd

    nc = _get_nc()
    q = np.asarray(inputs["q"], np.float32)
    k = np.asarray(inputs["k"], np.float32)
    v = np.asarray(inputs["v"], np.float32)
    Wq = np.asarray(inputs["Wq"], np.float32).astype(_BF)
    Wk = np.asarray(inputs["Wk"], np.float32).astype(_BF)
    Wv = np.asarray(inputs["Wv"], np.float32).astype(_BF)
    Wfc = np.asarray(inputs["Wfc"], np.float32).astype(_BF)
    fp = {n: np.asarray(inputs[n], np.float32)
          for n in ("bq", "bk", "bv", "bfc", "gamma", "beta")}

    in_maps = []
    for c in range(_NCORES):
        b, half = divmod(c, 2)
        t0 = half * _T
        in_maps.append({
            "qT": np.ascontiguousarray(q[b, t0:t0 + _T].T).astype(_BF),
            "kT": np.ascontiguousarray(k[b].T).astype(_BF),
            "vT": np.ascontiguousarray(v[b].T).astype(_BF),
            "Wq": Wq, "Wk": Wk, "Wv": Wv, "Wfc": Wfc, **fp,
        })

    res = run_bass_kernel_spmd(nc, in_maps, core_ids=list(range(_NCORES)),
                               trace=trace)
    out = np.empty((_B, _S, _D), np.float32)
    for c in range(_NCORES):
        b, half = divmod(c, 2)
        out[b, half * _T:(half + 1) * _T] = res.results[c]["out"]
    return out, res.exec_time_ns


def kernel(**inputs) -> np.ndarray:
    out, _ = _execute(inputs, trace=False)
    return out



# revision 26
# speedup vs baseline: 1.0697x; 1.0697x over previous
"""Trainium2 Bass kernel for nn_AttentionLayer (B=4, S=2048, D=1024, H=16).

Self-contained: builds and compiles an SPMD Bass/Tile program once, then
runs it across 8 NeuronCores via run_bass_kernel_spmd.

Sharding (no collectives): core c handles batch b = c // 2 and query-token
half c % 2 (T=1024 query tokens). Each core receives pre-transposed bf16
activations plus bf16 weights, computes its [1024, 1024] slice of the
final layernorm output in fp32, and the host reassembles.

v2 pipeline: one continuous ACT-overlapped stream. The exp stream
(ScalarE) runs near-continuously from ~30us onward; all projection work
(K/V per-pair, Q-natural + PE-transpose to qT-proj) is emitted as filler
thunks inside the attention loop so TensorE slack under the exp stream is
filled. Attention runs per head-pair with query-half sweeps so PSUM fits:
  sc pool 2x[128,1024] (4 banks) + cx 2x[65,512] (2) + K-proj (1) +
  V/qp/transpose fill (1) = 8 banks.
Scores for the two heads of a pair use disjoint PE row groups (contraction
64 at partitions 0-63 / 64-127) and are emitted adjacently so the HW packs
them concurrently. Softmax denominators come from a ones-column in the V
projection; reciprocals are computed in-PSUM and partition-broadcast via
GpSimd (no DRAM roundtrip). Residual q-projection is kept natural (bf16,
DRAM staging) and transposed on the PE for the attention layout.
"""

import numpy as np
import ml_dtypes

from contextlib import ExitStack

import concourse.bass as bass
import concourse.tile as tile
import concourse.mybir as mybir
from concourse import bacc
from concourse import masks

F32 = mybir.dt.float32
BF16 = mybir.dt.bfloat16
AF = mybir.ActivationFunctionType
ALU = mybir.AluOpType


def bcast_ap(ap: bass.AP, parts: int) -> bass.AP:
    """Partition-broadcast a [1, N]-shaped DRAM AP to [parts, N]."""
    return bass.AP(tensor=ap.tensor, offset=ap.offset,
                   ap=[[0, parts]] + list(ap.ap[-1:]))


def build(T=1024, S=2048, D=1024, H=16, DK=64, n_cores=8, eps=1e-5,
          trn_type="TRN2"):
    assert DK == 64 and H % 2 == 0 and D == H * DK
    DB = D // 128      # contraction chunks over d
    PAIRS = H // 2     # head pairs == 128-row output blocks
    TB = T // 128
    SB = S // 128      # key blocks
    KBP = SB // 2      # key-block pairs per sweep
    NTH = T // 512     # query halves
    VW = 65            # per-head vp stripe: 64 v columns + 1 ones column
    VCH = 2            # pairs per V-projection chunk

    nc = bacc.Bacc(trn_type, target_bir_lowering=False, debug=False,
                   num_devices=n_cores)

    qT = nc.dram_tensor("qT", [D, T], BF16, kind="ExternalInput").ap()
    kT = nc.dram_tensor("kT", [D, S], BF16, kind="ExternalInput").ap()
    vT = nc.dram_tensor("vT", [D, S], BF16, kind="ExternalInput").ap()
    Wq = nc.dram_tensor("Wq", [D, D], BF16, kind="ExternalInput").ap()
    Wk = nc.dram_tensor("Wk", [D, D], BF16, kind="ExternalInput").ap()
    Wv = nc.dram_tensor("Wv", [D, D], BF16, kind="ExternalInput").ap()
    Wfc = nc.dram_tensor("Wfc", [D, D], BF16, kind="ExternalInput").ap()
    bq = nc.dram_tensor("bq", [D], F32, kind="ExternalInput").ap()
    bk = nc.dram_tensor("bk", [D], F32, kind="ExternalInput").ap()
    bv = nc.dram_tensor("bv", [D], F32, kind="ExternalInput").ap()
    bfc = nc.dram_tensor("bfc", [D], F32, kind="ExternalInput").ap()
    gamma = nc.dram_tensor("gamma", [D], F32, kind="ExternalInput").ap()
    beta = nc.dram_tensor("beta", [D], F32, kind="ExternalInput").ap()
    out = nc.dram_tensor("out", [T, D], F32, kind="ExternalOutput").ap()

    qp_dram = nc.dram_tensor("qp_scratch", [T, D], F32).ap()
    den_dram = nc.dram_tensor("den_scratch", [H, T], F32).ap()

    WkR = Wk.rearrange("(db p) n -> p db n", p=128)
    WqR = Wq.rearrange("(db p) n -> p db n", p=128)
    WvR = Wv.rearrange("(db p) n -> p db n", p=128)
    WfcR = Wfc.rearrange("(db p) n -> p db n", p=128)

    with tile.TileContext(nc) as tc, ExitStack() as ctx:
        pconst = ctx.enter_context(tc.tile_pool(name="const", bufs=1))
        ppers = ctx.enter_context(tc.tile_pool(name="persist", bufs=1))
        pkpt = ctx.enter_context(tc.tile_pool(name="kpt", bufs=3))
        pqpt = ctx.enter_context(tc.tile_pool(name="qpt", bufs=3))
        pwfc = ctx.enter_context(tc.tile_pool(name="wfc", bufs=1))

        # ---- tiny constants -------------------------------------------
        bkT = pconst.tile([128, PAIRS], F32, tag="bkT", name="bkT")
        nc.sync.dma_start(out=bkT, in_=bk.rearrange("(e p) -> p e", p=128))
        bvT = pconst.tile([128, PAIRS], F32, tag="bvT", name="bvT")
        nc.sync.dma_start(out=bvT, in_=bv.rearrange("(e p) -> p e", p=128))
        eps_t = pconst.tile([128, 1], F32, tag="eps", name="eps")
        nc.vector.memset(eps_t, eps)
        ident = pconst.tile([128, 128], F32, tag="ident", name="ident")
        masks.make_identity(nc, ident)

        # ---- persistent tiles -----------------------------------------
        vp_sb = [ppers.tile([128, H * VW], BF16, tag=f"vp{s}", name=f"vp{s}")
                 for s in range(SB)]
        ctxT_sb = [ppers.tile([128, T], BF16, tag=f"ctxT{e}", name=f"ctxT{e}")
                   for e in range(PAIRS)]
        wfc_sb = pwfc.tile([128, DB, D], BF16, tag="wfc", name="wfc")

        kpT_t = {}   # pair -> rotating kpT tile [128, S]
        qpT_t = {}   # pair -> rotating qpT tile [128, T]

        with ExitStack() as attn_ctx:
            pkx = attn_ctx.enter_context(tc.tile_pool(name="kx", bufs=1))
            pvx = attn_ctx.enter_context(tc.tile_pool(name="vx", bufs=1))
            pqx = attn_ctx.enter_context(tc.tile_pool(name="qx", bufs=1))
            pwk = attn_ctx.enter_context(tc.tile_pool(name="wk", bufs=2))
            pwq = attn_ctx.enter_context(tc.tile_pool(name="wq", bufs=2))
            pwv = attn_ctx.enter_context(tc.tile_pool(name="wv", bufs=2))
            pbq = attn_ctx.enter_context(tc.tile_pool(name="bq", bufs=2))
            patn = attn_ctx.enter_context(tc.tile_pool(name="attn", bufs=4))
            pstg = attn_ctx.enter_context(tc.tile_pool(name="stg", bufs=3))
            ptmp = attn_ctx.enter_context(tc.tile_pool(name="ctmp", bufs=2))
            prec = attn_ctx.enter_context(tc.tile_pool(name="rec", bufs=1))
            psc = attn_ctx.enter_context(
                tc.tile_pool(name="scps", bufs=2, space="PSUM"))
            pcx = attn_ctx.enter_context(
                tc.tile_pool(name="cxps", bufs=2, space="PSUM"))
            pfil = attn_ctx.enter_context(
                tc.tile_pool(name="filps", bufs=2, space="PSUM"))

            # ---- input staging ----------------------------------------
            # Queue split so pair-0 work is not gated behind bulk loads:
            # sync: wk0 + kT; vector: vT; gpsimd: small weights + qT.
            wk_t = {}
            wq_t = {}
            wv_t = {}
            bq_t = {}

            def load_pair_weights(j):
                wk_t[j] = pwk.tile([128, DB, 128], BF16, tag="wk",
                                   name=f"wk{j}")
                nc.sync.dma_start(out=wk_t[j],
                                  in_=WkR[:, :, j * 128:(j + 1) * 128])
                wq_t[j] = pwq.tile([128, DB, 128], BF16, tag="wq",
                                   name=f"wq{j}")
                nc.gpsimd.dma_start(out=wq_t[j],
                                    in_=WqR[:, :, j * 128:(j + 1) * 128])
                bq_t[j] = pbq.tile([128, 128], F32, tag="bq", name=f"bq{j}")
                nc.gpsimd.dma_start(out=bq_t[j],
                                    in_=bcast_ap(bq[j * 128:(j + 1) * 128], 128))

            def load_vchunk_weights(c):
                wv_t[c] = pwv.tile([128, DB, VCH * 128], BF16, tag="wv",
                                   name=f"wv{c}")
                nc.gpsimd.dma_start(
                    out=wv_t[c],
                    in_=WvR[:, :, c * VCH * 128:(c + 1) * VCH * 128])

            load_pair_weights(0)
            load_vchunk_weights(0)
            kx_sb = [pkx.tile([128, S], BF16, tag=f"kx{d}", name=f"kx{d}")
                     for d in range(DB)]
            vx_sb = [pvx.tile([128, S], BF16, tag=f"vx{d}", name=f"vx{d}")
                     for d in range(DB)]
            qx_sb = [pqx.tile([128, T], BF16, tag=f"qx{d}", name=f"qx{d}")
                     for d in range(DB)]
            # kT loads column-chunk-major so K(0, ci) can start as soon as
            # chunk ci has landed for all d-blocks.
            for ci in range(S // 512):
                for d in range(DB):
                    nc.sync.dma_start(
                        out=kx_sb[d][:, ci * 512:(ci + 1) * 512],
                        in_=kT[d * 128:(d + 1) * 128, ci * 512:(ci + 1) * 512])
            for d in range(DB):
                nc.scalar.dma_start(out=vx_sb[d], in_=vT[d * 128:(d + 1) * 128, :])
            for d in range(DB):
                nc.gpsimd.dma_start(out=qx_sb[d], in_=qT[d * 128:(d + 1) * 128, :])

            # ---- thunk builders (emit one group of work each) ---------
            def k_thunk(j, ci):
                def f():
                    if j not in kpT_t:
                        kpT_t[j] = pkpt.tile([128, S], BF16, tag="kpT",
                                             name=f"kpT{j}")
                    ps = pfil.tile([128, 512], F32, tag="fil", name="kps")
                    for d in range(DB):
                        nc.tensor.matmul(
                            ps, lhsT=wk_t[j][:, d, :],
                            rhs=kx_sb[d][:, ci * 512:(ci + 1) * 512],
                            start=(d == 0), stop=(d == DB - 1))
                    nc.vector.tensor_scalar(
                        out=kpT_t[j][:, ci * 512:(ci + 1) * 512], in0=ps,
                        scalar1=bkT[:, j:j + 1], scalar2=None, op0=ALU.add)
                return f

            def v_thunk(c, s):
                def f():
                    ps = pfil.tile([128, 512], F32, tag="fil", name="vps")
                    psv = ps[:, 0:VCH * 128]
                    for d in range(DB):
                        nc.tensor.matmul(
                            psv, lhsT=vx_sb[d][:, s * 128:(s + 1) * 128],
                            rhs=wv_t[c][:, d, :],
                            start=(d == 0), stop=(d == DB - 1))
                    vr = vp_sb[s].rearrange("p (h c) -> p h c", c=VW)
                    nc.vector.tensor_copy(
                        out=vr[:, 2 * VCH * c:2 * VCH * (c + 1), 0:64],
                        in_=psv.rearrange("p (h c) -> p h c", c=64))
                return f

            def qp_thunk(j, t):
                def f():
                    if j not in qpT_t:
                        qpT_t[j] = pqpt.tile([128, T], BF16, tag="qpT",
                                             name=f"qpT{j}")
                    ps = pfil.tile([128, 512], F32, tag="fil", name="qps")
                    psq = ps[:, 0:128]
                    for d in range(DB):
                        nc.tensor.matmul(
                            psq, lhsT=qx_sb[d][:, t * 128:(t + 1) * 128],
                            rhs=wq_t[j][:, d, :],
                            start=(d == 0), stop=(d == DB - 1))
                    stg = pstg.tile([128, 128], F32, tag="qpn", name="qpn")
                    nc.vector.tensor_add(out=stg, in0=psq, in1=bq_t[j])
                    nc.sync.dma_start(
                        out=qp_dram[t * 128:(t + 1) * 128,
                                    j * 128:(j + 1) * 128],
                        in_=stg)
                    trp = pfil.tile([128, 512], F32, tag="fil", name="trp")
                    nc.tensor.transpose(trp[:, 0:128], stg, ident)
                    nc.vector.tensor_copy(
                        out=qpT_t[j][:, t * 128:(t + 1) * 128],
                        in_=trp[:, 0:128])
                return f

            def interleave(*lists):
                res = []
                n = max(len(x) for x in lists)
                for i in range(n):
                    for x in lists:
                        if i < len(x):
                            res.append(x[i])
                return res

            def pair_fillers(j):
                """Work to interleave into pair j's attention stream."""
                nxt = j + 1
                ks, qs, vs, misc = [], [], [], []
                if nxt < PAIRS:
                    load_pair_weights(nxt)
                    ks = [k_thunk(nxt, ci) for ci in range(S // 512)]
                    qs = [qp_thunk(nxt, t) for t in range(TB)]
                # V chunk c (pairs 2c, 2c+1): half during pair 2c-2, half
                # during pair 2c-1, so filler load is spread evenly.
                c = j // 2 + 1
                if c < PAIRS // VCH:
                    if j % 2 == 0:
                        load_vchunk_weights(c)
                    half = SB // 2
                    s0 = (j % 2) * half
                    vs = [v_thunk(c, s) for s in range(s0, s0 + half)]
                if j == PAIRS - 2:
                    def load_wfc():
                        nc.sync.dma_start(out=wfc_sb, in_=WfcR)
                    misc = [load_wfc]
                return interleave(ks, qs, vs) + misc

            # ================= prologue: pair 0 compute ================
            for ci in range(S // 512):
                k_thunk(0, ci)()
            for s in range(SB):
                v_thunk(0, s)()
            for s in range(SB):
                vr = vp_sb[s].rearrange("p (h c) -> p h c", c=VW)
                nc.vector.memset(vr[:, :, 64:65], 1.0)
            for t in range(TB):
                qp_thunk(0, t)()

            # ================= attention stream ========================
            for j in range(PAIRS):
                fillers = pair_fillers(j)
                rec_den = prec.tile([128, T], F32, tag="rec", name="rec")
                kpt = kpT_t.pop(j)
                qpt = qpT_t.pop(j)
                for th in range(NTH):
                    cx_e = pcx.tile([VW, 512], F32, tag="cx", name="cxe")
                    cx_o = pcx.tile([VW, 512], F32, tag="cx", name="cxo")
                    for kbp in range(KBP):
                        sc_e = psc.tile([128, 1024], F32, tag="sc", name="sce")
                        sc_o = psc.tile([128, 1024], F32, tag="sc", name="sco")
                        for kk in range(2):
                            kb = 2 * kbp + kk
                            for pr0, sc in ((0, sc_e), (64, sc_o)):
                                nc.tensor.matmul(
                                    sc[:, kk * 512:(kk + 1) * 512],
                                    lhsT=kpt[pr0:pr0 + 64,
                                             kb * 128:(kb + 1) * 128],
                                    rhs=qpt[pr0:pr0 + 64,
                                            th * 512:(th + 1) * 512],
                                    start=True, stop=True)
                        at_e = patn.tile([128, 1024], BF16, tag="at",
                                         name="ate")
                        nc.scalar.activation(out=at_e, in_=sc_e, func=AF.Exp,
                                             scale=0.125)
                        at_o = patn.tile([128, 1024], BF16, tag="at",
                                         name="ato")
                        nc.scalar.activation(out=at_o, in_=sc_o, func=AF.Exp,
                                             scale=0.125)
                        st = (kbp == 0)
                        sp = (kbp == KBP - 1)
                        for kk in range(2):
                            kb = 2 * kbp + kk
                            vr = vp_sb[kb].rearrange("p (h c) -> p h c", c=VW)
                            nc.tensor.matmul(
                                cx_e, lhsT=vr[:, 2 * j, :],
                                rhs=at_e[:, kk * 512:(kk + 1) * 512],
                                start=(st and kk == 0), stop=(sp and kk == 1))
                            nc.tensor.matmul(
                                cx_o, lhsT=vr[:, 2 * j + 1, :],
                                rhs=at_o[:, kk * 512:(kk + 1) * 512],
                                start=(st and kk == 0), stop=(sp and kk == 1))
                        for _ in range(2):
                            if fillers:
                                fillers.pop(0)()
                    # ---- sweep epilogue: den + ctx evacuation ---------
                    # Denominators go out to DRAM and come back as a
                    # partition-broadcast read (baseline-proven path).
                    tcol = slice(th * 512, (th + 1) * 512)
                    rr = pstg.tile([65, 512], F32, tag="recrow", name="recrow")
                    nc.vector.tensor_copy(out=rr[64:65, :],
                                          in_=cx_e[64:65, :])
                    nc.sync.dma_start(out=den_dram[2 * j, tcol],
                                      in_=rr[64:65, :])
                    rr2 = pstg.tile([65, 512], F32, tag="recrow",
                                    name="recrow2")
                    nc.vector.tensor_copy(out=rr2[64:65, :],
                                          in_=cx_o[64:65, :])
                    nc.sync.dma_start(out=den_dram[2 * j + 1, tcol],
                                      in_=rr2[64:65, :])
                    nc.vector.tensor_copy(out=ctxT_sb[j][0:64, tcol],
                                          in_=cx_e[0:64, :])
                    tmp = ptmp.tile([64, 512], BF16, tag="ctmp", name="ctmp")
                    nc.vector.tensor_copy(out=tmp, in_=cx_o[0:64, :])
                    nc.sync.dma_start(out=ctxT_sb[j][64:128, tcol], in_=tmp)
                while fillers:
                    fillers.pop(0)()
                # ---- normalize + bias (off critical path) -------------
                nc.gpsimd.dma_start(
                    out=rec_den[0:64, :],
                    in_=bcast_ap(den_dram[2 * j:2 * j + 1, :], 64))
                nc.gpsimd.dma_start(
                    out=rec_den[64:128, :],
                    in_=bcast_ap(den_dram[2 * j + 1:2 * j + 2, :], 64))
                nc.vector.reciprocal_approx_fast(out=rec_den, in_=rec_den)
                nc.vector.tensor_mul(out=ctxT_sb[j], in0=ctxT_sb[j],
                                     in1=rec_den)
                nc.vector.tensor_scalar(out=ctxT_sb[j], in0=ctxT_sb[j],
                                        scalar1=bvT[:, j:j + 1], scalar2=None,
                                        op0=ALU.add)

        # ================= FC + residual + layernorm ====================
        with tc.tile_pool(name="fcps", bufs=2, space="PSUM") as pfc, \
             tc.tile_pool(name="lnbc", bufs=1) as plnb, \
             tc.tile_pool(name="qpl", bufs=2) as pqp, \
             tc.tile_pool(name="xln", bufs=2) as px, \
             tc.tile_pool(name="stat", bufs=4) as pst:
            gamma_bc = plnb.tile([128, D], F32, tag="gamma_bc", name="gamma_bc")
            nc.gpsimd.dma_start(out=gamma_bc, in_=bcast_ap(gamma, 128))
            beta_bc = plnb.tile([128, D], F32, tag="beta_bc", name="beta_bc")
            nc.gpsimd.dma_start(out=beta_bc, in_=bcast_ap(beta, 128))
            bfc_bc = plnb.tile([128, D], F32, tag="bfc_bc", name="bfc_bc")
            nc.gpsimd.dma_start(out=bfc_bc, in_=bcast_ap(bfc, 128))

            for t in range(TB):
                qp_t = pqp.tile([128, D], F32, tag="qp_t", name="qp_t")
                nc.sync.dma_start(out=qp_t,
                                  in_=qp_dram[t * 128:(t + 1) * 128, :])
                # bfc-add is off the fc critical chain: runs as soon as the
                # readback lands, before fc is ready.
                nc.gpsimd.tensor_add(out=qp_t, in0=qp_t, in1=bfc_bc)
                fc = pfc.tile([128, D], F32, tag="fc", name="fc")
                for jj in range(PAIRS):
                    for n0 in range(0, D, 512):
                        nc.tensor.matmul(
                            fc[:, n0:n0 + 512],
                            lhsT=ctxT_sb[jj][:, t * 128:(t + 1) * 128],
                            rhs=wfc_sb[:, jj, n0:n0 + 512],
                            start=(jj == 0), stop=(jj == PAIRS - 1))
                # Alternate the heavy elementwise chain between DVE and
                # Pool per block so neither engine is the FC-phase tail.
                eng = nc.vector if t % 2 == 0 else nc.gpsimd
                alt = nc.gpsimd if t % 2 == 0 else nc.vector
                x = px.tile([128, D], F32, tag="x", name="x")
                # fc is PSUM: GpSimd cannot read it, so this add stays on DVE
                nc.vector.tensor_add(out=x, in0=fc, in1=qp_t)
                ngr = max(D // 512, 1)
                gsz = min(D, 512)
                stats = pst.tile([128, ngr, 6], F32, tag="stats", name="stats")
                for g in range(ngr):
                    nc.vector.bn_stats(out=stats[:, g, :],
                                       in_=x[:, g * gsz:(g + 1) * gsz])
                mv = pst.tile([128, 2], F32, tag="mv", name="mv")
                nc.vector.bn_aggr(out=mv, in_=stats)
                rstd = pst.tile([128, 1], F32, tag="rstd", name="rstd")
                nc.scalar.activation(out=rstd, in_=mv[:, 1:2], func=AF.Sqrt,
                                     bias=eps_t, scale=1.0)
                nc.vector.reciprocal(out=rstd, in_=rstd)
                xn = px.tile([128, D], F32, tag="xn", name="xn")
                eng.tensor_scalar(out=xn, in0=x, scalar1=mv[:, 0:1],
                                  scalar2=rstd, op0=ALU.subtract,
                                  op1=ALU.mult)
                alt.tensor_mul(out=xn, in0=xn, in1=gamma_bc)
                eng.tensor_add(out=xn, in0=xn, in1=beta_bc)
                nc.sync.dma_start(out=out[t * 128:(t + 1) * 128, :], in_=xn)

    nc.compile()
    return nc


_B, _S, _D, _H, _DK = 4, 2048, 1024, 16, 64
_T = _S // 2
_NCORES = 8
_BF = ml_dtypes.bfloat16

_nc_cache = [None]


def _get_nc():
    if _nc_cache[0] is None:
        _nc_cache[0] = build(T=_T, S=_S, D=_D, H=_H, DK=_DK, n_cores=_NCORES)
    return _nc_cache[0]


def _execute(inputs, trace=False):
    from concourse.bass_utils import run_bass_kernel_spmd

    nc = _get_nc()
    q = np.asarray(inputs["q"], np.float32)
    k = np.asarray(inputs["k"], np.float32)
    v = np.asarray(inputs["v"], np.float32)
    Wq = np.asarray(inputs["Wq"], np.float32).astype(_BF)
    Wk = np.asarray(inputs["Wk"], np.float32).astype(_BF)
    Wv = np.asarray(inputs["Wv"], np.float32).astype(_BF)
    Wfc = np.asarray(inputs["Wfc"], np.float32).astype(_BF)
    fp = {n: np.asarray(inputs[n], np.float32)
          for n in ("bq", "bk", "bv", "bfc", "gamma", "beta")}

    in_maps = []
    for c in range(_NCORES):
        b, half = divmod(c, 2)
        t0 = half * _T
        in_maps.append({
            "qT": np.ascontiguousarray(q[b, t0:t0 + _T].T).astype(_BF),
            "kT": np.ascontiguousarray(k[b].T).astype(_BF),
            "vT": np.ascontiguousarray(v[b].T).astype(_BF),
            "Wq": Wq, "Wk": Wk, "Wv": Wv, "Wfc": Wfc, **fp,
        })

    res = run_bass_kernel_spmd(nc, in_maps, core_ids=list(range(_NCORES)),
                               trace=trace)
    out = np.empty((_B, _S, _D), np.float32)
    for c in range(_NCORES):
        b, half = divmod(c, 2)
        out[b, half * _T:(half + 1) * _T] = res.results[c]["out"]
    return out, res.exec_time_ns


def kernel(**inputs) -> np.ndarray:
    out, _ = _execute(inputs, trace=False)
    return out


# revision 38
# speedup vs baseline: 1.1535x; 1.0783x over previous
"""Trainium2 Bass kernel for nn_AttentionLayer (B=4, S=2048, D=1024, H=16).

Self-contained: builds and compiles an SPMD Bass/Tile program once, then
runs it across 8 NeuronCores via run_bass_kernel_spmd.

Sharding (no collectives): core c handles batch b = c // 2 and query-token
half c % 2 (T=1024 query tokens). Each core receives pre-transposed bf16
activations plus bf16 weights, computes its [1024, 1024] slice of the
final layernorm output in fp32, and the host reassembles.

v2 pipeline: one continuous ACT-overlapped stream. The exp stream
(ScalarE) runs near-continuously from ~30us onward; all projection work
(K/V per-pair, Q-natural + PE-transpose to qT-proj) is emitted as filler
thunks inside the attention loop so TensorE slack under the exp stream is
filled. Attention runs per head-pair with query-half sweeps so PSUM fits:
  sc pool 2x[128,1024] (4 banks) + cx 2x[65,512] (2) + K-proj (1) +
  V/qp/transpose fill (1) = 8 banks.
Scores for the two heads of a pair use disjoint PE row groups (contraction
64 at partitions 0-63 / 64-127) and are emitted adjacently so the HW packs
them concurrently. Softmax denominators come from a ones-column in the V
projection; reciprocals are computed in-PSUM and partition-broadcast via
GpSimd (no DRAM roundtrip). Residual q-projection is kept natural (bf16,
DRAM staging) and transposed on the PE for the attention layout.
"""

import numpy as np
import ml_dtypes

from contextlib import ExitStack

import concourse.bass as bass
import concourse.tile as tile
import concourse.mybir as mybir
from concourse import bacc
from concourse import masks

F32 = mybir.dt.float32
BF16 = mybir.dt.bfloat16
AF = mybir.ActivationFunctionType
ALU = mybir.AluOpType


def bcast_ap(ap: bass.AP, parts: int) -> bass.AP:
    """Partition-broadcast a [1, N]-shaped DRAM AP to [parts, N]."""
    return bass.AP(tensor=ap.tensor, offset=ap.offset,
                   ap=[[0, parts]] + list(ap.ap[-1:]))


def build(T=1024, S=2048, D=1024, H=16, DK=64, n_cores=8, eps=1e-5,
          trn_type="TRN2"):
    assert DK == 64 and H % 2 == 0 and D == H * DK
    DB = D // 128      # contraction chunks over d
    PAIRS = H // 2     # head pairs == 128-row output blocks
    TB = T // 128
    SB = S // 128      # key blocks
    KBP = SB // 2      # key-block pairs per sweep
    NTH = T // 512     # query halves
    VW = 65            # per-head vp stripe: 64 v columns + 1 ones column
    VCH = 2            # pairs per V-projection chunk

    nc = bacc.Bacc(trn_type, target_bir_lowering=False, debug=False,
                   num_devices=n_cores)

    qT = nc.dram_tensor("qT", [D, T], BF16, kind="ExternalInput").ap()
    kT = nc.dram_tensor("kT", [D, S], BF16, kind="ExternalInput").ap()
    vT = nc.dram_tensor("vT", [D, S], BF16, kind="ExternalInput").ap()
    Wq = nc.dram_tensor("Wq", [D, D], BF16, kind="ExternalInput").ap()
    Wk = nc.dram_tensor("Wk", [D, D], BF16, kind="ExternalInput").ap()
    Wv = nc.dram_tensor("Wv", [D, D], BF16, kind="ExternalInput").ap()
    Wfc = nc.dram_tensor("Wfc", [D, D], BF16, kind="ExternalInput").ap()
    bq = nc.dram_tensor("bq", [D], F32, kind="ExternalInput").ap()
    bk = nc.dram_tensor("bk", [D], F32, kind="ExternalInput").ap()
    bv = nc.dram_tensor("bv", [D], F32, kind="ExternalInput").ap()
    bfc = nc.dram_tensor("bfc", [D], F32, kind="ExternalInput").ap()
    gamma = nc.dram_tensor("gamma", [D], F32, kind="ExternalInput").ap()
    beta = nc.dram_tensor("beta", [D], F32, kind="ExternalInput").ap()
    out = nc.dram_tensor("out", [T, D], F32, kind="ExternalOutput").ap()

    qp_dram = nc.dram_tensor("qp_scratch", [T, D], F32).ap()
    den_dram = nc.dram_tensor("den_scratch", [H, T], F32).ap()

    WkR = Wk.rearrange("(db p) n -> p db n", p=128)
    WqR = Wq.rearrange("(db p) n -> p db n", p=128)
    WvR = Wv.rearrange("(db p) n -> p db n", p=128)
    WfcR = Wfc.rearrange("(db p) n -> p db n", p=128)

    with tile.TileContext(nc) as tc, ExitStack() as ctx:
        pconst = ctx.enter_context(tc.tile_pool(name="const", bufs=1))
        ppers = ctx.enter_context(tc.tile_pool(name="persist", bufs=1))
        pkpt = ctx.enter_context(tc.tile_pool(name="kpt", bufs=3))
        pqpt = ctx.enter_context(tc.tile_pool(name="qpt", bufs=3))
        pwfc = ctx.enter_context(tc.tile_pool(name="wfc", bufs=1))

        # ---- tiny constants -------------------------------------------
        bkT = pconst.tile([128, PAIRS], F32, tag="bkT", name="bkT")
        nc.sync.dma_start(out=bkT, in_=bk.rearrange("(e p) -> p e", p=128))
        bvT = pconst.tile([128, PAIRS], F32, tag="bvT", name="bvT")
        nc.sync.dma_start(out=bvT, in_=bv.rearrange("(e p) -> p e", p=128))
        eps_t = pconst.tile([128, 1], F32, tag="eps", name="eps")
        nc.vector.memset(eps_t, eps)
        ident = pconst.tile([128, 128], F32, tag="ident", name="ident")
        masks.make_identity(nc, ident)

        # ---- persistent tiles -----------------------------------------
        vp_sb = [ppers.tile([128, H * VW], BF16, tag=f"vp{s}", name=f"vp{s}")
                 for s in range(SB)]
        ctxT_sb = [ppers.tile([128, T], BF16, tag=f"ctxT{e}", name=f"ctxT{e}")
                   for e in range(PAIRS)]
        wfc_sb = pwfc.tile([128, DB, D], BF16, tag="wfc", name="wfc")

        kpT_t = {}   # pair -> rotating kpT tile [128, S]
        qpT_t = {}   # pair -> rotating qpT tile [128, T]

        with ExitStack() as attn_ctx:
            pkx = attn_ctx.enter_context(tc.tile_pool(name="kx", bufs=1))
            pvx = attn_ctx.enter_context(tc.tile_pool(name="vx", bufs=1))
            pqx = attn_ctx.enter_context(tc.tile_pool(name="qx", bufs=1))
            pwk = attn_ctx.enter_context(tc.tile_pool(name="wk", bufs=2))
            pwq = attn_ctx.enter_context(tc.tile_pool(name="wq", bufs=2))
            pwv = attn_ctx.enter_context(tc.tile_pool(name="wv", bufs=2))
            pbq = attn_ctx.enter_context(tc.tile_pool(name="bq", bufs=2))
            patn = attn_ctx.enter_context(tc.tile_pool(name="attn", bufs=6))
            pstg = attn_ctx.enter_context(tc.tile_pool(name="stg", bufs=3))
            ptmp = attn_ctx.enter_context(tc.tile_pool(name="ctmp", bufs=2))
            prec = attn_ctx.enter_context(tc.tile_pool(name="rec", bufs=1))
            psc = attn_ctx.enter_context(
                tc.tile_pool(name="scps", bufs=2, space="PSUM"))
            pcx = attn_ctx.enter_context(
                tc.tile_pool(name="cxps", bufs=2, space="PSUM"))
            pfil = attn_ctx.enter_context(
                tc.tile_pool(name="filps", bufs=2, space="PSUM"))

            # ---- input staging ----------------------------------------
            # Queue split so pair-0 work is not gated behind bulk loads:
            # sync: wk0 + kT; vector: vT; gpsimd: small weights + qT.
            wk_t = {}
            wq_t = {}
            wv_t = {}
            bq_t = {}

            def load_pair_weights(j):
                wk_t[j] = pwk.tile([128, DB, 128], BF16, tag="wk",
                                   name=f"wk{j}")
                wk_eng = nc.gpsimd if j == 0 else nc.sync
                wk_eng.dma_start(out=wk_t[j],
                                 in_=WkR[:, :, j * 128:(j + 1) * 128])
                wq_t[j] = pwq.tile([128, DB, 128], BF16, tag="wq",
                                   name=f"wq{j}")
                nc.gpsimd.dma_start(out=wq_t[j],
                                    in_=WqR[:, :, j * 128:(j + 1) * 128])
                bq_t[j] = pbq.tile([128, 128], F32, tag="bq", name=f"bq{j}")
                nc.gpsimd.dma_start(out=bq_t[j],
                                    in_=bcast_ap(bq[j * 128:(j + 1) * 128], 128))

            def load_vchunk_weights(c):
                wv_t[c] = pwv.tile([128, DB, VCH * 128], BF16, tag="wv",
                                   name=f"wv{c}")
                nc.gpsimd.dma_start(
                    out=wv_t[c],
                    in_=WvR[:, :, c * VCH * 128:(c + 1) * VCH * 128])

            load_pair_weights(0)
            load_vchunk_weights(0)
            kx_sb = [pkx.tile([128, S], BF16, tag=f"kx{d}", name=f"kx{d}")
                     for d in range(DB)]
            vx_sb = [pvx.tile([128, S], BF16, tag=f"vx{d}", name=f"vx{d}")
                     for d in range(DB)]
            qx_sb = [pqx.tile([128, T], BF16, tag=f"qx{d}", name=f"qx{d}")
                     for d in range(DB)]
            # Input loads are column-chunk-major so the pair-0 projections
            # can start on the first chunk instead of the full tensor.
            for ci in range(S // 512):
                for d in range(DB):
                    eng = nc.sync if d < DB // 2 else nc.scalar
                    eng.dma_start(
                        out=kx_sb[d][:, ci * 512:(ci + 1) * 512],
                        in_=kT[d * 128:(d + 1) * 128, ci * 512:(ci + 1) * 512])
            for ci in range(S // 512):
                for d in range(DB):
                    eng = nc.sync if d < DB // 2 else nc.scalar
                    eng.dma_start(
                        out=vx_sb[d][:, ci * 512:(ci + 1) * 512],
                        in_=vT[d * 128:(d + 1) * 128, ci * 512:(ci + 1) * 512])
            for ci in range(T // 512):
                for d in range(DB):
                    nc.gpsimd.dma_start(
                        out=qx_sb[d][:, ci * 512:(ci + 1) * 512],
                        in_=qT[d * 128:(d + 1) * 128, ci * 512:(ci + 1) * 512])

            # ---- thunk builders (emit one group of work each) ---------
            def k_thunk(j, ci):
                def f():
                    if j not in kpT_t:
                        kpT_t[j] = pkpt.tile([128, S], BF16, tag="kpT",
                                             name=f"kpT{j}")
                    ps = pfil.tile([128, 512], F32, tag="fil", name="kps")
                    for d in range(DB):
                        nc.tensor.matmul(
                            ps, lhsT=wk_t[j][:, d, :],
                            rhs=kx_sb[d][:, ci * 512:(ci + 1) * 512],
                            start=(d == 0), stop=(d == DB - 1))
                    nc.vector.tensor_scalar(
                        out=kpT_t[j][:, ci * 512:(ci + 1) * 512], in0=ps,
                        scalar1=bkT[:, j:j + 1], scalar2=None, op0=ALU.add)
                return f

            def v_thunk(c, s):
                def f():
                    ps = pfil.tile([128, 512], F32, tag="fil", name="vps")
                    psv = ps[:, 0:VCH * 128]
                    for d in range(DB):
                        nc.tensor.matmul(
                            psv, lhsT=vx_sb[d][:, s * 128:(s + 1) * 128],
                            rhs=wv_t[c][:, d, :],
                            start=(d == 0), stop=(d == DB - 1))
                    vr = vp_sb[s].rearrange("p (h c) -> p h c", c=VW)
                    nc.vector.tensor_copy(
                        out=vr[:, 2 * VCH * c:2 * VCH * (c + 1), 0:64],
                        in_=psv.rearrange("p (h c) -> p h c", c=64))
                return f

            def qp_thunk(j, t):
                def f():
                    if j not in qpT_t:
                        qpT_t[j] = pqpt.tile([128, T], BF16, tag="qpT",
                                             name=f"qpT{j}")
                    ps = pfil.tile([128, 512], F32, tag="fil", name="qps")
                    psq = ps[:, 0:128]
                    for d in range(DB):
                        nc.tensor.matmul(
                            psq, lhsT=qx_sb[d][:, t * 128:(t + 1) * 128],
                            rhs=wq_t[j][:, d, :],
                            start=(d == 0), stop=(d == DB - 1))
                    stg = pstg.tile([128, 128], F32, tag="qpn", name="qpn")
                    nc.vector.tensor_add(out=stg, in0=psq, in1=bq_t[j])
                    nc.sync.dma_start(
                        out=qp_dram[t * 128:(t + 1) * 128,
                                    j * 128:(j + 1) * 128],
                        in_=stg)
                    trp = pfil.tile([128, 512], F32, tag="fil", name="trp")
                    nc.tensor.transpose(trp[:, 0:128], stg, ident)
                    nc.vector.tensor_copy(
                        out=qpT_t[j][:, t * 128:(t + 1) * 128],
                        in_=trp[:, 0:128])
                return f

            def interleave(*lists):
                res = []
                n = max(len(x) for x in lists)
                for i in range(n):
                    for x in lists:
                        if i < len(x):
                            res.append(x[i])
                return res

            def pair_fillers(j):
                """Work to interleave into pair j's attention stream."""
                nxt = j + 1
                ks, qs, vs, misc = [], [], [], []
                if nxt < PAIRS:
                    load_pair_weights(nxt)
                    ks = [k_thunk(nxt, ci) for ci in range(S // 512)]
                    qs = [qp_thunk(nxt, t) for t in range(TB)]
                # V chunk c (pairs 2c, 2c+1): half during pair 2c-2, half
                # during pair 2c-1, so filler load is spread evenly.
                c = j // 2 + 1
                if c < PAIRS // VCH:
                    if j % 2 == 0:
                        load_vchunk_weights(c)
                    half = SB // 2
                    s0 = (j % 2) * half
                    vs = [v_thunk(c, s) for s in range(s0, s0 + half)]
                if j == PAIRS - 2:
                    def load_wfc():
                        nc.sync.dma_start(out=wfc_sb, in_=WfcR)
                    misc = [load_wfc]
                return interleave(ks, qs, vs) + misc

            # ================= prologue: pair 0 compute ================
            # Interleaved by input chunk so each thunk starts as soon as
            # its DMA slice lands.
            for s in range(SB):
                vr = vp_sb[s].rearrange("p (h c) -> p h c", c=VW)
                nc.vector.memset(vr[:, :, 64:65], 1.0)
            for ci in range(S // 512):
                k_thunk(0, ci)()
                for s in range(4 * ci, 4 * ci + 4):
                    v_thunk(0, s)()
                for t in range(2 * ci, min(2 * ci + 2, TB)):
                    qp_thunk(0, t)()


            # ================= attention stream ========================
            for j in range(PAIRS):
                fillers = pair_fillers(j)
                rec_den = prec.tile([128, T], F32, tag="rec", name="rec")
                kpt = kpT_t.pop(j)
                qpt = qpT_t.pop(j)
                for th in range(NTH):
                    cx_e = pcx.tile([VW, 512], F32, tag="cx", name="cxe")
                    cx_o = pcx.tile([VW, 512], F32, tag="cx", name="cxo")
                    for kbp in range(KBP):
                        sc_e = psc.tile([128, 1024], F32, tag="sc", name="sce")
                        sc_o = psc.tile([128, 1024], F32, tag="sc", name="sco")
                        # High priority keeps the even/odd head score matmuls
                        # adjacent in the scheduled PE stream: they target
                        # disjoint PE row groups (contraction rows 0-63 vs
                        # 64-127), so the HW runs adjacent pairs concurrently.
                        with tc.high_priority():
                            for kk in range(2):
                                kb = 2 * kbp + kk
                                for pr0, sc in ((0, sc_e), (64, sc_o)):
                                    nc.tensor.matmul(
                                        sc[:, kk * 512:(kk + 1) * 512],
                                        lhsT=kpt[pr0:pr0 + 64,
                                                 kb * 128:(kb + 1) * 128],
                                        rhs=qpt[pr0:pr0 + 64,
                                                th * 512:(th + 1) * 512],
                                        start=True, stop=True)
                        at_e = patn.tile([128, 1024], BF16, tag="at",
                                         name="ate")
                        nc.scalar.activation(out=at_e, in_=sc_e, func=AF.Exp,
                                             scale=0.125)
                        at_o = patn.tile([128, 1024], BF16, tag="at",
                                         name="ato")
                        nc.scalar.activation(out=at_o, in_=sc_o, func=AF.Exp,
                                             scale=0.125)
                        st = (kbp == 0)
                        sp = (kbp == KBP - 1)
                        for kk in range(2):
                            kb = 2 * kbp + kk
                            vr = vp_sb[kb].rearrange("p (h c) -> p h c", c=VW)
                            nc.tensor.matmul(
                                cx_e, lhsT=vr[:, 2 * j, :],
                                rhs=at_e[:, kk * 512:(kk + 1) * 512],
                                start=(st and kk == 0), stop=(sp and kk == 1))
                            nc.tensor.matmul(
                                cx_o, lhsT=vr[:, 2 * j + 1, :],
                                rhs=at_o[:, kk * 512:(kk + 1) * 512],
                                start=(st and kk == 0), stop=(sp and kk == 1))
                        for _ in range(2):
                            if fillers:
                                fillers.pop(0)()
                    # ---- sweep epilogue: den + ctx evacuation ---------
                    # Denominators go out to DRAM and come back as a
                    # partition-broadcast read (baseline-proven path).
                    tcol = slice(th * 512, (th + 1) * 512)
                    rr = pstg.tile([65, 512], F32, tag="recrow", name="recrow")
                    nc.vector.tensor_copy(out=rr[64:65, :],
                                          in_=cx_e[64:65, :])
                    # den write + bcast read share the gpsimd queue so FIFO
                    # order guarantees the DRAM RAW dependency
                    nc.gpsimd.dma_start(out=den_dram[2 * j, tcol],
                                        in_=rr[64:65, :])
                    rr2 = pstg.tile([65, 512], F32, tag="recrow",
                                    name="recrow2")
                    nc.vector.tensor_copy(out=rr2[64:65, :],
                                          in_=cx_o[64:65, :])
                    nc.gpsimd.dma_start(out=den_dram[2 * j + 1, tcol],
                                        in_=rr2[64:65, :])
                    nc.vector.tensor_copy(out=ctxT_sb[j][0:64, tcol],
                                          in_=cx_e[0:64, :])
                    tmp = ptmp.tile([64, 512], BF16, tag="ctmp", name="ctmp")
                    nc.vector.tensor_copy(out=tmp, in_=cx_o[0:64, :])
                    nc.sync.dma_start(out=ctxT_sb[j][64:128, tcol], in_=tmp)
                    # ---- normalize + bias for this query half ---------
                    # (overlaps the next sweep; keeps the pair-boundary
                    # and attention->FC bubbles short)
                    nc.gpsimd.dma_start(
                        out=rec_den[0:64, tcol],
                        in_=bcast_ap(den_dram[2 * j:2 * j + 1, tcol], 64))
                    nc.gpsimd.dma_start(
                        out=rec_den[64:128, tcol],
                        in_=bcast_ap(den_dram[2 * j + 1:2 * j + 2, tcol], 64))
                    nc.vector.reciprocal_approx_fast(
                        out=rec_den[:, tcol], in_=rec_den[:, tcol])
                    nc.vector.tensor_mul(out=ctxT_sb[j][:, tcol],
                                         in0=ctxT_sb[j][:, tcol],
                                         in1=rec_den[:, tcol])
                    nc.vector.tensor_scalar(out=ctxT_sb[j][:, tcol],
                                            in0=ctxT_sb[j][:, tcol],
                                            scalar1=bvT[:, j:j + 1],
                                            scalar2=None, op0=ALU.add)
                while fillers:
                    fillers.pop(0)()

        # ================= FC + residual + layernorm ====================
        with tc.tile_pool(name="fcps", bufs=2, space="PSUM") as pfc, \
             tc.tile_pool(name="lnbc", bufs=1) as plnb, \
             tc.tile_pool(name="qpl", bufs=2) as pqp, \
             tc.tile_pool(name="xln", bufs=2) as px, \
             tc.tile_pool(name="stat", bufs=4) as pst:
            gamma_bc = plnb.tile([128, D], F32, tag="gamma_bc", name="gamma_bc")
            nc.gpsimd.dma_start(out=gamma_bc, in_=bcast_ap(gamma, 128))
            beta_bc = plnb.tile([128, D], F32, tag="beta_bc", name="beta_bc")
            nc.gpsimd.dma_start(out=beta_bc, in_=bcast_ap(beta, 128))
            bfc_bc = plnb.tile([128, D], F32, tag="bfc_bc", name="bfc_bc")
            nc.gpsimd.dma_start(out=bfc_bc, in_=bcast_ap(bfc, 128))

            for t in range(TB):
                qp_t = pqp.tile([128, D], F32, tag="qp_t", name="qp_t")
                nc.sync.dma_start(out=qp_t,
                                  in_=qp_dram[t * 128:(t + 1) * 128, :])
                # bfc-add is off the fc critical chain: runs as soon as the
                # readback lands, before fc is ready.
                nc.gpsimd.tensor_add(out=qp_t, in0=qp_t, in1=bfc_bc)
                fc = pfc.tile([128, D], F32, tag="fc", name="fc")
                for jj in range(PAIRS):
                    for n0 in range(0, D, 512):
                        nc.tensor.matmul(
                            fc[:, n0:n0 + 512],
                            lhsT=ctxT_sb[jj][:, t * 128:(t + 1) * 128],
                            rhs=wfc_sb[:, jj, n0:n0 + 512],
                            start=(jj == 0), stop=(jj == PAIRS - 1))
                # Split the elementwise chain between DVE and Pool so
                # neither engine is the FC-phase pace-setter.
                eng = nc.vector if t % 2 == 0 else nc.gpsimd
                x = px.tile([128, D], F32, tag="x", name="x")
                # fc is PSUM: GpSimd cannot read it, so this add stays on DVE
                nc.vector.tensor_add(out=x, in0=fc, in1=qp_t)
                ngr = max(D // 512, 1)
                gsz = min(D, 512)
                stats = pst.tile([128, ngr, 6], F32, tag="stats", name="stats")
                for g in range(ngr):
                    nc.vector.bn_stats(out=stats[:, g, :],
                                       in_=x[:, g * gsz:(g + 1) * gsz])
                mv = pst.tile([128, 2], F32, tag="mv", name="mv")
                nc.vector.bn_aggr(out=mv, in_=stats)
                rstd = pst.tile([128, 1], F32, tag="rstd", name="rstd")
                nc.scalar.activation(out=rstd, in_=mv[:, 1:2], func=AF.Sqrt,
                                     bias=eps_t, scale=1.0)
                nc.vector.reciprocal(out=rstd, in_=rstd)
                xn = px.tile([128, D], F32, tag="xn", name="xn")
                eng.tensor_scalar(out=xn, in0=x, scalar1=mv[:, 0:1],
                                  scalar2=rstd, op0=ALU.subtract,
                                  op1=ALU.mult)
                nc.gpsimd.tensor_mul(out=xn, in0=xn, in1=gamma_bc)
                nc.gpsimd.tensor_add(out=xn, in0=xn, in1=beta_bc)
                nc.sync.dma_start(out=out[t * 128:(t + 1) * 128, :], in_=xn)

    nc.compile()
    return nc


_B, _S, _D, _H, _DK = 4, 2048, 1024, 16, 64
_T = _S // 2
_NCORES = 8
_BF = ml_dtypes.bfloat16

_nc_cache = [None]


def _get_nc():
    if _nc_cache[0] is None:
        _nc_cache[0] = build(T=_T, S=_S, D=_D, H=_H, DK=_DK, n_cores=_NCORES)
    return _nc_cache[0]


def _execute(inputs, trace=False):
    from concourse.bass_utils import run_bass_kernel_spmd

    nc = _get_nc()
    q = np.asarray(inputs["q"], np.float32)
    k = np.asarray(inputs["k"], np.float32)
    v = np.asarray(inputs["v"], np.float32)
    Wq = np.asarray(inputs["Wq"], np.float32).astype(_BF)
    Wk = np.asarray(inputs["Wk"], np.float32).astype(_BF)
    Wv = np.asarray(inputs["Wv"], np.float32).astype(_BF)
    Wfc = np.asarray(inputs["Wfc"], np.float32).astype(_BF)
    fp = {n: np.asarray(inputs[n], np.float32)
          for n in ("bq", "bk", "bv", "bfc", "gamma", "beta")}

    in_maps = []
    for c in range(_NCORES):
        b, half = divmod(c, 2)
        t0 = half * _T
        in_maps.append({
            "qT": np.ascontiguousarray(q[b, t0:t0 + _T].T).astype(_BF),
            "kT": np.ascontiguousarray(k[b].T).astype(_BF),
            "vT": np.ascontiguousarray(v[b].T).astype(_BF),
            "Wq": Wq, "Wk": Wk, "Wv": Wv, "Wfc": Wfc, **fp,
        })

    res = run_bass_kernel_spmd(nc, in_maps, core_ids=list(range(_NCORES)),
                               trace=trace)
    out = np.empty((_B, _S, _D), np.float32)
    for c in range(_NCORES):
        b, half = divmod(c, 2)
        out[b, half * _T:(half + 1) * _T] = res.results[c]["out"]
    return out, res.exec_time_ns


def kernel(**inputs) -> np.ndarray:
    out, _ = _execute(inputs, trace=False)
    return out


# revision 44
# speedup vs baseline: 1.1619x; 1.0073x over previous
"""Trainium2 Bass kernel for nn_AttentionLayer (B=4, S=2048, D=1024, H=16).

Self-contained: builds and compiles an SPMD Bass/Tile program once, then
runs it across 8 NeuronCores via run_bass_kernel_spmd.

Sharding (no collectives): core c handles batch b = c // 2 and query-token
half c % 2 (T=1024 query tokens). Each core receives pre-transposed bf16
activations plus bf16 weights, computes its [1024, 1024] slice of the
final layernorm output in fp32, and the host reassembles.

v2 pipeline: one continuous ACT-overlapped stream. The exp stream
(ScalarE) runs near-continuously from ~30us onward; all projection work
(K/V per-pair, Q-natural + PE-transpose to qT-proj) is emitted as filler
thunks inside the attention loop so TensorE slack under the exp stream is
filled. Attention runs per head-pair with query-half sweeps so PSUM fits:
  sc pool 2x[128,1024] (4 banks) + cx 2x[65,512] (2) + K-proj (1) +
  V/qp/transpose fill (1) = 8 banks.
Scores for the two heads of a pair use disjoint PE row groups (contraction
64 at partitions 0-63 / 64-127) and are emitted adjacently so the HW packs
them concurrently. Softmax denominators come from a ones-column in the V
projection; reciprocals are computed in-PSUM and partition-broadcast via
GpSimd (no DRAM roundtrip). Residual q-projection is kept natural (bf16,
DRAM staging) and transposed on the PE for the attention layout.
"""

import numpy as np
import ml_dtypes

from contextlib import ExitStack

import concourse.bass as bass
import concourse.tile as tile
import concourse.mybir as mybir
from concourse import bacc
from concourse import masks

F32 = mybir.dt.float32
BF16 = mybir.dt.bfloat16
AF = mybir.ActivationFunctionType
ALU = mybir.AluOpType


def bcast_ap(ap: bass.AP, parts: int) -> bass.AP:
    """Partition-broadcast a [1, N]-shaped DRAM AP to [parts, N]."""
    return bass.AP(tensor=ap.tensor, offset=ap.offset,
                   ap=[[0, parts]] + list(ap.ap[-1:]))


def build(T=1024, S=2048, D=1024, H=16, DK=64, n_cores=8, eps=1e-5,
          trn_type="TRN2"):
    assert DK == 64 and H % 2 == 0 and D == H * DK
    DB = D // 128      # contraction chunks over d
    PAIRS = H // 2     # head pairs == 128-row output blocks
    TB = T // 128
    SB = S // 128      # key blocks
    KBP = SB // 2      # key-block pairs per sweep
    NTH = T // 512     # query halves
    VW = 65            # per-head vp stripe: 64 v columns + 1 ones column
    VCH = 2            # pairs per V-projection chunk

    nc = bacc.Bacc(trn_type, target_bir_lowering=False, debug=False,
                   num_devices=n_cores)

    qT = nc.dram_tensor("qT", [D, T], BF16, kind="ExternalInput").ap()
    kT = nc.dram_tensor("kT", [D, S], BF16, kind="ExternalInput").ap()
    vT = nc.dram_tensor("vT", [D, S], BF16, kind="ExternalInput").ap()
    Wq = nc.dram_tensor("Wq", [D, D], BF16, kind="ExternalInput").ap()
    Wk = nc.dram_tensor("Wk", [D, D], BF16, kind="ExternalInput").ap()
    Wv = nc.dram_tensor("Wv", [D, D], BF16, kind="ExternalInput").ap()
    Wfc = nc.dram_tensor("Wfc", [D, D], BF16, kind="ExternalInput").ap()
    bq = nc.dram_tensor("bq", [D], F32, kind="ExternalInput").ap()
    bk = nc.dram_tensor("bk", [D], F32, kind="ExternalInput").ap()
    bv = nc.dram_tensor("bv", [D], F32, kind="ExternalInput").ap()
    bfc = nc.dram_tensor("bfc", [D], F32, kind="ExternalInput").ap()
    gamma = nc.dram_tensor("gamma", [D], F32, kind="ExternalInput").ap()
    beta = nc.dram_tensor("beta", [D], F32, kind="ExternalInput").ap()
    out = nc.dram_tensor("out", [T, D], F32, kind="ExternalOutput").ap()

    qp_dram = nc.dram_tensor("qp_scratch", [T, D], F32).ap()
    den_dram = nc.dram_tensor("den_scratch", [H, T], F32).ap()

    WkR = Wk.rearrange("(db p) n -> p db n", p=128)
    WqR = Wq.rearrange("(db p) n -> p db n", p=128)
    WvR = Wv.rearrange("(db p) n -> p db n", p=128)
    WfcR = Wfc.rearrange("(db p) n -> p db n", p=128)

    with tile.TileContext(nc) as tc, ExitStack() as ctx:
        pconst = ctx.enter_context(tc.tile_pool(name="const", bufs=1))
        ppers = ctx.enter_context(tc.tile_pool(name="persist", bufs=1))
        pkpt = ctx.enter_context(tc.tile_pool(name="kpt", bufs=3))
        pqpt = ctx.enter_context(tc.tile_pool(name="qpt", bufs=3))
        pwfc = ctx.enter_context(tc.tile_pool(name="wfc", bufs=1))

        # ---- tiny constants -------------------------------------------
        bkT = pconst.tile([128, PAIRS], F32, tag="bkT", name="bkT")
        nc.sync.dma_start(out=bkT, in_=bk.rearrange("(e p) -> p e", p=128))
        bvT = pconst.tile([128, PAIRS], F32, tag="bvT", name="bvT")
        nc.sync.dma_start(out=bvT, in_=bv.rearrange("(e p) -> p e", p=128))
        eps_t = pconst.tile([128, 1], F32, tag="eps", name="eps")
        nc.vector.memset(eps_t, eps)
        ident = pconst.tile([128, 128], F32, tag="ident", name="ident")
        masks.make_identity(nc, ident)

        # ---- persistent tiles -----------------------------------------
        vp_sb = [ppers.tile([128, H * VW], BF16, tag=f"vp{s}", name=f"vp{s}")
                 for s in range(SB)]
        ctxT_sb = [ppers.tile([128, T], BF16, tag=f"ctxT{e}", name=f"ctxT{e}")
                   for e in range(PAIRS)]
        wfc_sb = pwfc.tile([128, DB, D], BF16, tag="wfc", name="wfc")

        kpT_t = {}   # pair -> rotating kpT tile [128, S]
        qpT_t = {}   # pair -> rotating qpT tile [128, T]

        with ExitStack() as attn_ctx:
            pkx = attn_ctx.enter_context(tc.tile_pool(name="kx", bufs=1))
            pvx = attn_ctx.enter_context(tc.tile_pool(name="vx", bufs=1))
            pqx = attn_ctx.enter_context(tc.tile_pool(name="qx", bufs=1))
            pwk = attn_ctx.enter_context(tc.tile_pool(name="wk", bufs=2))
            pwq = attn_ctx.enter_context(tc.tile_pool(name="wq", bufs=2))
            pwv = attn_ctx.enter_context(tc.tile_pool(name="wv", bufs=2))
            pbq = attn_ctx.enter_context(tc.tile_pool(name="bq", bufs=2))
            patn = attn_ctx.enter_context(tc.tile_pool(name="attn", bufs=6))
            pstg = attn_ctx.enter_context(tc.tile_pool(name="stg", bufs=3))
            ptmp = attn_ctx.enter_context(tc.tile_pool(name="ctmp", bufs=2))
            prec = attn_ctx.enter_context(tc.tile_pool(name="rec", bufs=1))
            psc = attn_ctx.enter_context(
                tc.tile_pool(name="scps", bufs=2, space="PSUM"))
            pcx = attn_ctx.enter_context(
                tc.tile_pool(name="cxps", bufs=2, space="PSUM"))
            pfil = attn_ctx.enter_context(
                tc.tile_pool(name="filps", bufs=2, space="PSUM"))

            # ---- input staging ----------------------------------------
            # Queue split so pair-0 work is not gated behind bulk loads:
            # sync: wk0 + kT; vector: vT; gpsimd: small weights + qT.
            wk_t = {}
            wq_t = {}
            wv_t = {}
            bq_t = {}

            def load_pair_weights(j):
                wk_t[j] = pwk.tile([128, DB, 128], BF16, tag="wk",
                                   name=f"wk{j}")
                nc.sync.dma_start(out=wk_t[j],
                                  in_=WkR[:, :, j * 128:(j + 1) * 128])
                wq_t[j] = pwq.tile([128, DB, 128], BF16, tag="wq",
                                   name=f"wq{j}")
                nc.gpsimd.dma_start(out=wq_t[j],
                                    in_=WqR[:, :, j * 128:(j + 1) * 128])
                bq_t[j] = pbq.tile([128, 128], F32, tag="bq", name=f"bq{j}")
                nc.gpsimd.dma_start(out=bq_t[j],
                                    in_=bcast_ap(bq[j * 128:(j + 1) * 128], 128))

            def load_vchunk_weights(c):
                wv_t[c] = pwv.tile([128, DB, VCH * 128], BF16, tag="wv",
                                   name=f"wv{c}")
                nc.gpsimd.dma_start(
                    out=wv_t[c],
                    in_=WvR[:, :, c * VCH * 128:(c + 1) * VCH * 128])

            load_pair_weights(0)
            load_vchunk_weights(0)
            kx_sb = [pkx.tile([128, S], BF16, tag=f"kx{d}", name=f"kx{d}")
                     for d in range(DB)]
            vx_sb = [pvx.tile([128, S], BF16, tag=f"vx{d}", name=f"vx{d}")
                     for d in range(DB)]
            qx_sb = [pqx.tile([128, T], BF16, tag=f"qx{d}", name=f"qx{d}")
                     for d in range(DB)]
            # Input loads are column-chunk-major so the pair-0 projections
            # can start on the first chunk instead of the full tensor.
            # Input loads are column-chunk-major so the pair-0 projections
            # can start on the first chunk instead of the full tensor.
            for ci in range(S // 512):
                for d in range(DB):
                    nc.sync.dma_start(
                        out=kx_sb[d][:, ci * 512:(ci + 1) * 512],
                        in_=kT[d * 128:(d + 1) * 128, ci * 512:(ci + 1) * 512])
            for ci in range(S // 512):
                for d in range(DB):
                    nc.scalar.dma_start(
                        out=vx_sb[d][:, ci * 512:(ci + 1) * 512],
                        in_=vT[d * 128:(d + 1) * 128, ci * 512:(ci + 1) * 512])
            for ci in range(T // 512):
                for d in range(DB):
                    nc.gpsimd.dma_start(
                        out=qx_sb[d][:, ci * 512:(ci + 1) * 512],
                        in_=qT[d * 128:(d + 1) * 128, ci * 512:(ci + 1) * 512])

            # ---- thunk builders (emit one group of work each) ---------
            def k_thunk(j, ci):
                def f():
                    if j not in kpT_t:
                        kpT_t[j] = pkpt.tile([128, S], BF16, tag="kpT",
                                             name=f"kpT{j}")
                    ps = pfil.tile([128, 512], F32, tag="fil", name="kps")
                    for d in range(DB):
                        nc.tensor.matmul(
                            ps, lhsT=wk_t[j][:, d, :],
                            rhs=kx_sb[d][:, ci * 512:(ci + 1) * 512],
                            start=(d == 0), stop=(d == DB - 1))
                    nc.vector.tensor_scalar(
                        out=kpT_t[j][:, ci * 512:(ci + 1) * 512], in0=ps,
                        scalar1=bkT[:, j:j + 1], scalar2=None, op0=ALU.add)
                return f

            def v_thunk(c, s):
                def f():
                    ps = pfil.tile([128, 512], F32, tag="fil", name="vps")
                    psv = ps[:, 0:VCH * 128]
                    for d in range(DB):
                        nc.tensor.matmul(
                            psv, lhsT=vx_sb[d][:, s * 128:(s + 1) * 128],
                            rhs=wv_t[c][:, d, :],
                            start=(d == 0), stop=(d == DB - 1))
                    vr = vp_sb[s].rearrange("p (h c) -> p h c", c=VW)
                    nc.vector.tensor_copy(
                        out=vr[:, 2 * VCH * c:2 * VCH * (c + 1), 0:64],
                        in_=psv.rearrange("p (h c) -> p h c", c=64))
                return f

            def qp_thunk(j, t):
                def f():
                    if j not in qpT_t:
                        qpT_t[j] = pqpt.tile([128, T], BF16, tag="qpT",
                                             name=f"qpT{j}")
                    ps = pfil.tile([128, 512], F32, tag="fil", name="qps")
                    psq = ps[:, 0:128]
                    for d in range(DB):
                        nc.tensor.matmul(
                            psq, lhsT=qx_sb[d][:, t * 128:(t + 1) * 128],
                            rhs=wq_t[j][:, d, :],
                            start=(d == 0), stop=(d == DB - 1))
                    stg = pstg.tile([128, 128], F32, tag="qpn", name="qpn")
                    nc.vector.tensor_add(out=stg, in0=psq, in1=bq_t[j])
                    nc.sync.dma_start(
                        out=qp_dram[t * 128:(t + 1) * 128,
                                    j * 128:(j + 1) * 128],
                        in_=stg)
                    trp = pfil.tile([128, 512], F32, tag="fil", name="trp")
                    nc.tensor.transpose(trp[:, 0:128], stg, ident)
                    nc.vector.tensor_copy(
                        out=qpT_t[j][:, t * 128:(t + 1) * 128],
                        in_=trp[:, 0:128])
                return f

            def interleave(*lists):
                res = []
                n = max(len(x) for x in lists)
                for i in range(n):
                    for x in lists:
                        if i < len(x):
                            res.append(x[i])
                return res

            def pair_fillers(j):
                """Work to interleave into pair j's attention stream."""
                nxt = j + 1
                ks, qs, vs, misc = [], [], [], []
                if nxt < PAIRS:
                    load_pair_weights(nxt)
                    ks = [k_thunk(nxt, ci) for ci in range(S // 512)]
                    qs = [qp_thunk(nxt, t) for t in range(TB)]
                # V chunk c (pairs 2c, 2c+1): half during pair 2c-2, half
                # during pair 2c-1, so filler load is spread evenly.
                c = j // 2 + 1
                if c < PAIRS // VCH:
                    if j % 2 == 0:
                        load_vchunk_weights(c)
                    half = SB // 2
                    s0 = (j % 2) * half
                    vs = [v_thunk(c, s) for s in range(s0, s0 + half)]
                if j == PAIRS - 2:
                    def load_wfc():
                        nc.sync.dma_start(out=wfc_sb, in_=WfcR)
                    misc = [load_wfc]
                return interleave(ks, qs, vs) + misc

            # ================= prologue: pair 0 compute ================
            # Interleaved by input chunk so each thunk starts as soon as
            # its DMA slice lands.
            for s in range(SB):
                vr = vp_sb[s].rearrange("p (h c) -> p h c", c=VW)
                nc.vector.memset(vr[:, :, 64:65], 1.0)
            for ci in range(S // 512):
                k_thunk(0, ci)()
                for s in range(4 * ci, 4 * ci + 4):
                    v_thunk(0, s)()
                for t in range(2 * ci, min(2 * ci + 2, TB)):
                    qp_thunk(0, t)()


            # ================= attention stream ========================
            for j in range(PAIRS):
                fillers = pair_fillers(j)
                rec_den = prec.tile([128, T], F32, tag="rec", name="rec")
                kpt = kpT_t.pop(j)
                qpt = qpT_t.pop(j)
                for th in range(NTH):
                    cx_e = pcx.tile([VW, 512], F32, tag="cx", name="cxe")
                    cx_o = pcx.tile([VW, 512], F32, tag="cx", name="cxo")
                    for kbp in range(KBP):
                        sc_e = psc.tile([128, 1024], F32, tag="sc", name="sce")
                        sc_o = psc.tile([128, 1024], F32, tag="sc", name="sco")
                        # High priority keeps the even/odd head score matmuls
                        # adjacent in the scheduled PE stream: they target
                        # disjoint PE row groups (contraction rows 0-63 vs
                        # 64-127), so the HW runs adjacent pairs concurrently.
                        with tc.high_priority():
                            for kk in range(2):
                                kb = 2 * kbp + kk
                                for pr0, sc in ((0, sc_e), (64, sc_o)):
                                    nc.tensor.matmul(
                                        sc[:, kk * 512:(kk + 1) * 512],
                                        lhsT=kpt[pr0:pr0 + 64,
                                                 kb * 128:(kb + 1) * 128],
                                        rhs=qpt[pr0:pr0 + 64,
                                                th * 512:(th + 1) * 512],
                                        start=True, stop=True)
                        at_e = patn.tile([128, 1024], BF16, tag="at",
                                         name="ate")
                        nc.scalar.activation(out=at_e, in_=sc_e, func=AF.Exp,
                                             scale=0.125)
                        at_o = patn.tile([128, 1024], BF16, tag="at",
                                         name="ato")
                        nc.scalar.activation(out=at_o, in_=sc_o, func=AF.Exp,
                                             scale=0.125)
                        st = (kbp == 0)
                        sp = (kbp == KBP - 1)
                        for kk in range(2):
                            kb = 2 * kbp + kk
                            vr = vp_sb[kb].rearrange("p (h c) -> p h c", c=VW)
                            nc.tensor.matmul(
                                cx_e, lhsT=vr[:, 2 * j, :],
                                rhs=at_e[:, kk * 512:(kk + 1) * 512],
                                start=(st and kk == 0), stop=(sp and kk == 1))
                            nc.tensor.matmul(
                                cx_o, lhsT=vr[:, 2 * j + 1, :],
                                rhs=at_o[:, kk * 512:(kk + 1) * 512],
                                start=(st and kk == 0), stop=(sp and kk == 1))
                        npop = 1 if (th == 0 and kbp < 6) else 3
                        for _ in range(npop):
                            if fillers:
                                fillers.pop(0)()
                    # ---- sweep epilogue: den + ctx evacuation ---------
                    # Denominators go out to DRAM and come back as a
                    # partition-broadcast read (baseline-proven path).
                    tcol = slice(th * 512, (th + 1) * 512)
                    rr = pstg.tile([65, 512], F32, tag="recrow", name="recrow")
                    nc.vector.tensor_copy(out=rr[64:65, :],
                                          in_=cx_e[64:65, :])
                    # den write + bcast read share the gpsimd queue so FIFO
                    # order guarantees the DRAM RAW dependency
                    nc.gpsimd.dma_start(out=den_dram[2 * j, tcol],
                                        in_=rr[64:65, :])
                    rr2 = pstg.tile([65, 512], F32, tag="recrow",
                                    name="recrow2")
                    nc.vector.tensor_copy(out=rr2[64:65, :],
                                          in_=cx_o[64:65, :])
                    nc.gpsimd.dma_start(out=den_dram[2 * j + 1, tcol],
                                        in_=rr2[64:65, :])
                    nc.vector.tensor_copy(out=ctxT_sb[j][0:64, tcol],
                                          in_=cx_e[0:64, :])
                    tmp = ptmp.tile([64, 512], BF16, tag="ctmp", name="ctmp")
                    nc.vector.tensor_copy(out=tmp, in_=cx_o[0:64, :])
                    nc.sync.dma_start(out=ctxT_sb[j][64:128, tcol], in_=tmp)
                    # ---- normalize + bias for this query half ---------
                    # (overlaps the next sweep; keeps the pair-boundary
                    # and attention->FC bubbles short)
                    nc.gpsimd.dma_start(
                        out=rec_den[0:64, tcol],
                        in_=bcast_ap(den_dram[2 * j:2 * j + 1, tcol], 64))
                    nc.gpsimd.dma_start(
                        out=rec_den[64:128, tcol],
                        in_=bcast_ap(den_dram[2 * j + 1:2 * j + 2, tcol], 64))
                    nc.vector.reciprocal_approx_fast(
                        out=rec_den[:, tcol], in_=rec_den[:, tcol])
                    nc.vector.tensor_mul(out=ctxT_sb[j][:, tcol],
                                         in0=ctxT_sb[j][:, tcol],
                                         in1=rec_den[:, tcol])
                    nc.vector.tensor_scalar(out=ctxT_sb[j][:, tcol],
                                            in0=ctxT_sb[j][:, tcol],
                                            scalar1=bvT[:, j:j + 1],
                                            scalar2=None, op0=ALU.add)
                while fillers:
                    fillers.pop(0)()

        # ================= FC + residual + layernorm ====================
        with tc.tile_pool(name="fcps", bufs=2, space="PSUM") as pfc, \
             tc.tile_pool(name="lnbc", bufs=1) as plnb, \
             tc.tile_pool(name="qpl", bufs=2) as pqp, \
             tc.tile_pool(name="xln", bufs=2) as px, \
             tc.tile_pool(name="stat", bufs=4) as pst:
            gamma_bc = plnb.tile([128, D], F32, tag="gamma_bc", name="gamma_bc")
            nc.gpsimd.dma_start(out=gamma_bc, in_=bcast_ap(gamma, 128))
            beta_bc = plnb.tile([128, D], F32, tag="beta_bc", name="beta_bc")
            nc.gpsimd.dma_start(out=beta_bc, in_=bcast_ap(beta, 128))
            bfc_bc = plnb.tile([128, D], F32, tag="bfc_bc", name="bfc_bc")
            nc.gpsimd.dma_start(out=bfc_bc, in_=bcast_ap(bfc, 128))

            for t in range(TB):
                qp_t = pqp.tile([128, D], F32, tag="qp_t", name="qp_t")
                nc.sync.dma_start(out=qp_t,
                                  in_=qp_dram[t * 128:(t + 1) * 128, :])
                # bfc-add is off the fc critical chain: runs as soon as the
                # readback lands, before fc is ready.
                nc.gpsimd.tensor_add(out=qp_t, in0=qp_t, in1=bfc_bc)
                fc = pfc.tile([128, D], F32, tag="fc", name="fc")
                for jj in range(PAIRS):
                    for n0 in range(0, D, 512):
                        nc.tensor.matmul(
                            fc[:, n0:n0 + 512],
                            lhsT=ctxT_sb[jj][:, t * 128:(t + 1) * 128],
                            rhs=wfc_sb[:, jj, n0:n0 + 512],
                            start=(jj == 0), stop=(jj == PAIRS - 1))
                # Split the elementwise chain between DVE and Pool so
                # neither engine is the FC-phase pace-setter.
                eng = nc.vector if t % 2 == 0 else nc.gpsimd
                x = px.tile([128, D], F32, tag="x", name="x")
                # fc is PSUM: GpSimd cannot read it, so this add stays on DVE
                nc.vector.tensor_add(out=x, in0=fc, in1=qp_t)
                ngr = max(D // 512, 1)
                gsz = min(D, 512)
                stats = pst.tile([128, ngr, 6], F32, tag="stats", name="stats")
                for g in range(ngr):
                    nc.vector.bn_stats(out=stats[:, g, :],
                                       in_=x[:, g * gsz:(g + 1) * gsz])
                mv = pst.tile([128, 2], F32, tag="mv", name="mv")
                nc.vector.bn_aggr(out=mv, in_=stats)
                rstd = pst.tile([128, 1], F32, tag="rstd", name="rstd")
                nc.scalar.activation(out=rstd, in_=mv[:, 1:2], func=AF.Sqrt,
                                     bias=eps_t, scale=1.0)
                nc.vector.reciprocal(out=rstd, in_=rstd)
                xn = px.tile([128, D], F32, tag="xn", name="xn")
                eng.tensor_scalar(out=xn, in0=x, scalar1=mv[:, 0:1],
                                  scalar2=rstd, op0=ALU.subtract,
                                  op1=ALU.mult)
                nc.gpsimd.tensor_mul(out=xn, in0=xn, in1=gamma_bc)
                nc.gpsimd.tensor_add(out=xn, in0=xn, in1=beta_bc)
                nc.sync.dma_start(out=out[t * 128:(t + 1) * 128, :], in_=xn)

    nc.compile()
    return nc


_B, _S, _D, _H, _DK = 4, 2048, 1024, 16, 64
_T = _S // 2
_NCORES = 8
_BF = ml_dtypes.bfloat16

_nc_cache = [None]


def _get_nc():
    if _nc_cache[0] is None:
        _nc_cache[0] = build(T=_T, S=_S, D=_D, H=_H, DK=_DK, n_cores=_NCORES)
    return _nc_cache[0]


def _execute(inputs, trace=False):
    from concourse.bass_utils import run_bass_kernel_spmd

    nc = _get_nc()
    q = np.asarray(inputs["q"], np.float32)
    k = np.asarray(inputs["k"], np.float32)
    v = np.asarray(inputs["v"], np.float32)
    Wq = np.asarray(inputs["Wq"], np.float32).astype(_BF)
    Wk = np.asarray(inputs["Wk"], np.float32).astype(_BF)
    Wv = np.asarray(inputs["Wv"], np.float32).astype(_BF)
    Wfc = np.asarray(inputs["Wfc"], np.float32).astype(_BF)
    fp = {n: np.asarray(inputs[n], np.float32)
          for n in ("bq", "bk", "bv", "bfc", "gamma", "beta")}

    in_maps = []
    for c in range(_NCORES):
        b, half = divmod(c, 2)
        t0 = half * _T
        in_maps.append({
            "qT": np.ascontiguousarray(q[b, t0:t0 + _T].T).astype(_BF),
            "kT": np.ascontiguousarray(k[b].T).astype(_BF),
            "vT": np.ascontiguousarray(v[b].T).astype(_BF),
            "Wq": Wq, "Wk": Wk, "Wv": Wv, "Wfc": Wfc, **fp,
        })

    res = run_bass_kernel_spmd(nc, in_maps, core_ids=list(range(_NCORES)),
                               trace=trace)
    out = np.empty((_B, _S, _D), np.float32)
    for c in range(_NCORES):
        b, half = divmod(c, 2)
        out[b, half * _T:(half + 1) * _T] = res.results[c]["out"]
    return out, res.exec_time_ns


def kernel(**inputs) -> np.ndarray:
    out, _ = _execute(inputs, trace=False)
    return out


# revision 56
# speedup vs baseline: 1.1644x; 1.0022x over previous
"""Trainium2 Bass kernel for nn_AttentionLayer (B=4, S=2048, D=1024, H=16).

Self-contained: builds and compiles an SPMD Bass/Tile program once, then
runs it across 8 NeuronCores via run_bass_kernel_spmd.

Sharding (no collectives): core c handles batch b = c // 2 and query-token
half c % 2 (T=1024 query tokens). Each core receives pre-transposed bf16
activations plus bf16 weights, computes its [1024, 1024] slice of the
final layernorm output in fp32, and the host reassembles.

v2 pipeline: one continuous exp-overlapped stream. All projection work
(K and Q per-pair, V per-2-pair-chunk) is emitted as filler thunks inside
the attention loop so TensorE slack under the ScalarE exp stream is
filled; only pair 0's projections run up front (column-chunked DMAs so
compute starts on the first chunk). Attention runs per head-pair with
query-half sweeps so PSUM fits:
  sc 2x[128,1024] (4 banks) + cx 2x[65,512] (2) + fill 2x[128,512] (2).
The two heads' score matmuls contract over 64 rows at partitions 0-63 /
64-127 (disjoint PE row groups) and are forced adjacent in the schedule
via high_priority, so the hardware runs each pair concurrently (~2x).
Softmax denominators come from a ones-column in the V projection and take
a DRAM roundtrip for the partition-broadcast (GpSimd cannot touch PSUM,
and partition_broadcast corrupts on HW). The residual q-projection is
computed natural (fp32, DRAM staging for the FC phase) and PE-transposed
into the attention layout, replacing a second full Q projection. FC
matmuls overlap the final attention pair; layernorm alternates DVE/Pool.
"""

import numpy as np
import ml_dtypes

from contextlib import ExitStack

import concourse.bass as bass
import concourse.tile as tile
import concourse.mybir as mybir
from concourse import bacc
from concourse import masks

F32 = mybir.dt.float32
BF16 = mybir.dt.bfloat16
AF = mybir.ActivationFunctionType
ALU = mybir.AluOpType


def bcast_ap(ap: bass.AP, parts: int) -> bass.AP:
    """Partition-broadcast a [1, N]-shaped DRAM AP to [parts, N]."""
    return bass.AP(tensor=ap.tensor, offset=ap.offset,
                   ap=[[0, parts]] + list(ap.ap[-1:]))


def build(T=1024, S=2048, D=1024, H=16, DK=64, n_cores=8, eps=1e-5,
          trn_type="TRN2"):
    assert DK == 64 and H % 2 == 0 and D == H * DK
    DB = D // 128      # contraction chunks over d
    PAIRS = H // 2     # head pairs == 128-row output blocks
    TB = T // 128
    SB = S // 128      # key blocks
    KBP = SB // 2      # key-block pairs per sweep
    NTH = T // 512     # query halves
    VW = 65            # per-head vp stripe: 64 v columns + 1 ones column
    VCH = 2            # pairs per V-projection chunk

    nc = bacc.Bacc(trn_type, target_bir_lowering=False, debug=False,
                   num_devices=n_cores)

    qT = nc.dram_tensor("qT", [D, T], BF16, kind="ExternalInput").ap()
    kT = nc.dram_tensor("kT", [D, S], BF16, kind="ExternalInput").ap()
    vT = nc.dram_tensor("vT", [D, S], BF16, kind="ExternalInput").ap()
    Wq = nc.dram_tensor("Wq", [D, D], BF16, kind="ExternalInput").ap()
    Wk = nc.dram_tensor("Wk", [D, D], BF16, kind="ExternalInput").ap()
    Wv = nc.dram_tensor("Wv", [D, D], BF16, kind="ExternalInput").ap()
    Wfc = nc.dram_tensor("Wfc", [D, D], BF16, kind="ExternalInput").ap()
    bq = nc.dram_tensor("bq", [D], F32, kind="ExternalInput").ap()
    bk = nc.dram_tensor("bk", [D], F32, kind="ExternalInput").ap()
    bv = nc.dram_tensor("bv", [D], F32, kind="ExternalInput").ap()
    bfc = nc.dram_tensor("bfc", [D], F32, kind="ExternalInput").ap()
    gamma = nc.dram_tensor("gamma", [D], F32, kind="ExternalInput").ap()
    beta = nc.dram_tensor("beta", [D], F32, kind="ExternalInput").ap()
    out = nc.dram_tensor("out", [T, D], F32, kind="ExternalOutput").ap()

    qp_dram = nc.dram_tensor("qp_scratch", [T, D], F32).ap()
    den_dram = nc.dram_tensor("den_scratch", [H, T], F32).ap()

    WkR = Wk.rearrange("(db p) n -> p db n", p=128)
    WqR = Wq.rearrange("(db p) n -> p db n", p=128)
    WvR = Wv.rearrange("(db p) n -> p db n", p=128)
    WfcR = Wfc.rearrange("(db p) n -> p db n", p=128)

    with tile.TileContext(nc) as tc, ExitStack() as ctx:
        pconst = ctx.enter_context(tc.tile_pool(name="const", bufs=1))
        ppers = ctx.enter_context(tc.tile_pool(name="persist", bufs=1))
        pkpt = ctx.enter_context(tc.tile_pool(name="kpt", bufs=3))
        pqpt = ctx.enter_context(tc.tile_pool(name="qpt", bufs=3))
        pwfc = ctx.enter_context(tc.tile_pool(name="wfc", bufs=1))

        # ---- tiny constants -------------------------------------------
        bkT = pconst.tile([128, PAIRS], F32, tag="bkT", name="bkT")
        nc.gpsimd.dma_start(out=bkT, in_=bk.rearrange("(e p) -> p e", p=128))
        bvT = pconst.tile([128, PAIRS], F32, tag="bvT", name="bvT")
        nc.gpsimd.dma_start(out=bvT, in_=bv.rearrange("(e p) -> p e", p=128))
        eps_t = pconst.tile([128, 1], F32, tag="eps", name="eps")
        nc.vector.memset(eps_t, eps)
        ident = pconst.tile([128, 128], F32, tag="ident", name="ident")
        masks.make_identity(nc, ident)

        # ---- persistent tiles -----------------------------------------
        vp_sb = [ppers.tile([128, H * VW], BF16, tag=f"vp{s}", name=f"vp{s}")
                 for s in range(SB)]
        ctxT_sb = [ppers.tile([128, T], BF16, tag=f"ctxT{e}", name=f"ctxT{e}")
                   for e in range(PAIRS)]
        wfc_sb = pwfc.tile([128, DB, D], BF16, tag="wfc", name="wfc")

        kpT_t = {}   # pair -> rotating kpT tile [128, S]
        qpT_t = {}   # pair -> rotating qpT tile [128, T]

        with ExitStack() as attn_ctx:
            pkx = attn_ctx.enter_context(tc.tile_pool(name="kx", bufs=1))
            pvx = attn_ctx.enter_context(tc.tile_pool(name="vx", bufs=1))
            pqx = attn_ctx.enter_context(tc.tile_pool(name="qx", bufs=1))
            pwk = attn_ctx.enter_context(tc.tile_pool(name="wk", bufs=2))
            pwq = attn_ctx.enter_context(tc.tile_pool(name="wq", bufs=2))
            pwv = attn_ctx.enter_context(tc.tile_pool(name="wv", bufs=2))
            pbq = attn_ctx.enter_context(tc.tile_pool(name="bq", bufs=2))
            patn = attn_ctx.enter_context(tc.tile_pool(name="attn", bufs=6))
            pstg = attn_ctx.enter_context(tc.tile_pool(name="stg", bufs=3))
            ptmp = attn_ctx.enter_context(tc.tile_pool(name="ctmp", bufs=2))
            prec = attn_ctx.enter_context(tc.tile_pool(name="rec", bufs=1))
            psc = attn_ctx.enter_context(
                tc.tile_pool(name="scps", bufs=2, space="PSUM"))
            pcx = attn_ctx.enter_context(
                tc.tile_pool(name="cxps", bufs=2, space="PSUM"))
            pfil = attn_ctx.enter_context(
                tc.tile_pool(name="filps", bufs=2, space="PSUM"))

            # ---- input staging ----------------------------------------
            # Queue split so pair-0 work is not gated behind bulk loads:
            # sync: wk + kT; scalar: vT; gpsimd: small weights + qT.
            wk_t = {}
            wq_t = {}
            wv_t = {}
            bq_t = {}

            def load_pair_weights(j):
                wk_t[j] = pwk.tile([128, DB, 128], BF16, tag="wk",
                                   name=f"wk{j}")
                nc.sync.dma_start(out=wk_t[j],
                                  in_=WkR[:, :, j * 128:(j + 1) * 128])
                wq_t[j] = pwq.tile([128, DB, 128], BF16, tag="wq",
                                   name=f"wq{j}")
                nc.gpsimd.dma_start(out=wq_t[j],
                                    in_=WqR[:, :, j * 128:(j + 1) * 128])
                bq_t[j] = pbq.tile([128, 128], F32, tag="bq", name=f"bq{j}")
                nc.gpsimd.dma_start(out=bq_t[j],
                                    in_=bcast_ap(bq[j * 128:(j + 1) * 128], 128))

            def load_vchunk_weights(c):
                wv_t[c] = pwv.tile([128, DB, VCH * 128], BF16, tag="wv",
                                   name=f"wv{c}")
                nc.gpsimd.dma_start(
                    out=wv_t[c],
                    in_=WvR[:, :, c * VCH * 128:(c + 1) * VCH * 128])

            load_pair_weights(0)
            load_vchunk_weights(0)
            kx_sb = [pkx.tile([128, S], BF16, tag=f"kx{d}", name=f"kx{d}")
                     for d in range(DB)]
            vx_sb = [pvx.tile([128, S], BF16, tag=f"vx{d}", name=f"vx{d}")
                     for d in range(DB)]
            qx_sb = [pqx.tile([128, T], BF16, tag=f"qx{d}", name=f"qx{d}")
                     for d in range(DB)]
            # Input loads are column-chunk-major so the pair-0 projections
            # can start on the first chunk instead of the full tensor.
            for ci in range(S // 512):
                for d in range(DB):
                    nc.sync.dma_start(
                        out=kx_sb[d][:, ci * 512:(ci + 1) * 512],
                        in_=kT[d * 128:(d + 1) * 128, ci * 512:(ci + 1) * 512])
            for ci in range(S // 512):
                for d in range(DB):
                    nc.scalar.dma_start(
                        out=vx_sb[d][:, ci * 512:(ci + 1) * 512],
                        in_=vT[d * 128:(d + 1) * 128, ci * 512:(ci + 1) * 512])
            for ci in range(T // 512):
                for d in range(DB):
                    nc.gpsimd.dma_start(
                        out=qx_sb[d][:, ci * 512:(ci + 1) * 512],
                        in_=qT[d * 128:(d + 1) * 128, ci * 512:(ci + 1) * 512])

            # ---- thunk builders (emit one group of work each) ---------
            def k_thunk(j, ci):
                def f():
                    if j not in kpT_t:
                        kpT_t[j] = pkpt.tile([128, S], BF16, tag="kpT",
                                             name=f"kpT{j}")
                    ps = pfil.tile([128, 512], F32, tag="fil", name="kps")
                    for d in range(DB):
                        nc.tensor.matmul(
                            ps, lhsT=wk_t[j][:, d, :],
                            rhs=kx_sb[d][:, ci * 512:(ci + 1) * 512],
                            start=(d == 0), stop=(d == DB - 1))
                    nc.vector.tensor_scalar(
                        out=kpT_t[j][:, ci * 512:(ci + 1) * 512], in0=ps,
                        scalar1=bkT[:, j:j + 1], scalar2=None, op0=ALU.add)
                return f

            def v_thunk(c, s):
                def f():
                    ps = pfil.tile([128, 512], F32, tag="fil", name="vps")
                    psv = ps[:, 0:VCH * 128]
                    for d in range(DB):
                        nc.tensor.matmul(
                            psv, lhsT=vx_sb[d][:, s * 128:(s + 1) * 128],
                            rhs=wv_t[c][:, d, :],
                            start=(d == 0), stop=(d == DB - 1))
                    vr = vp_sb[s].rearrange("p (h c) -> p h c", c=VW)
                    nc.vector.tensor_copy(
                        out=vr[:, 2 * VCH * c:2 * VCH * (c + 1), 0:64],
                        in_=psv.rearrange("p (h c) -> p h c", c=64))
                return f

            def qp_thunk(j, t):
                def f():
                    if j not in qpT_t:
                        qpT_t[j] = pqpt.tile([128, T], BF16, tag="qpT",
                                             name=f"qpT{j}")
                    ps = pfil.tile([128, 512], F32, tag="fil", name="qps")
                    psq = ps[:, 0:128]
                    for d in range(DB):
                        nc.tensor.matmul(
                            psq, lhsT=qx_sb[d][:, t * 128:(t + 1) * 128],
                            rhs=wq_t[j][:, d, :],
                            start=(d == 0), stop=(d == DB - 1))
                    stg = pstg.tile([128, 128], F32, tag="qpn", name="qpn")
                    nc.vector.tensor_add(out=stg, in0=psq, in1=bq_t[j])
                    nc.sync.dma_start(
                        out=qp_dram[t * 128:(t + 1) * 128,
                                    j * 128:(j + 1) * 128],
                        in_=stg)
                    trp = pfil.tile([128, 512], F32, tag="fil", name="trp")
                    nc.tensor.transpose(trp[:, 0:128], stg, ident)
                    nc.vector.tensor_copy(
                        out=qpT_t[j][:, t * 128:(t + 1) * 128],
                        in_=trp[:, 0:128])
                return f

            def interleave(*lists):
                res = []
                n = max(len(x) for x in lists)
                for i in range(n):
                    for x in lists:
                        if i < len(x):
                            res.append(x[i])
                return res

            def pair_fillers(j):
                """Work to interleave into pair j's attention stream."""
                nxt = j + 1
                ks, qs, vs, misc = [], [], [], []
                if nxt < PAIRS:
                    load_pair_weights(nxt)
                    ks = [k_thunk(nxt, ci) for ci in range(S // 512)]
                    qs = [qp_thunk(nxt, t) for t in range(TB)]
                # V chunk c (pairs 2c, 2c+1): half during pair 2c-2, half
                # during pair 2c-1, so filler load is spread evenly.
                c = j // 2 + 1
                if c < PAIRS // VCH:
                    if j % 2 == 0:
                        load_vchunk_weights(c)
                    half = SB // 2
                    s0 = (j % 2) * half
                    vs = [v_thunk(c, s) for s in range(s0, s0 + half)]
                if j == PAIRS - 2:
                    def load_wfc():
                        nc.sync.dma_start(out=wfc_sb, in_=WfcR)
                    misc = [load_wfc]
                return interleave(ks, qs, vs) + misc

            # ================= prologue: pair 0 compute ================
            # Interleaved by input chunk so each thunk starts as soon as
            # its DMA slice lands.
            for s in range(SB):
                vr = vp_sb[s].rearrange("p (h c) -> p h c", c=VW)
                nc.vector.memset(vr[:, :, 64:65], 1.0)
            for ci in range(S // 512):
                k_thunk(0, ci)()
                for s in range(4 * ci, 4 * ci + 4):
                    v_thunk(0, s)()
                for t in range(2 * ci, min(2 * ci + 2, TB)):
                    qp_thunk(0, t)()

            # ================= attention stream ========================
            for j in range(PAIRS):
                fillers = pair_fillers(j)
                rec_den = prec.tile([128, T], F32, tag="rec", name="rec")
                kpt = kpT_t.pop(j)
                qpt = qpT_t.pop(j)
                for th in range(NTH):
                    cx_e = pcx.tile([VW, 512], F32, tag="cx", name="cxe")
                    cx_o = pcx.tile([VW, 512], F32, tag="cx", name="cxo")
                    for kbp in range(KBP):
                        sc_e = psc.tile([128, 1024], F32, tag="sc", name="sce")
                        sc_o = psc.tile([128, 1024], F32, tag="sc", name="sco")
                        # High priority keeps the even/odd head score matmuls
                        # adjacent in the scheduled PE stream: they target
                        # disjoint PE row groups (contraction rows 0-63 vs
                        # 64-127), so the HW runs adjacent pairs concurrently.
                        with tc.high_priority():
                            for kk in range(2):
                                kb = 2 * kbp + kk
                                for pr0, sc in ((0, sc_e), (64, sc_o)):
                                    nc.tensor.matmul(
                                        sc[:, kk * 512:(kk + 1) * 512],
                                        lhsT=kpt[pr0:pr0 + 64,
                                                 kb * 128:(kb + 1) * 128],
                                        rhs=qpt[pr0:pr0 + 64,
                                                th * 512:(th + 1) * 512],
                                        start=True, stop=True)
                        at_e = patn.tile([128, 1024], BF16, tag="at",
                                         name="ate")
                        nc.scalar.activation(out=at_e, in_=sc_e, func=AF.Exp,
                                             scale=0.125)
                        at_o = patn.tile([128, 1024], BF16, tag="at",
                                         name="ato")
                        nc.scalar.activation(out=at_o, in_=sc_o, func=AF.Exp,
                                             scale=0.125)
                        st = (kbp == 0)
                        sp = (kbp == KBP - 1)
                        for kk in range(2):
                            kb = 2 * kbp + kk
                            vr = vp_sb[kb].rearrange("p (h c) -> p h c", c=VW)
                            nc.tensor.matmul(
                                cx_e, lhsT=vr[:, 2 * j, :],
                                rhs=at_e[:, kk * 512:(kk + 1) * 512],
                                start=(st and kk == 0), stop=(sp and kk == 1))
                            nc.tensor.matmul(
                                cx_o, lhsT=vr[:, 2 * j + 1, :],
                                rhs=at_o[:, kk * 512:(kk + 1) * 512],
                                start=(st and kk == 0), stop=(sp and kk == 1))
                        npop = 1 if (th == 0 and kbp < 6) else 3
                        for _ in range(npop):
                            if fillers:
                                fillers.pop(0)()
                    # ---- sweep epilogue: den + ctx evacuation ---------
                    # Denominators go out to DRAM and come back as a
                    # partition-broadcast read (baseline-proven path).
                    tcol = slice(th * 512, (th + 1) * 512)
                    rr = pstg.tile([65, 512], F32, tag="recrow", name="recrow")
                    nc.vector.tensor_copy(out=rr[64:65, :],
                                          in_=cx_e[64:65, :])
                    # den write + bcast read share the gpsimd queue so FIFO
                    # order guarantees the DRAM RAW dependency
                    nc.gpsimd.dma_start(out=den_dram[2 * j, tcol],
                                        in_=rr[64:65, :])
                    rr2 = pstg.tile([65, 512], F32, tag="recrow",
                                    name="recrow2")
                    nc.vector.tensor_copy(out=rr2[64:65, :],
                                          in_=cx_o[64:65, :])
                    nc.gpsimd.dma_start(out=den_dram[2 * j + 1, tcol],
                                        in_=rr2[64:65, :])
                    nc.vector.tensor_copy(out=ctxT_sb[j][0:64, tcol],
                                          in_=cx_e[0:64, :])
                    tmp = ptmp.tile([64, 512], BF16, tag="ctmp", name="ctmp")
                    nc.vector.tensor_copy(out=tmp, in_=cx_o[0:64, :])
                    nc.sync.dma_start(out=ctxT_sb[j][64:128, tcol], in_=tmp)
                    # ---- normalize + bias for this query half ---------
                    # (overlaps the next sweep; keeps the pair-boundary
                    # and attention->FC bubbles short)
                    nc.gpsimd.dma_start(
                        out=rec_den[0:64, tcol],
                        in_=bcast_ap(den_dram[2 * j:2 * j + 1, tcol], 64))
                    nc.gpsimd.dma_start(
                        out=rec_den[64:128, tcol],
                        in_=bcast_ap(den_dram[2 * j + 1:2 * j + 2, tcol], 64))
                    nc.vector.reciprocal_approx_fast(
                        out=rec_den[:, tcol], in_=rec_den[:, tcol])
                    nc.vector.tensor_mul(out=ctxT_sb[j][:, tcol],
                                         in0=ctxT_sb[j][:, tcol],
                                         in1=rec_den[:, tcol])
                    nc.vector.tensor_scalar(out=ctxT_sb[j][:, tcol],
                                            in0=ctxT_sb[j][:, tcol],
                                            scalar1=bvT[:, j:j + 1],
                                            scalar2=None, op0=ALU.add)
                while fillers:
                    fillers.pop(0)()

        # ================= FC + residual + layernorm ====================
        with tc.tile_pool(name="fcps", bufs=2, space="PSUM") as pfc, \
             tc.tile_pool(name="lnbc", bufs=1) as plnb, \
             tc.tile_pool(name="qpl", bufs=2) as pqp, \
             tc.tile_pool(name="xln", bufs=2) as px, \
             tc.tile_pool(name="stat", bufs=4) as pst:
            gamma_bc = plnb.tile([128, D], F32, tag="gamma_bc", name="gamma_bc")
            nc.gpsimd.dma_start(out=gamma_bc, in_=bcast_ap(gamma, 128))
            beta_bc = plnb.tile([128, D], F32, tag="beta_bc", name="beta_bc")
            nc.gpsimd.dma_start(out=beta_bc, in_=bcast_ap(beta, 128))
            bfc_bc = plnb.tile([128, D], F32, tag="bfc_bc", name="bfc_bc")
            nc.gpsimd.dma_start(out=bfc_bc, in_=bcast_ap(bfc, 128))

            for t in range(TB):
                qp_t = pqp.tile([128, D], F32, tag="qp_t", name="qp_t")
                nc.sync.dma_start(out=qp_t,
                                  in_=qp_dram[t * 128:(t + 1) * 128, :])
                # bfc-add is off the fc critical chain: runs as soon as the
                # readback lands, before fc is ready.
                nc.gpsimd.tensor_add(out=qp_t, in0=qp_t, in1=bfc_bc)
                fc = pfc.tile([128, D], F32, tag="fc", name="fc")
                for jj in range(PAIRS):
                    for n0 in range(0, D, 512):
                        nc.tensor.matmul(
                            fc[:, n0:n0 + 512],
                            lhsT=ctxT_sb[jj][:, t * 128:(t + 1) * 128],
                            rhs=wfc_sb[:, jj, n0:n0 + 512],
                            start=(jj == 0), stop=(jj == PAIRS - 1))
                x = px.tile([128, D], F32, tag="x", name="x")
                # fc is PSUM: GpSimd cannot read it, so this add stays on DVE
                nc.vector.tensor_add(out=x, in0=fc, in1=qp_t)
                ngr = max(D // 512, 1)
                gsz = min(D, 512)
                stats = pst.tile([128, ngr, 6], F32, tag="stats", name="stats")
                for g in range(ngr):
                    nc.vector.bn_stats(out=stats[:, g, :],
                                       in_=x[:, g * gsz:(g + 1) * gsz])
                mv = pst.tile([128, 2], F32, tag="mv", name="mv")
                nc.vector.bn_aggr(out=mv, in_=stats)
                rstd = pst.tile([128, 1], F32, tag="rstd", name="rstd")
                nc.scalar.activation(out=rstd, in_=mv[:, 1:2], func=AF.Sqrt,
                                     bias=eps_t, scale=1.0)
                nc.vector.reciprocal(out=rstd, in_=rstd)
                xn = px.tile([128, D], F32, tag="xn", name="xn")
                eng = nc.vector if t % 2 == 0 else nc.gpsimd
                eng.tensor_scalar(out=xn, in0=x, scalar1=mv[:, 0:1],
                                  scalar2=rstd, op0=ALU.subtract,
                                  op1=ALU.mult)
                nc.gpsimd.tensor_mul(out=xn, in0=xn, in1=gamma_bc)
                nc.gpsimd.tensor_add(out=xn, in0=xn, in1=beta_bc)
                out_eng = nc.sync if t % 2 == 0 else nc.scalar
                out_eng.dma_start(out=out[t * 128:(t + 1) * 128, :], in_=xn)

    nc.compile()
    return nc


_B, _S, _D, _H, _DK = 4, 2048, 1024, 16, 64
_T = _S // 2
_NCORES = 8
_BF = ml_dtypes.bfloat16

_nc_cache = [None]


def _get_nc():
    if _nc_cache[0] is None:
        _nc_cache[0] = build(T=_T, S=_S, D=_D, H=_H, DK=_DK, n_cores=_NCORES)
    return _nc_cache[0]


def _execute(inputs, trace=False):
    from concourse.bass_utils import run_bass_kernel_spmd

    nc = _get_nc()
    q = np.asarray(inputs["q"], np.float32)
    k = np.asarray(inputs["k"], np.float32)
    v = np.asarray(inputs["v"], np.float32)
    Wq = np.asarray(inputs["Wq"], np.float32).astype(_BF)
    Wk = np.asarray(inputs["Wk"], np.float32).astype(_BF)
    Wv = np.asarray(inputs["Wv"], np.float32).astype(_BF)
    Wfc = np.asarray(inputs["Wfc"], np.float32).astype(_BF)
    fp = {n: np.asarray(inputs[n], np.float32)
          for n in ("bq", "bk", "bv", "bfc", "gamma", "beta")}

    in_maps = []
    for c in range(_NCORES):
        b, half = divmod(c, 2)
        t0 = half * _T
        in_maps.append({
            "qT": np.ascontiguousarray(q[b, t0:t0 + _T].T).astype(_BF),
            "kT": np.ascontiguousarray(k[b].T).astype(_BF),
            "vT": np.ascontiguousarray(v[b].T).astype(_BF),
            "Wq": Wq, "Wk": Wk, "Wv": Wv, "Wfc": Wfc, **fp,
        })

    res = run_bass_kernel_spmd(nc, in_maps, core_ids=list(range(_NCORES)),
                               trace=trace)
    out = np.empty((_B, _S, _D), np.float32)
    for c in range(_NCORES):
        b, half = divmod(c, 2)
        out[b, half * _T:(half + 1) * _T] = res.results[c]["out"]
    return out, res.exec_time_ns


def kernel(**inputs) -> np.ndarray:
    out, _ = _execute(inputs, trace=False)
    return out



# revision 57
# speedup vs baseline: 1.1678x; 1.0030x over previous
"""Trainium2 Bass kernel for nn_AttentionLayer (B=4, S=2048, D=1024, H=16).

Self-contained: builds and compiles an SPMD Bass/Tile program once, then
runs it across 8 NeuronCores via run_bass_kernel_spmd.

Sharding (no collectives): core c handles batch b = c // 2 and query-token
half c % 2 (T=1024 query tokens). Each core receives pre-transposed bf16
activations plus bf16 weights, computes its [1024, 1024] slice of the
final layernorm output in fp32, and the host reassembles.

v2 pipeline: one continuous exp-overlapped stream. All projection work
(K and Q per-pair, V per-2-pair-chunk) is emitted as filler thunks inside
the attention loop so TensorE slack under the ScalarE exp stream is
filled; only pair 0's projections run up front (column-chunked DMAs so
compute starts on the first chunk). Attention runs per head-pair with
query-half sweeps so PSUM fits:
  sc 2x[128,1024] (4 banks) + cx 2x[65,512] (2) + fill 2x[128,512] (2).
The two heads' score matmuls contract over 64 rows at partitions 0-63 /
64-127 (disjoint PE row groups) and are forced adjacent in the schedule
via high_priority, so the hardware runs each pair concurrently (~2x).
Softmax denominators come from a ones-column in the V projection and take
a DRAM roundtrip for the partition-broadcast (GpSimd cannot touch PSUM,
and partition_broadcast corrupts on HW). The residual q-projection is
computed natural (fp32, DRAM staging for the FC phase) and PE-transposed
into the attention layout, replacing a second full Q projection. FC
matmuls overlap the final attention pair; layernorm alternates DVE/Pool.
"""

import numpy as np
import ml_dtypes

from contextlib import ExitStack

import concourse.bass as bass
import concourse.tile as tile
import concourse.mybir as mybir
from concourse import bacc
from concourse import masks

F32 = mybir.dt.float32
BF16 = mybir.dt.bfloat16
AF = mybir.ActivationFunctionType
ALU = mybir.AluOpType


def bcast_ap(ap: bass.AP, parts: int) -> bass.AP:
    """Partition-broadcast a [1, N]-shaped DRAM AP to [parts, N]."""
    return bass.AP(tensor=ap.tensor, offset=ap.offset,
                   ap=[[0, parts]] + list(ap.ap[-1:]))


def build(T=1024, S=2048, D=1024, H=16, DK=64, n_cores=8, eps=1e-5,
          trn_type="TRN2"):
    assert DK == 64 and H % 2 == 0 and D == H * DK
    DB = D // 128      # contraction chunks over d
    PAIRS = H // 2     # head pairs == 128-row output blocks
    TB = T // 128
    SB = S // 128      # key blocks
    KBP = SB // 2      # key-block pairs per sweep
    NTH = T // 512     # query halves
    VW = 65            # per-head vp stripe: 64 v columns + 1 ones column
    VCH = 2            # pairs per V-projection chunk

    nc = bacc.Bacc(trn_type, target_bir_lowering=False, debug=False,
                   num_devices=n_cores)

    qT = nc.dram_tensor("qT", [D, T], BF16, kind="ExternalInput").ap()
    kT = nc.dram_tensor("kT", [D, S], BF16, kind="ExternalInput").ap()
    vT = nc.dram_tensor("vT", [D, S], BF16, kind="ExternalInput").ap()
    Wq = nc.dram_tensor("Wq", [D, D], BF16, kind="ExternalInput").ap()
    Wk = nc.dram_tensor("Wk", [D, D], BF16, kind="ExternalInput").ap()
    Wv = nc.dram_tensor("Wv", [D, D], BF16, kind="ExternalInput").ap()
    Wfc = nc.dram_tensor("Wfc", [D, D], BF16, kind="ExternalInput").ap()
    bq = nc.dram_tensor("bq", [D], F32, kind="ExternalInput").ap()
    bk = nc.dram_tensor("bk", [D], F32, kind="ExternalInput").ap()
    bv = nc.dram_tensor("bv", [D], F32, kind="ExternalInput").ap()
    bfc = nc.dram_tensor("bfc", [D], F32, kind="ExternalInput").ap()
    gamma = nc.dram_tensor("gamma", [D], F32, kind="ExternalInput").ap()
    beta = nc.dram_tensor("beta", [D], F32, kind="ExternalInput").ap()
    out = nc.dram_tensor("out", [T, D], F32, kind="ExternalOutput").ap()

    qp_dram = nc.dram_tensor("qp_scratch", [T, D], F32).ap()
    den_dram = nc.dram_tensor("den_scratch", [H, T], F32).ap()

    WkR = Wk.rearrange("(db p) n -> p db n", p=128)
    WqR = Wq.rearrange("(db p) n -> p db n", p=128)
    WvR = Wv.rearrange("(db p) n -> p db n", p=128)
    WfcR = Wfc.rearrange("(db p) n -> p db n", p=128)

    with tile.TileContext(nc) as tc, ExitStack() as ctx:
        pconst = ctx.enter_context(tc.tile_pool(name="const", bufs=1))
        ppers = ctx.enter_context(tc.tile_pool(name="persist", bufs=1))
        pkpt = ctx.enter_context(tc.tile_pool(name="kpt", bufs=2))
        pqpt = ctx.enter_context(tc.tile_pool(name="qpt", bufs=3))
        pwfc = ctx.enter_context(tc.tile_pool(name="wfc", bufs=1))

        # ---- tiny constants -------------------------------------------
        bkT = pconst.tile([128, PAIRS], F32, tag="bkT", name="bkT")
        nc.gpsimd.dma_start(out=bkT, in_=bk.rearrange("(e p) -> p e", p=128))
        bvT = pconst.tile([128, PAIRS], F32, tag="bvT", name="bvT")
        nc.gpsimd.dma_start(out=bvT, in_=bv.rearrange("(e p) -> p e", p=128))
        eps_t = pconst.tile([128, 1], F32, tag="eps", name="eps")
        nc.vector.memset(eps_t, eps)
        ident = pconst.tile([128, 128], F32, tag="ident", name="ident")
        masks.make_identity(nc, ident)

        # ---- persistent tiles -----------------------------------------
        vp_sb = [ppers.tile([128, H * VW], BF16, tag=f"vp{s}", name=f"vp{s}")
                 for s in range(SB)]
        ctxT_sb = [ppers.tile([128, T], BF16, tag=f"ctxT{e}", name=f"ctxT{e}")
                   for e in range(PAIRS)]
        wfc_sb = pwfc.tile([128, DB, D], BF16, tag="wfc", name="wfc")

        kpT_t = {}   # pair -> rotating kpT tile [128, S]
        qpT_t = {}   # pair -> rotating qpT tile [128, T]

        with ExitStack() as attn_ctx:
            pkx = attn_ctx.enter_context(tc.tile_pool(name="kx", bufs=1))
            pvx = attn_ctx.enter_context(tc.tile_pool(name="vx", bufs=1))
            pqx = attn_ctx.enter_context(tc.tile_pool(name="qx", bufs=1))
            pwk = attn_ctx.enter_context(tc.tile_pool(name="wk", bufs=2))
            pwq = attn_ctx.enter_context(tc.tile_pool(name="wq", bufs=2))
            pwv = attn_ctx.enter_context(tc.tile_pool(name="wv", bufs=2))
            pbq = attn_ctx.enter_context(tc.tile_pool(name="bq", bufs=2))
            patn = attn_ctx.enter_context(tc.tile_pool(name="attn", bufs=8))
            pstg = attn_ctx.enter_context(tc.tile_pool(name="stg", bufs=3))
            ptmp = attn_ctx.enter_context(tc.tile_pool(name="ctmp", bufs=2))
            prec = attn_ctx.enter_context(tc.tile_pool(name="rec", bufs=1))
            psc = attn_ctx.enter_context(
                tc.tile_pool(name="scps", bufs=2, space="PSUM"))
            pcx = attn_ctx.enter_context(
                tc.tile_pool(name="cxps", bufs=2, space="PSUM"))
            pfil = attn_ctx.enter_context(
                tc.tile_pool(name="filps", bufs=2, space="PSUM"))

            # ---- input staging ----------------------------------------
            # Queue split so pair-0 work is not gated behind bulk loads:
            # sync: wk + kT; scalar: vT; gpsimd: small weights + qT.
            wk_t = {}
            wq_t = {}
            wv_t = {}
            bq_t = {}

            def load_pair_weights(j):
                wk_t[j] = pwk.tile([128, DB, 128], BF16, tag="wk",
                                   name=f"wk{j}")
                nc.sync.dma_start(out=wk_t[j],
                                  in_=WkR[:, :, j * 128:(j + 1) * 128])
                wq_t[j] = pwq.tile([128, DB, 128], BF16, tag="wq",
                                   name=f"wq{j}")
                nc.gpsimd.dma_start(out=wq_t[j],
                                    in_=WqR[:, :, j * 128:(j + 1) * 128])
                bq_t[j] = pbq.tile([128, 128], F32, tag="bq", name=f"bq{j}")
                nc.gpsimd.dma_start(out=bq_t[j],
                                    in_=bcast_ap(bq[j * 128:(j + 1) * 128], 128))

            def load_vchunk_weights(c):
                wv_t[c] = pwv.tile([128, DB, VCH * 128], BF16, tag="wv",
                                   name=f"wv{c}")
                nc.gpsimd.dma_start(
                    out=wv_t[c],
                    in_=WvR[:, :, c * VCH * 128:(c + 1) * VCH * 128])

            load_pair_weights(0)
            load_vchunk_weights(0)
            kx_sb = [pkx.tile([128, S], BF16, tag=f"kx{d}", name=f"kx{d}")
                     for d in range(DB)]
            vx_sb = [pvx.tile([128, S], BF16, tag=f"vx{d}", name=f"vx{d}")
                     for d in range(DB)]
            qx_sb = [pqx.tile([128, T], BF16, tag=f"qx{d}", name=f"qx{d}")
                     for d in range(DB)]
            # Input loads are column-chunk-major so the pair-0 projections
            # can start on the first chunk instead of the full tensor.
            for ci in range(S // 512):
                for d in range(DB):
                    nc.sync.dma_start(
                        out=kx_sb[d][:, ci * 512:(ci + 1) * 512],
                        in_=kT[d * 128:(d + 1) * 128, ci * 512:(ci + 1) * 512])
            for ci in range(S // 512):
                for d in range(DB):
                    nc.scalar.dma_start(
                        out=vx_sb[d][:, ci * 512:(ci + 1) * 512],
                        in_=vT[d * 128:(d + 1) * 128, ci * 512:(ci + 1) * 512])
            for ci in range(T // 512):
                for d in range(DB):
                    nc.gpsimd.dma_start(
                        out=qx_sb[d][:, ci * 512:(ci + 1) * 512],
                        in_=qT[d * 128:(d + 1) * 128, ci * 512:(ci + 1) * 512])

            # ---- thunk builders (emit one group of work each) ---------
            def k_thunk(j, ci):
                def f():
                    if j not in kpT_t:
                        kpT_t[j] = pkpt.tile([128, S], BF16, tag="kpT",
                                             name=f"kpT{j}")
                    ps = pfil.tile([128, 512], F32, tag="fil", name="kps")
                    for d in range(DB):
                        nc.tensor.matmul(
                            ps, lhsT=wk_t[j][:, d, :],
                            rhs=kx_sb[d][:, ci * 512:(ci + 1) * 512],
                            start=(d == 0), stop=(d == DB - 1))
                    nc.vector.tensor_scalar(
                        out=kpT_t[j][:, ci * 512:(ci + 1) * 512], in0=ps,
                        scalar1=bkT[:, j:j + 1], scalar2=None, op0=ALU.add)
                return f

            def v_thunk(c, s):
                def f():
                    ps = pfil.tile([128, 512], F32, tag="fil", name="vps")
                    psv = ps[:, 0:VCH * 128]
                    for d in range(DB):
                        nc.tensor.matmul(
                            psv, lhsT=vx_sb[d][:, s * 128:(s + 1) * 128],
                            rhs=wv_t[c][:, d, :],
                            start=(d == 0), stop=(d == DB - 1))
                    vr = vp_sb[s].rearrange("p (h c) -> p h c", c=VW)
                    nc.vector.tensor_copy(
                        out=vr[:, 2 * VCH * c:2 * VCH * (c + 1), 0:64],
                        in_=psv.rearrange("p (h c) -> p h c", c=64))
                return f

            def qp_thunk(j, t):
                def f():
                    if j not in qpT_t:
                        qpT_t[j] = pqpt.tile([128, T], BF16, tag="qpT",
                                             name=f"qpT{j}")
                    ps = pfil.tile([128, 512], F32, tag="fil", name="qps")
                    psq = ps[:, 0:128]
                    for d in range(DB):
                        nc.tensor.matmul(
                            psq, lhsT=qx_sb[d][:, t * 128:(t + 1) * 128],
                            rhs=wq_t[j][:, d, :],
                            start=(d == 0), stop=(d == DB - 1))
                    stg = pstg.tile([128, 128], F32, tag="qpn", name="qpn")
                    nc.vector.tensor_add(out=stg, in0=psq, in1=bq_t[j])
                    nc.sync.dma_start(
                        out=qp_dram[t * 128:(t + 1) * 128,
                                    j * 128:(j + 1) * 128],
                        in_=stg)
                    trp = pfil.tile([128, 512], F32, tag="fil", name="trp")
                    nc.tensor.transpose(trp[:, 0:128], stg, ident)
                    nc.vector.tensor_copy(
                        out=qpT_t[j][:, t * 128:(t + 1) * 128],
                        in_=trp[:, 0:128])
                return f

            def interleave(*lists):
                res = []
                n = max(len(x) for x in lists)
                for i in range(n):
                    for x in lists:
                        if i < len(x):
                            res.append(x[i])
                return res

            def pair_fillers(j):
                """Work to interleave into pair j's attention stream."""
                nxt = j + 1
                ks, qs, vs, misc = [], [], [], []
                if nxt < PAIRS:
                    load_pair_weights(nxt)
                    ks = [k_thunk(nxt, ci) for ci in range(S // 512)]
                    qs = [qp_thunk(nxt, t) for t in range(TB)]
                # V chunk c (pairs 2c, 2c+1): half during pair 2c-2, half
                # during pair 2c-1, so filler load is spread evenly.
                c = j // 2 + 1
                if c < PAIRS // VCH:
                    if j % 2 == 0:
                        load_vchunk_weights(c)
                    half = SB // 2
                    s0 = (j % 2) * half
                    vs = [v_thunk(c, s) for s in range(s0, s0 + half)]
                if j == PAIRS - 2:
                    def load_wfc():
                        nc.sync.dma_start(out=wfc_sb, in_=WfcR)
                    misc = [load_wfc]
                return interleave(ks, qs, vs) + misc

            # ================= prologue: pair 0 compute ================
            # Interleaved by input chunk so each thunk starts as soon as
            # its DMA slice lands.
            for s in range(SB):
                vr = vp_sb[s].rearrange("p (h c) -> p h c", c=VW)
                nc.vector.memset(vr[:, :, 64:65], 1.0)
            for ci in range(S // 512):
                k_thunk(0, ci)()
                for s in range(4 * ci, 4 * ci + 4):
                    v_thunk(0, s)()
                for t in range(2 * ci, min(2 * ci + 2, TB)):
                    qp_thunk(0, t)()

            # ================= attention stream ========================
            for j in range(PAIRS):
                fillers = pair_fillers(j)
                rec_den = prec.tile([128, T], F32, tag="rec", name="rec")
                kpt = kpT_t.pop(j)
                qpt = qpT_t.pop(j)
                for th in range(NTH):
                    cx_e = pcx.tile([VW, 512], F32, tag="cx", name="cxe")
                    cx_o = pcx.tile([VW, 512], F32, tag="cx", name="cxo")
                    for kbp in range(KBP):
                        sc_e = psc.tile([128, 1024], F32, tag="sc", name="sce")
                        sc_o = psc.tile([128, 1024], F32, tag="sc", name="sco")
                        # High priority keeps the even/odd head score matmuls
                        # adjacent in the scheduled PE stream: they target
                        # disjoint PE row groups (contraction rows 0-63 vs
                        # 64-127), so the HW runs adjacent pairs concurrently.
                        with tc.high_priority():
                            for kk in range(2):
                                kb = 2 * kbp + kk
                                for pr0, sc in ((0, sc_e), (64, sc_o)):
                                    nc.tensor.matmul(
                                        sc[:, kk * 512:(kk + 1) * 512],
                                        lhsT=kpt[pr0:pr0 + 64,
                                                 kb * 128:(kb + 1) * 128],
                                        rhs=qpt[pr0:pr0 + 64,
                                                th * 512:(th + 1) * 512],
                                        start=True, stop=True)
                        at_e = patn.tile([128, 1024], BF16, tag="at",
                                         name="ate")
                        nc.scalar.activation(out=at_e, in_=sc_e, func=AF.Exp,
                                             scale=0.125)
                        at_o = patn.tile([128, 1024], BF16, tag="at",
                                         name="ato")
                        nc.scalar.activation(out=at_o, in_=sc_o, func=AF.Exp,
                                             scale=0.125)
                        st = (kbp == 0)
                        sp = (kbp == KBP - 1)
                        for kk in range(2):
                            kb = 2 * kbp + kk
                            vr = vp_sb[kb].rearrange("p (h c) -> p h c", c=VW)
                            nc.tensor.matmul(
                                cx_e, lhsT=vr[:, 2 * j, :],
                                rhs=at_e[:, kk * 512:(kk + 1) * 512],
                                start=(st and kk == 0), stop=(sp and kk == 1))
                            nc.tensor.matmul(
                                cx_o, lhsT=vr[:, 2 * j + 1, :],
                                rhs=at_o[:, kk * 512:(kk + 1) * 512],
                                start=(st and kk == 0), stop=(sp and kk == 1))
                        npop = 1 if (th == 0 and kbp < 6) else 3
                        for _ in range(npop):
                            if fillers:
                                fillers.pop(0)()
                    # ---- sweep epilogue: den + ctx evacuation ---------
                    # Denominators go out to DRAM and come back as a
                    # partition-broadcast read (baseline-proven path).
                    tcol = slice(th * 512, (th + 1) * 512)
                    rr = pstg.tile([65, 512], F32, tag="recrow", name="recrow")
                    nc.vector.tensor_copy(out=rr[64:65, :],
                                          in_=cx_e[64:65, :])
                    # den write + bcast read share the gpsimd queue so FIFO
                    # order guarantees the DRAM RAW dependency
                    nc.gpsimd.dma_start(out=den_dram[2 * j, tcol],
                                        in_=rr[64:65, :])
                    rr2 = pstg.tile([65, 512], F32, tag="recrow",
                                    name="recrow2")
                    nc.vector.tensor_copy(out=rr2[64:65, :],
                                          in_=cx_o[64:65, :])
                    nc.gpsimd.dma_start(out=den_dram[2 * j + 1, tcol],
                                        in_=rr2[64:65, :])
                    nc.vector.tensor_copy(out=ctxT_sb[j][0:64, tcol],
                                          in_=cx_e[0:64, :])
                    tmp = ptmp.tile([64, 512], BF16, tag="ctmp", name="ctmp")
                    nc.vector.tensor_copy(out=tmp, in_=cx_o[0:64, :])
                    nc.sync.dma_start(out=ctxT_sb[j][64:128, tcol], in_=tmp)
                    # ---- normalize + bias for this query half ---------
                    # (overlaps the next sweep; keeps the pair-boundary
                    # and attention->FC bubbles short)
                    nc.gpsimd.dma_start(
                        out=rec_den[0:64, tcol],
                        in_=bcast_ap(den_dram[2 * j:2 * j + 1, tcol], 64))
                    nc.gpsimd.dma_start(
                        out=rec_den[64:128, tcol],
                        in_=bcast_ap(den_dram[2 * j + 1:2 * j + 2, tcol], 64))
                    nc.vector.reciprocal_approx_fast(
                        out=rec_den[:, tcol], in_=rec_den[:, tcol])
                    nc.vector.tensor_mul(out=ctxT_sb[j][:, tcol],
                                         in0=ctxT_sb[j][:, tcol],
                                         in1=rec_den[:, tcol])
                    nc.vector.tensor_scalar(out=ctxT_sb[j][:, tcol],
                                            in0=ctxT_sb[j][:, tcol],
                                            scalar1=bvT[:, j:j + 1],
                                            scalar2=None, op0=ALU.add)
                while fillers:
                    fillers.pop(0)()

        # ================= FC + residual + layernorm ====================
        with tc.tile_pool(name="fcps", bufs=2, space="PSUM") as pfc, \
             tc.tile_pool(name="lnbc", bufs=1) as plnb, \
             tc.tile_pool(name="qpl", bufs=2) as pqp, \
             tc.tile_pool(name="xln", bufs=2) as px, \
             tc.tile_pool(name="stat", bufs=4) as pst:
            gamma_bc = plnb.tile([128, D], F32, tag="gamma_bc", name="gamma_bc")
            nc.gpsimd.dma_start(out=gamma_bc, in_=bcast_ap(gamma, 128))
            beta_bc = plnb.tile([128, D], F32, tag="beta_bc", name="beta_bc")
            nc.gpsimd.dma_start(out=beta_bc, in_=bcast_ap(beta, 128))
            bfc_bc = plnb.tile([128, D], F32, tag="bfc_bc", name="bfc_bc")
            nc.gpsimd.dma_start(out=bfc_bc, in_=bcast_ap(bfc, 128))

            for t in range(TB):
                qp_t = pqp.tile([128, D], F32, tag="qp_t", name="qp_t")
                nc.sync.dma_start(out=qp_t,
                                  in_=qp_dram[t * 128:(t + 1) * 128, :])
                # bfc-add is off the fc critical chain: runs as soon as the
                # readback lands, before fc is ready.
                nc.gpsimd.tensor_add(out=qp_t, in0=qp_t, in1=bfc_bc)
                fc = pfc.tile([128, D], F32, tag="fc", name="fc")
                for jj in range(PAIRS):
                    for n0 in range(0, D, 512):
                        nc.tensor.matmul(
                            fc[:, n0:n0 + 512],
                            lhsT=ctxT_sb[jj][:, t * 128:(t + 1) * 128],
                            rhs=wfc_sb[:, jj, n0:n0 + 512],
                            start=(jj == 0), stop=(jj == PAIRS - 1))
                x = px.tile([128, D], F32, tag="x", name="x")
                # fc is PSUM: GpSimd cannot read it, so this add stays on DVE
                nc.vector.tensor_add(out=x, in0=fc, in1=qp_t)
                ngr = max(D // 512, 1)
                gsz = min(D, 512)
                stats = pst.tile([128, ngr, 6], F32, tag="stats", name="stats")
                for g in range(ngr):
                    nc.vector.bn_stats(out=stats[:, g, :],
                                       in_=x[:, g * gsz:(g + 1) * gsz])
                mv = pst.tile([128, 2], F32, tag="mv", name="mv")
                nc.vector.bn_aggr(out=mv, in_=stats)
                rstd = pst.tile([128, 1], F32, tag="rstd", name="rstd")
                nc.scalar.activation(out=rstd, in_=mv[:, 1:2], func=AF.Sqrt,
                                     bias=eps_t, scale=1.0)
                nc.vector.reciprocal(out=rstd, in_=rstd)
                xn = px.tile([128, D], F32, tag="xn", name="xn")
                eng = nc.vector if t % 2 == 0 else nc.gpsimd
                eng.tensor_scalar(out=xn, in0=x, scalar1=mv[:, 0:1],
                                  scalar2=rstd, op0=ALU.subtract,
                                  op1=ALU.mult)
                nc.gpsimd.tensor_mul(out=xn, in0=xn, in1=gamma_bc)
                nc.gpsimd.tensor_add(out=xn, in0=xn, in1=beta_bc)
                out_eng = nc.sync if t % 2 == 0 else nc.scalar
                out_eng.dma_start(out=out[t * 128:(t + 1) * 128, :], in_=xn)

    nc.compile()
    return nc


_B, _S, _D, _H, _DK = 4, 2048, 1024, 16, 64
_T = _S // 2
_NCORES = 8
_BF = ml_dtypes.bfloat16

_nc_cache = [None]


def _get_nc():
    if _nc_cache[0] is None:
        _nc_cache[0] = build(T=_T, S=_S, D=_D, H=_H, DK=_DK, n_cores=_NCORES)
    return _nc_cache[0]


def _execute(inputs, trace=False):
    from concourse.bass_utils import run_bass_kernel_spmd

    nc = _get_nc()
    q = np.asarray(inputs["q"], np.float32)
    k = np.asarray(inputs["k"], np.float32)
    v = np.asarray(inputs["v"], np.float32)
    Wq = np.asarray(inputs["Wq"], np.float32).astype(_BF)
    Wk = np.asarray(inputs["Wk"], np.float32).astype(_BF)
    Wv = np.asarray(inputs["Wv"], np.float32).astype(_BF)
    Wfc = np.asarray(inputs["Wfc"], np.float32).astype(_BF)
    fp = {n: np.asarray(inputs[n], np.float32)
          for n in ("bq", "bk", "bv", "bfc", "gamma", "beta")}

    in_maps = []
    for c in range(_NCORES):
        b, half = divmod(c, 2)
        t0 = half * _T
        in_maps.append({
            "qT": np.ascontiguousarray(q[b, t0:t0 + _T].T).astype(_BF),
            "kT": np.ascontiguousarray(k[b].T).astype(_BF),
            "vT": np.ascontiguousarray(v[b].T).astype(_BF),
            "Wq": Wq, "Wk": Wk, "Wv": Wv, "Wfc": Wfc, **fp,
        })

    res = run_bass_kernel_spmd(nc, in_maps, core_ids=list(range(_NCORES)),
                               trace=trace)
    out = np.empty((_B, _S, _D), np.float32)
    for c in range(_NCORES):
        b, half = divmod(c, 2)
        out[b, half * _T:(half + 1) * _T] = res.results[c]["out"]
    return out, res.exec_time_ns


def kernel(**inputs) -> np.ndarray:
    out, _ = _execute(inputs, trace=False)
    return out



# revision 58
# speedup vs baseline: 1.1697x; 1.0016x over previous
"""Trainium2 Bass kernel for nn_AttentionLayer (B=4, S=2048, D=1024, H=16).

Self-contained: builds and compiles an SPMD Bass/Tile program once, then
runs it across 8 NeuronCores via run_bass_kernel_spmd.

Sharding (no collectives): core c handles batch b = c // 2 and query-token
half c % 2 (T=1024 query tokens). Each core receives pre-transposed bf16
activations plus bf16 weights, computes its [1024, 1024] slice of the
final layernorm output in fp32, and the host reassembles.

v2 pipeline: one continuous exp-overlapped stream. All projection work
(K and Q per-pair, V per-2-pair-chunk) is emitted as filler thunks inside
the attention loop so TensorE slack under the ScalarE exp stream is
filled; only pair 0's projections run up front (column-chunked DMAs so
compute starts on the first chunk). Attention runs per head-pair with
query-half sweeps so PSUM fits:
  sc 2x[128,1024] (4 banks) + cx 2x[65,512] (2) + fill 2x[128,512] (2).
The two heads' score matmuls contract over 64 rows at partitions 0-63 /
64-127 (disjoint PE row groups) and are forced adjacent in the schedule
via high_priority, so the hardware runs each pair concurrently (~2x).
Softmax denominators come from a ones-column in the V projection and take
a DRAM roundtrip for the partition-broadcast (GpSimd cannot touch PSUM,
and partition_broadcast corrupts on HW). The residual q-projection is
computed natural (fp32, DRAM staging for the FC phase) and PE-transposed
into the attention layout, replacing a second full Q projection. FC
matmuls overlap the final attention pair; layernorm alternates DVE/Pool.
"""

import numpy as np
import ml_dtypes

from contextlib import ExitStack

import concourse.bass as bass
import concourse.tile as tile
import concourse.mybir as mybir
from concourse import bacc
from concourse import masks

F32 = mybir.dt.float32
BF16 = mybir.dt.bfloat16
AF = mybir.ActivationFunctionType
ALU = mybir.AluOpType


def bcast_ap(ap: bass.AP, parts: int) -> bass.AP:
    """Partition-broadcast a [1, N]-shaped DRAM AP to [parts, N]."""
    return bass.AP(tensor=ap.tensor, offset=ap.offset,
                   ap=[[0, parts]] + list(ap.ap[-1:]))


def build(T=1024, S=2048, D=1024, H=16, DK=64, n_cores=8, eps=1e-5,
          trn_type="TRN2"):
    assert DK == 64 and H % 2 == 0 and D == H * DK
    DB = D // 128      # contraction chunks over d
    PAIRS = H // 2     # head pairs == 128-row output blocks
    TB = T // 128
    SB = S // 128      # key blocks
    KBP = SB // 2      # key-block pairs per sweep
    NTH = T // 512     # query halves
    VW = 65            # per-head vp stripe: 64 v columns + 1 ones column
    VCH = 2            # pairs per V-projection chunk

    nc = bacc.Bacc(trn_type, target_bir_lowering=False, debug=False,
                   num_devices=n_cores)

    qT = nc.dram_tensor("qT", [D, T], BF16, kind="ExternalInput").ap()
    kT = nc.dram_tensor("kT", [D, S], BF16, kind="ExternalInput").ap()
    vT = nc.dram_tensor("vT", [D, S], BF16, kind="ExternalInput").ap()
    Wq = nc.dram_tensor("Wq", [D, D], BF16, kind="ExternalInput").ap()
    Wk = nc.dram_tensor("Wk", [D, D], BF16, kind="ExternalInput").ap()
    Wv = nc.dram_tensor("Wv", [D, D], BF16, kind="ExternalInput").ap()
    Wfc = nc.dram_tensor("Wfc", [D, D], BF16, kind="ExternalInput").ap()
    bq = nc.dram_tensor("bq", [D], F32, kind="ExternalInput").ap()
    bk = nc.dram_tensor("bk", [D], F32, kind="ExternalInput").ap()
    bv = nc.dram_tensor("bv", [D], F32, kind="ExternalInput").ap()
    bfc = nc.dram_tensor("bfc", [D], F32, kind="ExternalInput").ap()
    gamma = nc.dram_tensor("gamma", [D], F32, kind="ExternalInput").ap()
    beta = nc.dram_tensor("beta", [D], F32, kind="ExternalInput").ap()
    out = nc.dram_tensor("out", [T, D], F32, kind="ExternalOutput").ap()

    qp_dram = nc.dram_tensor("qp_scratch", [T, D], F32).ap()
    den_dram = nc.dram_tensor("den_scratch", [H, T], F32).ap()

    WkR = Wk.rearrange("(db p) n -> p db n", p=128)
    WqR = Wq.rearrange("(db p) n -> p db n", p=128)
    WvR = Wv.rearrange("(db p) n -> p db n", p=128)
    WfcR = Wfc.rearrange("(db p) n -> p db n", p=128)

    with tile.TileContext(nc) as tc, ExitStack() as ctx:
        pconst = ctx.enter_context(tc.tile_pool(name="const", bufs=1))
        ppers = ctx.enter_context(tc.tile_pool(name="persist", bufs=1))
        pkpt = ctx.enter_context(tc.tile_pool(name="kpt", bufs=2))
        pqpt = ctx.enter_context(tc.tile_pool(name="qpt", bufs=3))
        pwfc = ctx.enter_context(tc.tile_pool(name="wfc", bufs=1))

        # ---- tiny constants -------------------------------------------
        bkT = pconst.tile([128, PAIRS], F32, tag="bkT", name="bkT")
        nc.gpsimd.dma_start(out=bkT, in_=bk.rearrange("(e p) -> p e", p=128))
        bvT = pconst.tile([128, PAIRS], F32, tag="bvT", name="bvT")
        nc.gpsimd.dma_start(out=bvT, in_=bv.rearrange("(e p) -> p e", p=128))
        eps_t = pconst.tile([128, 1], F32, tag="eps", name="eps")
        nc.vector.memset(eps_t, eps)
        ident = pconst.tile([128, 128], F32, tag="ident", name="ident")
        masks.make_identity(nc, ident)

        # ---- persistent tiles -----------------------------------------
        vp_sb = [ppers.tile([128, H * VW], BF16, tag=f"vp{s}", name=f"vp{s}")
                 for s in range(SB)]
        ctxT_sb = [ppers.tile([128, T], BF16, tag=f"ctxT{e}", name=f"ctxT{e}")
                   for e in range(PAIRS)]
        wfc_sb = pwfc.tile([128, DB, D], BF16, tag="wfc", name="wfc")

        kpT_t = {}   # pair -> rotating kpT tile [128, S]
        qpT_t = {}   # pair -> rotating qpT tile [128, T]

        with ExitStack() as attn_ctx:
            pkx = attn_ctx.enter_context(tc.tile_pool(name="kx", bufs=1))
            pvx = attn_ctx.enter_context(tc.tile_pool(name="vx", bufs=1))
            pqx = attn_ctx.enter_context(tc.tile_pool(name="qx", bufs=1))
            pwk = attn_ctx.enter_context(tc.tile_pool(name="wk", bufs=2))
            pwq = attn_ctx.enter_context(tc.tile_pool(name="wq", bufs=2))
            pwv = attn_ctx.enter_context(tc.tile_pool(name="wv", bufs=2))
            pbq = attn_ctx.enter_context(tc.tile_pool(name="bq", bufs=2))
            patn = attn_ctx.enter_context(tc.tile_pool(name="attn", bufs=8))
            pstg = attn_ctx.enter_context(tc.tile_pool(name="stg", bufs=3))
            ptmp = attn_ctx.enter_context(tc.tile_pool(name="ctmp", bufs=2))
            prec = attn_ctx.enter_context(tc.tile_pool(name="rec", bufs=1))
            psc = attn_ctx.enter_context(
                tc.tile_pool(name="scps", bufs=2, space="PSUM"))
            pcx = attn_ctx.enter_context(
                tc.tile_pool(name="cxps", bufs=2, space="PSUM"))
            pfil = attn_ctx.enter_context(
                tc.tile_pool(name="filps", bufs=2, space="PSUM"))

            # ---- input staging ----------------------------------------
            # Queue split so pair-0 work is not gated behind bulk loads:
            # sync: wk + kT; scalar: vT; gpsimd: small weights + qT.
            wk_t = {}
            wq_t = {}
            wv_t = {}
            bq_t = {}

            def load_pair_weights(j):
                wk_t[j] = pwk.tile([128, DB, 128], BF16, tag="wk",
                                   name=f"wk{j}")
                nc.sync.dma_start(out=wk_t[j],
                                  in_=WkR[:, :, j * 128:(j + 1) * 128])
                wq_t[j] = pwq.tile([128, DB, 128], BF16, tag="wq",
                                   name=f"wq{j}")
                nc.gpsimd.dma_start(out=wq_t[j],
                                    in_=WqR[:, :, j * 128:(j + 1) * 128])
                bq_t[j] = pbq.tile([128, 128], F32, tag="bq", name=f"bq{j}")
                nc.gpsimd.dma_start(out=bq_t[j],
                                    in_=bcast_ap(bq[j * 128:(j + 1) * 128], 128))

            def load_vchunk_weights(c):
                wv_t[c] = pwv.tile([128, DB, VCH * 128], BF16, tag="wv",
                                   name=f"wv{c}")
                nc.gpsimd.dma_start(
                    out=wv_t[c],
                    in_=WvR[:, :, c * VCH * 128:(c + 1) * VCH * 128])

            load_pair_weights(0)
            load_vchunk_weights(0)
            kx_sb = [pkx.tile([128, S], BF16, tag=f"kx{d}", name=f"kx{d}")
                     for d in range(DB)]
            vx_sb = [pvx.tile([128, S], BF16, tag=f"vx{d}", name=f"vx{d}")
                     for d in range(DB)]
            qx_sb = [pqx.tile([128, T], BF16, tag=f"qx{d}", name=f"qx{d}")
                     for d in range(DB)]
            # Input loads are column-chunk-major so the pair-0 projections
            # can start on the first chunk instead of the full tensor.
            for ci in range(S // 512):
                for d in range(DB):
                    nc.sync.dma_start(
                        out=kx_sb[d][:, ci * 512:(ci + 1) * 512],
                        in_=kT[d * 128:(d + 1) * 128, ci * 512:(ci + 1) * 512])
            for ci in range(S // 512):
                for d in range(DB):
                    nc.scalar.dma_start(
                        out=vx_sb[d][:, ci * 512:(ci + 1) * 512],
                        in_=vT[d * 128:(d + 1) * 128, ci * 512:(ci + 1) * 512])
            for ci in range(T // 512):
                for d in range(DB):
                    nc.gpsimd.dma_start(
                        out=qx_sb[d][:, ci * 512:(ci + 1) * 512],
                        in_=qT[d * 128:(d + 1) * 128, ci * 512:(ci + 1) * 512])

            # ---- thunk builders (emit one group of work each) ---------
            def k_thunk(j, ci):
                def f():
                    if j not in kpT_t:
                        kpT_t[j] = pkpt.tile([128, S], BF16, tag="kpT",
                                             name=f"kpT{j}")
                    ps = pfil.tile([128, 512], F32, tag="fil", name="kps")
                    for d in range(DB):
                        nc.tensor.matmul(
                            ps, lhsT=wk_t[j][:, d, :],
                            rhs=kx_sb[d][:, ci * 512:(ci + 1) * 512],
                            start=(d == 0), stop=(d == DB - 1))
                    nc.vector.tensor_scalar(
                        out=kpT_t[j][:, ci * 512:(ci + 1) * 512], in0=ps,
                        scalar1=bkT[:, j:j + 1], scalar2=None, op0=ALU.add)
                return f

            def v_thunk(c, s):
                def f():
                    ps = pfil.tile([128, 512], F32, tag="fil", name="vps")
                    psv = ps[:, 0:VCH * 128]
                    for d in range(DB):
                        nc.tensor.matmul(
                            psv, lhsT=vx_sb[d][:, s * 128:(s + 1) * 128],
                            rhs=wv_t[c][:, d, :],
                            start=(d == 0), stop=(d == DB - 1))
                    vr = vp_sb[s].rearrange("p (h c) -> p h c", c=VW)
                    nc.vector.tensor_copy(
                        out=vr[:, 2 * VCH * c:2 * VCH * (c + 1), 0:64],
                        in_=psv.rearrange("p (h c) -> p h c", c=64))
                return f

            def qp_thunk(j, t):
                def f():
                    if j not in qpT_t:
                        qpT_t[j] = pqpt.tile([128, T], BF16, tag="qpT",
                                             name=f"qpT{j}")
                    ps = pfil.tile([128, 512], F32, tag="fil", name="qps")
                    psq = ps[:, 0:128]
                    for d in range(DB):
                        nc.tensor.matmul(
                            psq, lhsT=qx_sb[d][:, t * 128:(t + 1) * 128],
                            rhs=wq_t[j][:, d, :],
                            start=(d == 0), stop=(d == DB - 1))
                    stg = pstg.tile([128, 128], F32, tag="qpn", name="qpn")
                    nc.vector.tensor_add(out=stg, in0=psq, in1=bq_t[j])
                    nc.sync.dma_start(
                        out=qp_dram[t * 128:(t + 1) * 128,
                                    j * 128:(j + 1) * 128],
                        in_=stg)
                    trp = pfil.tile([128, 512], F32, tag="fil", name="trp")
                    nc.tensor.transpose(trp[:, 0:128], stg, ident)
                    nc.vector.tensor_copy(
                        out=qpT_t[j][:, t * 128:(t + 1) * 128],
                        in_=trp[:, 0:128])
                return f

            def interleave(*lists):
                res = []
                n = max(len(x) for x in lists)
                for i in range(n):
                    for x in lists:
                        if i < len(x):
                            res.append(x[i])
                return res

            def pair_fillers(j):
                """Work to interleave into pair j's attention stream."""
                nxt = j + 1
                ks, qs, vs, misc = [], [], [], []
                if nxt < PAIRS:
                    load_pair_weights(nxt)
                    ks = [k_thunk(nxt, ci) for ci in range(S // 512)]
                    qs = [qp_thunk(nxt, t) for t in range(TB)]
                # V chunk c (pairs 2c, 2c+1): half during pair 2c-2, half
                # during pair 2c-1, so filler load is spread evenly.
                c = j // 2 + 1
                if c < PAIRS // VCH:
                    if j % 2 == 0:
                        load_vchunk_weights(c)
                    half = SB // 2
                    s0 = (j % 2) * half
                    vs = [v_thunk(c, s) for s in range(s0, s0 + half)]
                if j == PAIRS - 2:
                    def load_wfc():
                        nc.sync.dma_start(out=wfc_sb, in_=WfcR)
                    misc = [load_wfc]
                return interleave(ks, qs, vs) + misc

            # ================= prologue: pair 0 compute ================
            # Interleaved by input chunk so each thunk starts as soon as
            # its DMA slice lands.
            for s in range(SB):
                vr = vp_sb[s].rearrange("p (h c) -> p h c", c=VW)
                nc.vector.memset(vr[:, :, 64:65], 1.0)
            for ci in range(S // 512):
                k_thunk(0, ci)()
                for s in range(4 * ci, 4 * ci + 4):
                    v_thunk(0, s)()
                for t in range(2 * ci, min(2 * ci + 2, TB)):
                    qp_thunk(0, t)()

            # ================= attention stream ========================
            for j in range(PAIRS):
                fillers = pair_fillers(j)
                rec_den = prec.tile([128, T], F32, tag="rec", name="rec")
                kpt = kpT_t.pop(j)
                qpt = qpT_t.pop(j)
                for th in range(NTH):
                    cx_e = pcx.tile([VW, 512], F32, tag="cx", name="cxe")
                    cx_o = pcx.tile([VW, 512], F32, tag="cx", name="cxo")
                    for kbp in range(KBP):
                        sc_e = psc.tile([128, 1024], F32, tag="sc", name="sce")
                        sc_o = psc.tile([128, 1024], F32, tag="sc", name="sco")
                        # High priority keeps the even/odd head score matmuls
                        # adjacent in the scheduled PE stream: they target
                        # disjoint PE row groups (contraction rows 0-63 vs
                        # 64-127), so the HW runs adjacent pairs concurrently.
                        with tc.high_priority():
                            for kk in range(2):
                                kb = 2 * kbp + kk
                                for pr0, sc in ((0, sc_e), (64, sc_o)):
                                    nc.tensor.matmul(
                                        sc[:, kk * 512:(kk + 1) * 512],
                                        lhsT=kpt[pr0:pr0 + 64,
                                                 kb * 128:(kb + 1) * 128],
                                        rhs=qpt[pr0:pr0 + 64,
                                                th * 512:(th + 1) * 512],
                                        start=True, stop=True)
                        at_e = patn.tile([128, 1024], BF16, tag="at",
                                         name="ate")
                        nc.scalar.activation(out=at_e, in_=sc_e, func=AF.Exp,
                                             scale=0.125)
                        at_o = patn.tile([128, 1024], BF16, tag="at",
                                         name="ato")
                        nc.scalar.activation(out=at_o, in_=sc_o, func=AF.Exp,
                                             scale=0.125)
                        st = (kbp == 0)
                        sp = (kbp == KBP - 1)
                        for kk in range(2):
                            kb = 2 * kbp + kk
                            vr = vp_sb[kb].rearrange("p (h c) -> p h c", c=VW)
                            nc.tensor.matmul(
                                cx_e, lhsT=vr[:, 2 * j, :],
                                rhs=at_e[:, kk * 512:(kk + 1) * 512],
                                start=(st and kk == 0), stop=(sp and kk == 1))
                            nc.tensor.matmul(
                                cx_o, lhsT=vr[:, 2 * j + 1, :],
                                rhs=at_o[:, kk * 512:(kk + 1) * 512],
                                start=(st and kk == 0), stop=(sp and kk == 1))
                        npop = 1 if (th == 0 and kbp < 6) else 3
                        for _ in range(npop):
                            if fillers:
                                fillers.pop(0)()
                    # ---- sweep epilogue: den + ctx evacuation ---------
                    # Denominators go out to DRAM and come back as a
                    # partition-broadcast read (baseline-proven path).
                    tcol = slice(th * 512, (th + 1) * 512)
                    rr = pstg.tile([65, 512], F32, tag="recrow", name="recrow")
                    nc.vector.tensor_copy(out=rr[64:65, :],
                                          in_=cx_e[64:65, :])
                    # den write + bcast read share the gpsimd queue so FIFO
                    # order guarantees the DRAM RAW dependency
                    nc.gpsimd.dma_start(out=den_dram[2 * j, tcol],
                                        in_=rr[64:65, :])
                    rr2 = pstg.tile([65, 512], F32, tag="recrow",
                                    name="recrow2")
                    nc.vector.tensor_copy(out=rr2[64:65, :],
                                          in_=cx_o[64:65, :])
                    nc.gpsimd.dma_start(out=den_dram[2 * j + 1, tcol],
                                        in_=rr2[64:65, :])
                    nc.vector.tensor_copy(out=ctxT_sb[j][0:64, tcol],
                                          in_=cx_e[0:64, :])
                    tmp = ptmp.tile([64, 512], BF16, tag="ctmp", name="ctmp")
                    nc.vector.tensor_copy(out=tmp, in_=cx_o[0:64, :])
                    nc.sync.dma_start(out=ctxT_sb[j][64:128, tcol], in_=tmp)
                    # ---- normalize + bias for this query half ---------
                    # (overlaps the next sweep; keeps the pair-boundary
                    # and attention->FC bubbles short)
                    nc.gpsimd.dma_start(
                        out=rec_den[0:64, tcol],
                        in_=bcast_ap(den_dram[2 * j:2 * j + 1, tcol], 64))
                    nc.gpsimd.dma_start(
                        out=rec_den[64:128, tcol],
                        in_=bcast_ap(den_dram[2 * j + 1:2 * j + 2, tcol], 64))
                    nc.vector.reciprocal_approx_fast(
                        out=rec_den[:, tcol], in_=rec_den[:, tcol])
                    nc.vector.tensor_mul(out=ctxT_sb[j][:, tcol],
                                         in0=ctxT_sb[j][:, tcol],
                                         in1=rec_den[:, tcol])
                    nc.vector.tensor_scalar(out=ctxT_sb[j][:, tcol],
                                            in0=ctxT_sb[j][:, tcol],
                                            scalar1=bvT[:, j:j + 1],
                                            scalar2=None, op0=ALU.add)
                while fillers:
                    fillers.pop(0)()

        # ================= FC + residual + layernorm ====================
        with tc.tile_pool(name="fcps", bufs=2, space="PSUM") as pfc, \
             tc.tile_pool(name="lnbc", bufs=1) as plnb, \
             tc.tile_pool(name="qpl", bufs=2) as pqp, \
             tc.tile_pool(name="xln", bufs=2) as px, \
             tc.tile_pool(name="stat", bufs=4) as pst:
            gamma_bc = plnb.tile([128, D], F32, tag="gamma_bc", name="gamma_bc")
            nc.gpsimd.dma_start(out=gamma_bc, in_=bcast_ap(gamma, 128))
            beta_bc = plnb.tile([128, D], F32, tag="beta_bc", name="beta_bc")
            nc.gpsimd.dma_start(out=beta_bc, in_=bcast_ap(beta, 128))
            bfc_bc = plnb.tile([128, D], F32, tag="bfc_bc", name="bfc_bc")
            nc.gpsimd.dma_start(out=bfc_bc, in_=bcast_ap(bfc, 128))

            for t in range(TB):
                qp_t = pqp.tile([128, D], F32, tag="qp_t", name="qp_t")
                nc.sync.dma_start(out=qp_t,
                                  in_=qp_dram[t * 128:(t + 1) * 128, :])
                # bfc-add is off the fc critical chain: runs as soon as the
                # readback lands, before fc is ready.
                nc.gpsimd.tensor_add(out=qp_t, in0=qp_t, in1=bfc_bc)
                fc = pfc.tile([128, D], F32, tag="fc", name="fc")
                for jj in range(PAIRS):
                    for n0 in range(0, D, 512):
                        nc.tensor.matmul(
                            fc[:, n0:n0 + 512],
                            lhsT=ctxT_sb[jj][:, t * 128:(t + 1) * 128],
                            rhs=wfc_sb[:, jj, n0:n0 + 512],
                            start=(jj == 0), stop=(jj == PAIRS - 1))
                x = px.tile([128, D], F32, tag="x", name="x")
                # fc is PSUM: GpSimd cannot read it, so this add stays on DVE
                nc.vector.tensor_add(out=x, in0=fc, in1=qp_t)
                ngr = max(D // 512, 1)
                gsz = min(D, 512)
                stats = pst.tile([128, ngr, 6], F32, tag="stats", name="stats")
                for g in range(ngr):
                    nc.vector.bn_stats(out=stats[:, g, :],
                                       in_=x[:, g * gsz:(g + 1) * gsz])
                mv = pst.tile([128, 2], F32, tag="mv", name="mv")
                nc.vector.bn_aggr(out=mv, in_=stats)
                rstd = pst.tile([128, 1], F32, tag="rstd", name="rstd")
                nc.scalar.activation(out=rstd, in_=mv[:, 1:2], func=AF.Sqrt,
                                     bias=eps_t, scale=1.0)
                nc.vector.reciprocal(out=rstd, in_=rstd)
                xn = px.tile([128, D], F32, tag="xn", name="xn")
                # Last block runs fully on DVE (idle by then) so its chain
                # does not queue behind earlier blocks' Pool ops.
                last = (t == TB - 1)
                eng = nc.vector if (t % 2 == 0 or last) else nc.gpsimd
                gb = nc.vector if last else nc.gpsimd
                eng.tensor_scalar(out=xn, in0=x, scalar1=mv[:, 0:1],
                                  scalar2=rstd, op0=ALU.subtract,
                                  op1=ALU.mult)
                gb.tensor_mul(out=xn, in0=xn, in1=gamma_bc)
                gb.tensor_add(out=xn, in0=xn, in1=beta_bc)
                out_eng = nc.sync if t % 2 == 0 else nc.scalar
                out_eng.dma_start(out=out[t * 128:(t + 1) * 128, :], in_=xn)

    nc.compile()
    return nc


_B, _S, _D, _H, _DK = 4, 2048, 1024, 16, 64
_T = _S // 2
_NCORES = 8
_BF = ml_dtypes.bfloat16

_nc_cache = [None]


def _get_nc():
    if _nc_cache[0] is None:
        _nc_cache[0] = build(T=_T, S=_S, D=_D, H=_H, DK=_DK, n_cores=_NCORES)
    return _nc_cache[0]


def _execute(inputs, trace=False):
    from concourse.bass_utils import run_bass_kernel_spmd

    nc = _get_nc()
    q = np.asarray(inputs["q"], np.float32)
    k = np.asarray(inputs["k"], np.float32)
    v = np.asarray(inputs["v"], np.float32)
    Wq = np.asarray(inputs["Wq"], np.float32).astype(_BF)
    Wk = np.asarray(inputs["Wk"], np.float32).astype(_BF)
    Wv = np.asarray(inputs["Wv"], np.float32).astype(_BF)
    Wfc = np.asarray(inputs["Wfc"], np.float32).astype(_BF)
    fp = {n: np.asarray(inputs[n], np.float32)
          for n in ("bq", "bk", "bv", "bfc", "gamma", "beta")}

    in_maps = []
    for c in range(_NCORES):
        b, half = divmod(c, 2)
        t0 = half * _T
        in_maps.append({
            "qT": np.ascontiguousarray(q[b, t0:t0 + _T].T).astype(_BF),
            "kT": np.ascontiguousarray(k[b].T).astype(_BF),
            "vT": np.ascontiguousarray(v[b].T).astype(_BF),
            "Wq": Wq, "Wk": Wk, "Wv": Wv, "Wfc": Wfc, **fp,
        })

    res = run_bass_kernel_spmd(nc, in_maps, core_ids=list(range(_NCORES)),
                               trace=trace)
    out = np.empty((_B, _S, _D), np.float32)
    for c in range(_NCORES):
        b, half = divmod(c, 2)
        out[b, half * _T:(half + 1) * _T] = res.results[c]["out"]
    return out, res.exec_time_ns


def kernel(**inputs) -> np.ndarray:
    out, _ = _execute(inputs, trace=False)
    return out



# revision 61
# speedup vs baseline: 1.1708x; 1.0009x over previous
"""Trainium2 Bass kernel for nn_AttentionLayer (B=4, S=2048, D=1024, H=16).

Self-contained: builds and compiles an SPMD Bass/Tile program once, then
runs it across 8 NeuronCores via run_bass_kernel_spmd.

Sharding (no collectives): core c handles batch b = c // 2 and query-token
half c % 2 (T=1024 query tokens). Each core receives pre-transposed bf16
activations plus bf16 weights, computes its [1024, 1024] slice of the
final layernorm output in fp32, and the host reassembles.

v2 pipeline: one continuous exp-overlapped stream. All projection work
(K and Q per-pair, V per-2-pair-chunk) is emitted as filler thunks inside
the attention loop so TensorE slack under the ScalarE exp stream is
filled; only pair 0's projections run up front (column-chunked DMAs so
compute starts on the first chunk). Attention runs per head-pair with
query-half sweeps so PSUM fits:
  sc 2x[128,1024] (4 banks) + cx 2x[65,512] (2) + fill 2x[128,512] (2).
The two heads' score matmuls contract over 64 rows at partitions 0-63 /
64-127 (disjoint PE row groups) and are forced adjacent in the schedule
via high_priority, so the hardware runs each pair concurrently (~2x).
Softmax denominators come from a ones-column in the V projection and take
a DRAM roundtrip for the partition-broadcast (GpSimd cannot touch PSUM,
and partition_broadcast corrupts on HW). The residual q-projection is
computed natural (fp32, DRAM staging for the FC phase) and PE-transposed
into the attention layout, replacing a second full Q projection. FC
matmuls overlap the final attention pair; layernorm alternates DVE/Pool.
"""

import numpy as np
import ml_dtypes

from contextlib import ExitStack

import concourse.bass as bass
import concourse.tile as tile
import concourse.mybir as mybir
from concourse import bacc
from concourse import masks

F32 = mybir.dt.float32
BF16 = mybir.dt.bfloat16
AF = mybir.ActivationFunctionType
ALU = mybir.AluOpType


def bcast_ap(ap: bass.AP, parts: int) -> bass.AP:
    """Partition-broadcast a [1, N]-shaped DRAM AP to [parts, N]."""
    return bass.AP(tensor=ap.tensor, offset=ap.offset,
                   ap=[[0, parts]] + list(ap.ap[-1:]))


def build(T=1024, S=2048, D=1024, H=16, DK=64, n_cores=8, eps=1e-5,
          trn_type="TRN2"):
    assert DK == 64 and H % 2 == 0 and D == H * DK
    DB = D // 128      # contraction chunks over d
    PAIRS = H // 2     # head pairs == 128-row output blocks
    TB = T // 128
    SB = S // 128      # key blocks
    KBP = SB // 2      # key-block pairs per sweep
    NTH = T // 512     # query halves
    VW = 65            # per-head vp stripe: 64 v columns + 1 ones column
    VCH = 2            # pairs per V-projection chunk

    nc = bacc.Bacc(trn_type, target_bir_lowering=False, debug=False,
                   num_devices=n_cores)

    qT = nc.dram_tensor("qT", [D, T], BF16, kind="ExternalInput").ap()
    kT = nc.dram_tensor("kT", [D, S], BF16, kind="ExternalInput").ap()
    vT = nc.dram_tensor("vT", [D, S], BF16, kind="ExternalInput").ap()
    Wq = nc.dram_tensor("Wq", [D, D], BF16, kind="ExternalInput").ap()
    Wk = nc.dram_tensor("Wk", [D, D], BF16, kind="ExternalInput").ap()
    Wv = nc.dram_tensor("Wv", [D, D], BF16, kind="ExternalInput").ap()
    Wfc = nc.dram_tensor("Wfc", [D, D], BF16, kind="ExternalInput").ap()
    bq = nc.dram_tensor("bq", [D], F32, kind="ExternalInput").ap()
    bk = nc.dram_tensor("bk", [D], F32, kind="ExternalInput").ap()
    bv = nc.dram_tensor("bv", [D], F32, kind="ExternalInput").ap()
    bfc = nc.dram_tensor("bfc", [D], F32, kind="ExternalInput").ap()
    gamma = nc.dram_tensor("gamma", [D], F32, kind="ExternalInput").ap()
    beta = nc.dram_tensor("beta", [D], F32, kind="ExternalInput").ap()
    out = nc.dram_tensor("out", [T, D], F32, kind="ExternalOutput").ap()

    qp_dram = nc.dram_tensor("qp_scratch", [T, D], F32).ap()
    den_dram = nc.dram_tensor("den_scratch", [H, T], F32).ap()

    WkR = Wk.rearrange("(db p) n -> p db n", p=128)
    WqR = Wq.rearrange("(db p) n -> p db n", p=128)
    WvR = Wv.rearrange("(db p) n -> p db n", p=128)
    WfcR = Wfc.rearrange("(db p) n -> p db n", p=128)

    with tile.TileContext(nc) as tc, ExitStack() as ctx:
        pconst = ctx.enter_context(tc.tile_pool(name="const", bufs=1))
        ppers = ctx.enter_context(tc.tile_pool(name="persist", bufs=1))
        pkpt = ctx.enter_context(tc.tile_pool(name="kpt", bufs=2))
        pqpt = ctx.enter_context(tc.tile_pool(name="qpt", bufs=3))
        pwfc = ctx.enter_context(tc.tile_pool(name="wfc", bufs=1))

        # ---- tiny constants -------------------------------------------
        bkT = pconst.tile([128, PAIRS], F32, tag="bkT", name="bkT")
        nc.gpsimd.dma_start(out=bkT, in_=bk.rearrange("(e p) -> p e", p=128))
        bvT = pconst.tile([128, PAIRS], F32, tag="bvT", name="bvT")
        nc.gpsimd.dma_start(out=bvT, in_=bv.rearrange("(e p) -> p e", p=128))
        eps_t = pconst.tile([128, 1], F32, tag="eps", name="eps")
        nc.vector.memset(eps_t, eps)
        ident = pconst.tile([128, 128], F32, tag="ident", name="ident")
        masks.make_identity(nc, ident)

        # ---- persistent tiles -----------------------------------------
        vp_sb = [ppers.tile([128, H * VW], BF16, tag=f"vp{s}", name=f"vp{s}")
                 for s in range(SB)]
        ctxT_sb = [ppers.tile([128, T], BF16, tag=f"ctxT{e}", name=f"ctxT{e}")
                   for e in range(PAIRS)]
        wfc_sb = pwfc.tile([128, DB, D], BF16, tag="wfc", name="wfc")

        kpT_t = {}   # pair -> rotating kpT tile [128, S]
        qpT_t = {}   # pair -> rotating qpT tile [128, T]

        with ExitStack() as attn_ctx:
            pkx = attn_ctx.enter_context(tc.tile_pool(name="kx", bufs=1))
            pvx = attn_ctx.enter_context(tc.tile_pool(name="vx", bufs=1))
            pqx = attn_ctx.enter_context(tc.tile_pool(name="qx", bufs=1))
            pwk = attn_ctx.enter_context(tc.tile_pool(name="wk", bufs=2))
            pwq = attn_ctx.enter_context(tc.tile_pool(name="wq", bufs=2))
            pwv = attn_ctx.enter_context(tc.tile_pool(name="wv", bufs=2))
            pbq = attn_ctx.enter_context(tc.tile_pool(name="bq", bufs=2))
            patn = attn_ctx.enter_context(tc.tile_pool(name="attn", bufs=8))
            pstg = attn_ctx.enter_context(tc.tile_pool(name="stg", bufs=3))
            ptmp = attn_ctx.enter_context(tc.tile_pool(name="ctmp", bufs=2))
            prec = attn_ctx.enter_context(tc.tile_pool(name="rec", bufs=1))
            psc = attn_ctx.enter_context(
                tc.tile_pool(name="scps", bufs=2, space="PSUM"))
            pcx = attn_ctx.enter_context(
                tc.tile_pool(name="cxps", bufs=2, space="PSUM"))
            pfil = attn_ctx.enter_context(
                tc.tile_pool(name="filps", bufs=2, space="PSUM"))

            # ---- input staging ----------------------------------------
            # Queue split so pair-0 work is not gated behind bulk loads:
            # sync: wk + kT; scalar: vT; gpsimd: small weights + qT.
            wk_t = {}
            wq_t = {}
            wv_t = {}
            bq_t = {}

            def load_pair_weights(j):
                wk_t[j] = pwk.tile([128, DB, 128], BF16, tag="wk",
                                   name=f"wk{j}")
                nc.sync.dma_start(out=wk_t[j],
                                  in_=WkR[:, :, j * 128:(j + 1) * 128])
                wq_t[j] = pwq.tile([128, DB, 128], BF16, tag="wq",
                                   name=f"wq{j}")
                nc.gpsimd.dma_start(out=wq_t[j],
                                    in_=WqR[:, :, j * 128:(j + 1) * 128])
                bq_t[j] = pbq.tile([128, 128], F32, tag="bq", name=f"bq{j}")
                nc.gpsimd.dma_start(out=bq_t[j],
                                    in_=bcast_ap(bq[j * 128:(j + 1) * 128], 128))

            def load_vchunk_weights(c):
                wv_t[c] = pwv.tile([128, DB, VCH * 128], BF16, tag="wv",
                                   name=f"wv{c}")
                nc.gpsimd.dma_start(
                    out=wv_t[c],
                    in_=WvR[:, :, c * VCH * 128:(c + 1) * VCH * 128])

            load_pair_weights(0)
            load_vchunk_weights(0)
            kx_sb = [pkx.tile([128, S], BF16, tag=f"kx{d}", name=f"kx{d}")
                     for d in range(DB)]
            vx_sb = [pvx.tile([128, S], BF16, tag=f"vx{d}", name=f"vx{d}")
                     for d in range(DB)]
            qx_sb = [pqx.tile([128, T], BF16, tag=f"qx{d}", name=f"qx{d}")
                     for d in range(DB)]
            # Input loads are column-chunk-major so the pair-0 projections
            # can start on the first chunk instead of the full tensor.
            for ci in range(S // 512):
                for d in range(DB):
                    nc.sync.dma_start(
                        out=kx_sb[d][:, ci * 512:(ci + 1) * 512],
                        in_=kT[d * 128:(d + 1) * 128, ci * 512:(ci + 1) * 512])
            for ci in range(S // 512):
                for d in range(DB):
                    nc.scalar.dma_start(
                        out=vx_sb[d][:, ci * 512:(ci + 1) * 512],
                        in_=vT[d * 128:(d + 1) * 128, ci * 512:(ci + 1) * 512])
            for ci in range(T // 512):
                for d in range(DB):
                    nc.gpsimd.dma_start(
                        out=qx_sb[d][:, ci * 512:(ci + 1) * 512],
                        in_=qT[d * 128:(d + 1) * 128, ci * 512:(ci + 1) * 512])

            # ---- thunk builders (emit one group of work each) ---------
            def k_thunk(j, ci):
                def f():
                    if j not in kpT_t:
                        kpT_t[j] = pkpt.tile([128, S], BF16, tag="kpT",
                                             name=f"kpT{j}")
                    ps = pfil.tile([128, 512], F32, tag="fil", name="kps")
                    for d in range(DB):
                        nc.tensor.matmul(
                            ps, lhsT=wk_t[j][:, d, :],
                            rhs=kx_sb[d][:, ci * 512:(ci + 1) * 512],
                            start=(d == 0), stop=(d == DB - 1))
                    nc.vector.tensor_scalar(
                        out=kpT_t[j][:, ci * 512:(ci + 1) * 512], in0=ps,
                        scalar1=bkT[:, j:j + 1], scalar2=None, op0=ALU.add)
                return f

            def v_thunk(c, s):
                def f():
                    ps = pfil.tile([128, 512], F32, tag="fil", name="vps")
                    psv = ps[:, 0:VCH * 128]
                    for d in range(DB):
                        nc.tensor.matmul(
                            psv, lhsT=vx_sb[d][:, s * 128:(s + 1) * 128],
                            rhs=wv_t[c][:, d, :],
                            start=(d == 0), stop=(d == DB - 1))
                    vr = vp_sb[s].rearrange("p (h c) -> p h c", c=VW)
                    nc.vector.tensor_copy(
                        out=vr[:, 2 * VCH * c:2 * VCH * (c + 1), 0:64],
                        in_=psv.rearrange("p (h c) -> p h c", c=64))
                return f

            def qp_thunk(j, t):
                def f():
                    if j not in qpT_t:
                        qpT_t[j] = pqpt.tile([128, T], BF16, tag="qpT",
                                             name=f"qpT{j}")
                    ps = pfil.tile([128, 512], F32, tag="fil", name="qps")
                    psq = ps[:, 0:128]
                    for d in range(DB):
                        nc.tensor.matmul(
                            psq, lhsT=qx_sb[d][:, t * 128:(t + 1) * 128],
                            rhs=wq_t[j][:, d, :],
                            start=(d == 0), stop=(d == DB - 1))
                    stg = pstg.tile([128, 128], F32, tag="qpn", name="qpn")
                    nc.vector.tensor_add(out=stg, in0=psq, in1=bq_t[j])
                    nc.sync.dma_start(
                        out=qp_dram[t * 128:(t + 1) * 128,
                                    j * 128:(j + 1) * 128],
                        in_=stg)
                    trp = pfil.tile([128, 512], F32, tag="fil", name="trp")
                    nc.tensor.transpose(trp[:, 0:128], stg, ident)
                    nc.vector.tensor_copy(
                        out=qpT_t[j][:, t * 128:(t + 1) * 128],
                        in_=trp[:, 0:128])
                return f

            def interleave(*lists):
                res = []
                n = max(len(x) for x in lists)
                for i in range(n):
                    for x in lists:
                        if i < len(x):
                            res.append(x[i])
                return res

            def pair_fillers(j):
                """Work to interleave into pair j's attention stream."""
                nxt = j + 1
                ks, qs, vs, misc = [], [], [], []
                if nxt < PAIRS:
                    load_pair_weights(nxt)
                    ks = [k_thunk(nxt, ci) for ci in range(S // 512)]
                    qs = [qp_thunk(nxt, t) for t in range(TB)]
                # V chunk c (pairs 2c, 2c+1): half during pair 2c-2, half
                # during pair 2c-1, so filler load is spread evenly.
                c = j // 2 + 1
                if c < PAIRS // VCH:
                    if j % 2 == 0:
                        load_vchunk_weights(c)
                    half = SB // 2
                    s0 = (j % 2) * half
                    vs = [v_thunk(c, s) for s in range(s0, s0 + half)]
                if j == PAIRS - 2:
                    def load_wfc():
                        nc.sync.dma_start(out=wfc_sb, in_=WfcR)
                    misc = [load_wfc]
                return interleave(ks, qs, vs) + misc

            # ================= prologue: pair 0 compute ================
            # Interleaved by input chunk so each thunk starts as soon as
            # its DMA slice lands.
            for s in range(SB):
                vr = vp_sb[s].rearrange("p (h c) -> p h c", c=VW)
                nc.vector.memset(vr[:, :, 64:65], 1.0)
            for ci in range(S // 512):
                k_thunk(0, ci)()
                for s in range(4 * ci, 4 * ci + 4):
                    v_thunk(0, s)()
                for t in range(2 * ci, min(2 * ci + 2, TB)):
                    qp_thunk(0, t)()

            # ================= attention stream ========================
            for j in range(PAIRS):
                fillers = pair_fillers(j)
                rec_den = prec.tile([128, T], F32, tag="rec", name="rec")
                kpt = kpT_t.pop(j)
                qpt = qpT_t.pop(j)
                for th in range(NTH):
                    cx_e = pcx.tile([VW, 512], F32, tag="cx", name="cxe")
                    cx_o = pcx.tile([VW, 512], F32, tag="cx", name="cxo")
                    for kbp in range(KBP):
                        sc_e = psc.tile([128, 1024], F32, tag="sc", name="sce")
                        sc_o = psc.tile([128, 1024], F32, tag="sc", name="sco")
                        # High priority keeps the even/odd head score matmuls
                        # adjacent in the scheduled PE stream: they target
                        # disjoint PE row groups (contraction rows 0-63 vs
                        # 64-127), so the HW runs adjacent pairs concurrently.
                        with tc.high_priority():
                            for kk in range(2):
                                kb = 2 * kbp + kk
                                for pr0, sc in ((0, sc_e), (64, sc_o)):
                                    nc.tensor.matmul(
                                        sc[:, kk * 512:(kk + 1) * 512],
                                        lhsT=kpt[pr0:pr0 + 64,
                                                 kb * 128:(kb + 1) * 128],
                                        rhs=qpt[pr0:pr0 + 64,
                                                th * 512:(th + 1) * 512],
                                        start=True, stop=True)
                        at_e = patn.tile([128, 1024], BF16, tag="at",
                                         name="ate")
                        nc.scalar.activation(out=at_e, in_=sc_e, func=AF.Exp,
                                             scale=0.125)
                        at_o = patn.tile([128, 1024], BF16, tag="at",
                                         name="ato")
                        nc.scalar.activation(out=at_o, in_=sc_o, func=AF.Exp,
                                             scale=0.125)
                        st = (kbp == 0)
                        sp = (kbp == KBP - 1)
                        for kk in range(2):
                            kb = 2 * kbp + kk
                            vr = vp_sb[kb].rearrange("p (h c) -> p h c", c=VW)
                            nc.tensor.matmul(
                                cx_e, lhsT=vr[:, 2 * j, :],
                                rhs=at_e[:, kk * 512:(kk + 1) * 512],
                                start=(st and kk == 0), stop=(sp and kk == 1))
                            nc.tensor.matmul(
                                cx_o, lhsT=vr[:, 2 * j + 1, :],
                                rhs=at_o[:, kk * 512:(kk + 1) * 512],
                                start=(st and kk == 0), stop=(sp and kk == 1))
                        npop = 1 if (th == 0 and kbp < 4) else 3
                        for _ in range(npop):
                            if fillers:
                                fillers.pop(0)()
                    # ---- sweep epilogue: den + ctx evacuation ---------
                    # Denominators go out to DRAM and come back as a
                    # partition-broadcast read (baseline-proven path).
                    tcol = slice(th * 512, (th + 1) * 512)
                    rr = pstg.tile([65, 512], F32, tag="recrow", name="recrow")
                    nc.vector.tensor_copy(out=rr[64:65, :],
                                          in_=cx_e[64:65, :])
                    # den write + bcast read share the gpsimd queue so FIFO
                    # order guarantees the DRAM RAW dependency
                    nc.gpsimd.dma_start(out=den_dram[2 * j, tcol],
                                        in_=rr[64:65, :])
                    rr2 = pstg.tile([65, 512], F32, tag="recrow",
                                    name="recrow2")
                    nc.vector.tensor_copy(out=rr2[64:65, :],
                                          in_=cx_o[64:65, :])
                    nc.gpsimd.dma_start(out=den_dram[2 * j + 1, tcol],
                                        in_=rr2[64:65, :])
                    nc.vector.tensor_copy(out=ctxT_sb[j][0:64, tcol],
                                          in_=cx_e[0:64, :])
                    tmp = ptmp.tile([64, 512], BF16, tag="ctmp", name="ctmp")
                    nc.vector.tensor_copy(out=tmp, in_=cx_o[0:64, :])
                    nc.sync.dma_start(out=ctxT_sb[j][64:128, tcol], in_=tmp)
                    # ---- normalize + bias for this query half ---------
                    # (overlaps the next sweep; keeps the pair-boundary
                    # and attention->FC bubbles short)
                    nc.gpsimd.dma_start(
                        out=rec_den[0:64, tcol],
                        in_=bcast_ap(den_dram[2 * j:2 * j + 1, tcol], 64))
                    nc.gpsimd.dma_start(
                        out=rec_den[64:128, tcol],
                        in_=bcast_ap(den_dram[2 * j + 1:2 * j + 2, tcol], 64))
                    nc.vector.reciprocal_approx_fast(
                        out=rec_den[:, tcol], in_=rec_den[:, tcol])
                    nc.vector.tensor_mul(out=ctxT_sb[j][:, tcol],
                                         in0=ctxT_sb[j][:, tcol],
                                         in1=rec_den[:, tcol])
                    nc.vector.tensor_scalar(out=ctxT_sb[j][:, tcol],
                                            in0=ctxT_sb[j][:, tcol],
                                            scalar1=bvT[:, j:j + 1],
                                            scalar2=None, op0=ALU.add)
                while fillers:
                    fillers.pop(0)()

        # ================= FC + residual + layernorm ====================
        with tc.tile_pool(name="fcps", bufs=2, space="PSUM") as pfc, \
             tc.tile_pool(name="lnbc", bufs=1) as plnb, \
             tc.tile_pool(name="qpl", bufs=2) as pqp, \
             tc.tile_pool(name="xln", bufs=2) as px, \
             tc.tile_pool(name="stat", bufs=4) as pst:
            gamma_bc = plnb.tile([128, D], F32, tag="gamma_bc", name="gamma_bc")
            nc.gpsimd.dma_start(out=gamma_bc, in_=bcast_ap(gamma, 128))
            beta_bc = plnb.tile([128, D], F32, tag="beta_bc", name="beta_bc")
            nc.gpsimd.dma_start(out=beta_bc, in_=bcast_ap(beta, 128))
            bfc_bc = plnb.tile([128, D], F32, tag="bfc_bc", name="bfc_bc")
            nc.gpsimd.dma_start(out=bfc_bc, in_=bcast_ap(bfc, 128))

            for t in range(TB):
                qp_t = pqp.tile([128, D], F32, tag="qp_t", name="qp_t")
                nc.sync.dma_start(out=qp_t,
                                  in_=qp_dram[t * 128:(t + 1) * 128, :])
                # bfc-add is off the fc critical chain: runs as soon as the
                # readback lands, before fc is ready.
                nc.gpsimd.tensor_add(out=qp_t, in0=qp_t, in1=bfc_bc)
                fc = pfc.tile([128, D], F32, tag="fc", name="fc")
                for jj in range(PAIRS):
                    for n0 in range(0, D, 512):
                        nc.tensor.matmul(
                            fc[:, n0:n0 + 512],
                            lhsT=ctxT_sb[jj][:, t * 128:(t + 1) * 128],
                            rhs=wfc_sb[:, jj, n0:n0 + 512],
                            start=(jj == 0), stop=(jj == PAIRS - 1))
                x = px.tile([128, D], F32, tag="x", name="x")
                # fc is PSUM: GpSimd cannot read it, so this add stays on DVE
                nc.vector.tensor_add(out=x, in0=fc, in1=qp_t)
                ngr = max(D // 512, 1)
                gsz = min(D, 512)
                stats = pst.tile([128, ngr, 6], F32, tag="stats", name="stats")
                for g in range(ngr):
                    nc.vector.bn_stats(out=stats[:, g, :],
                                       in_=x[:, g * gsz:(g + 1) * gsz])
                mv = pst.tile([128, 2], F32, tag="mv", name="mv")
                nc.vector.bn_aggr(out=mv, in_=stats)
                rstd = pst.tile([128, 1], F32, tag="rstd", name="rstd")
                nc.scalar.activation(out=rstd, in_=mv[:, 1:2], func=AF.Sqrt,
                                     bias=eps_t, scale=1.0)
                nc.vector.reciprocal(out=rstd, in_=rstd)
                xn = px.tile([128, D], F32, tag="xn", name="xn")
                # Last block runs fully on DVE (idle by then) so its chain
                # does not queue behind earlier blocks' Pool ops.
                last = (t == TB - 1)
                eng = nc.vector if (t % 2 == 0 or last) else nc.gpsimd
                gb = nc.vector if last else nc.gpsimd
                eng.tensor_scalar(out=xn, in0=x, scalar1=mv[:, 0:1],
                                  scalar2=rstd, op0=ALU.subtract,
                                  op1=ALU.mult)
                gb.tensor_mul(out=xn, in0=xn, in1=gamma_bc)
                gb.tensor_add(out=xn, in0=xn, in1=beta_bc)
                out_eng = nc.sync if t % 2 == 0 else nc.scalar
                out_eng.dma_start(out=out[t * 128:(t + 1) * 128, :], in_=xn)

    nc.compile()
    return nc


_B, _S, _D, _H, _DK = 4, 2048, 1024, 16, 64
_T = _S // 2
_NCORES = 8
_BF = ml_dtypes.bfloat16

_nc_cache = [None]


def _get_nc():
    if _nc_cache[0] is None:
        _nc_cache[0] = build(T=_T, S=_S, D=_D, H=_H, DK=_DK, n_cores=_NCORES)
    return _nc_cache[0]


def _execute(inputs, trace=False):
    from concourse.bass_utils import run_bass_kernel_spmd

    nc = _get_nc()
    q = np.asarray(inputs["q"], np.float32)
    k = np.asarray(inputs["k"], np.float32)
    v = np.asarray(inputs["v"], np.float32)
    Wq = np.asarray(inputs["Wq"], np.float32).astype(_BF)
    Wk = np.asarray(inputs["Wk"], np.float32).astype(_BF)
    Wv = np.asarray(inputs["Wv"], np.float32).astype(_BF)
    Wfc = np.asarray(inputs["Wfc"], np.float32).astype(_BF)
    fp = {n: np.asarray(inputs[n], np.float32)
          for n in ("bq", "bk", "bv", "bfc", "gamma", "beta")}

    in_maps = []
    for c in range(_NCORES):
        b, half = divmod(c, 2)
        t0 = half * _T
        in_maps.append({
            "qT": np.ascontiguousarray(q[b, t0:t0 + _T].T).astype(_BF),
            "kT": np.ascontiguousarray(k[b].T).astype(_BF),
            "vT": np.ascontiguousarray(v[b].T).astype(_BF),
            "Wq": Wq, "Wk": Wk, "Wv": Wv, "Wfc": Wfc, **fp,
        })

    res = run_bass_kernel_spmd(nc, in_maps, core_ids=list(range(_NCORES)),
                               trace=trace)
    out = np.empty((_B, _S, _D), np.float32)
    for c in range(_NCORES):
        b, half = divmod(c, 2)
        out[b, half * _T:(half + 1) * _T] = res.results[c]["out"]
    return out, res.exec_time_ns


def kernel(**inputs) -> np.ndarray:
    out, _ = _execute(inputs, trace=False)
    return out



# revision 66
# speedup vs baseline: 1.1742x; 1.0029x over previous
"""Trainium2 Bass kernel for nn_AttentionLayer (B=4, S=2048, D=1024, H=16).

Self-contained: builds and compiles an SPMD Bass/Tile program once, then
runs it across 8 NeuronCores via run_bass_kernel_spmd.

Sharding (no collectives): core c handles batch b = c // 2 and query-token
half c % 2 (T=1024 query tokens). Each core receives pre-transposed bf16
activations plus bf16 weights, computes its [1024, 1024] slice of the
final layernorm output in fp32, and the host reassembles.

v2 pipeline: one continuous exp-overlapped stream. All projection work
(K and Q per-pair, V per-2-pair-chunk) is emitted as filler thunks inside
the attention loop so TensorE slack under the ScalarE exp stream is
filled; only pair 0's projections run up front (column-chunked DMAs so
compute starts on the first chunk). Attention runs per head-pair with
query-half sweeps so PSUM fits:
  sc 2x[128,1024] (4 banks) + cx 2x[65,512] (2) + fill 2x[128,512] (2).
The two heads' score matmuls contract over 64 rows at partitions 0-63 /
64-127 (disjoint PE row groups) and are forced adjacent in the schedule
via high_priority, so the hardware runs each pair concurrently (~2x).
Softmax denominators come from a ones-column in the V projection and take
a DRAM roundtrip for the partition-broadcast (GpSimd cannot touch PSUM,
and partition_broadcast corrupts on HW). The residual q-projection is
computed natural (fp32, DRAM staging for the FC phase) and PE-transposed
into the attention layout, replacing a second full Q projection. FC
matmuls overlap the final attention pair; layernorm alternates DVE/Pool.
"""

import numpy as np
import ml_dtypes

from contextlib import ExitStack

import concourse.bass as bass
import concourse.tile as tile
import concourse.mybir as mybir
from concourse import bacc
from concourse import masks

F32 = mybir.dt.float32
BF16 = mybir.dt.bfloat16
AF = mybir.ActivationFunctionType
ALU = mybir.AluOpType


def bcast_ap(ap: bass.AP, parts: int) -> bass.AP:
    """Partition-broadcast a [1, N]-shaped DRAM AP to [parts, N]."""
    return bass.AP(tensor=ap.tensor, offset=ap.offset,
                   ap=[[0, parts]] + list(ap.ap[-1:]))


def build(T=1024, S=2048, D=1024, H=16, DK=64, n_cores=8, eps=1e-5,
          trn_type="TRN2"):
    assert DK == 64 and H % 2 == 0 and D == H * DK
    DB = D // 128      # contraction chunks over d
    PAIRS = H // 2     # head pairs == 128-row output blocks
    TB = T // 128
    SB = S // 128      # key blocks
    KBP = SB // 2      # key-block pairs per sweep
    NTH = T // 512     # query halves
    VW = 65            # per-head vp stripe: 64 v columns + 1 ones column
    VCH = 2            # pairs per V-projection chunk

    nc = bacc.Bacc(trn_type, target_bir_lowering=False, debug=False,
                   num_devices=n_cores)

    qT = nc.dram_tensor("qT", [D, T], BF16, kind="ExternalInput").ap()
    kT = nc.dram_tensor("kT", [D, S], BF16, kind="ExternalInput").ap()
    vT = nc.dram_tensor("vT", [D, S], BF16, kind="ExternalInput").ap()
    Wq = nc.dram_tensor("Wq", [D, D], BF16, kind="ExternalInput").ap()
    Wk = nc.dram_tensor("Wk", [D, D], BF16, kind="ExternalInput").ap()
    Wv = nc.dram_tensor("Wv", [D, D], BF16, kind="ExternalInput").ap()
    Wfc = nc.dram_tensor("Wfc", [D, D], BF16, kind="ExternalInput").ap()
    bq = nc.dram_tensor("bq", [D], F32, kind="ExternalInput").ap()
    bk = nc.dram_tensor("bk", [D], F32, kind="ExternalInput").ap()
    bv = nc.dram_tensor("bv", [D], F32, kind="ExternalInput").ap()
    bfc = nc.dram_tensor("bfc", [D], F32, kind="ExternalInput").ap()
    gamma = nc.dram_tensor("gamma", [D], F32, kind="ExternalInput").ap()
    beta = nc.dram_tensor("beta", [D], F32, kind="ExternalInput").ap()
    out = nc.dram_tensor("out", [T, D], F32, kind="ExternalOutput").ap()

    qp_dram = nc.dram_tensor("qp_scratch", [T, D], F32).ap()
    den_dram = nc.dram_tensor("den_scratch", [H, T], F32).ap()

    WkR = Wk.rearrange("(db p) n -> p db n", p=128)
    WqR = Wq.rearrange("(db p) n -> p db n", p=128)
    WvR = Wv.rearrange("(db p) n -> p db n", p=128)
    WfcR = Wfc.rearrange("(db p) n -> p db n", p=128)

    with tile.TileContext(nc) as tc, ExitStack() as ctx:
        pconst = ctx.enter_context(tc.tile_pool(name="const", bufs=1))
        ppers = ctx.enter_context(tc.tile_pool(name="persist", bufs=1))
        pkpt = ctx.enter_context(tc.tile_pool(name="kpt", bufs=2))
        pqpt = ctx.enter_context(tc.tile_pool(name="qpt", bufs=3))
        pwfc = ctx.enter_context(tc.tile_pool(name="wfc", bufs=1))

        # ---- tiny constants -------------------------------------------
        bkT = pconst.tile([128, PAIRS], F32, tag="bkT", name="bkT")
        nc.gpsimd.dma_start(out=bkT, in_=bk.rearrange("(e p) -> p e", p=128))
        bvT = pconst.tile([128, PAIRS], F32, tag="bvT", name="bvT")
        nc.gpsimd.dma_start(out=bvT, in_=bv.rearrange("(e p) -> p e", p=128))
        eps_t = pconst.tile([128, 1], F32, tag="eps", name="eps")
        nc.vector.memset(eps_t, eps)
        ident = pconst.tile([128, 128], F32, tag="ident", name="ident")
        masks.make_identity(nc, ident)

        # ---- persistent tiles -----------------------------------------
        vp_sb = [ppers.tile([128, H * VW], BF16, tag=f"vp{s}", name=f"vp{s}")
                 for s in range(SB)]
        ctxT_sb = [ppers.tile([128, T], BF16, tag=f"ctxT{e}", name=f"ctxT{e}")
                   for e in range(PAIRS)]
        wfc_sb = pwfc.tile([128, DB, D], BF16, tag="wfc", name="wfc")

        kpT_t = {}   # pair -> rotating kpT tile [128, S]
        qpT_t = {}   # pair -> rotating qpT tile [128, T]

        with ExitStack() as attn_ctx:
            pkx = attn_ctx.enter_context(tc.tile_pool(name="kx", bufs=1))
            pvx = attn_ctx.enter_context(tc.tile_pool(name="vx", bufs=1))
            pqx = attn_ctx.enter_context(tc.tile_pool(name="qx", bufs=1))
            pwk = attn_ctx.enter_context(tc.tile_pool(name="wk", bufs=2))
            pwq = attn_ctx.enter_context(tc.tile_pool(name="wq", bufs=2))
            pwv = attn_ctx.enter_context(tc.tile_pool(name="wv", bufs=2))
            pbq = attn_ctx.enter_context(tc.tile_pool(name="bq", bufs=2))
            patn = attn_ctx.enter_context(tc.tile_pool(name="attn", bufs=8))
            pstg = attn_ctx.enter_context(tc.tile_pool(name="stg", bufs=3))
            ptmp = attn_ctx.enter_context(tc.tile_pool(name="ctmp", bufs=2))
            prec = attn_ctx.enter_context(tc.tile_pool(name="rec", bufs=1))
            psc = attn_ctx.enter_context(
                tc.tile_pool(name="scps", bufs=2, space="PSUM"))
            pcx = attn_ctx.enter_context(
                tc.tile_pool(name="cxps", bufs=2, space="PSUM"))
            pfil = attn_ctx.enter_context(
                tc.tile_pool(name="filps", bufs=2, space="PSUM"))

            # ---- input staging ----------------------------------------
            # Queue split so pair-0 work is not gated behind bulk loads:
            # sync: wk + kT; scalar: vT; gpsimd: small weights + qT.
            wk_t = {}
            wq_t = {}
            wv_t = {}
            bq_t = {}

            def load_pair_weights(j):
                wk_t[j] = pwk.tile([128, DB, 128], BF16, tag="wk",
                                   name=f"wk{j}")
                nc.sync.dma_start(out=wk_t[j],
                                  in_=WkR[:, :, j * 128:(j + 1) * 128])
                wq_t[j] = pwq.tile([128, DB, 128], BF16, tag="wq",
                                   name=f"wq{j}")
                nc.gpsimd.dma_start(out=wq_t[j],
                                    in_=WqR[:, :, j * 128:(j + 1) * 128])
                bq_t[j] = pbq.tile([128, 128], F32, tag="bq", name=f"bq{j}")
                nc.gpsimd.dma_start(out=bq_t[j],
                                    in_=bcast_ap(bq[j * 128:(j + 1) * 128], 128))

            def load_vchunk_weights(c):
                wv_t[c] = pwv.tile([128, DB, VCH * 128], BF16, tag="wv",
                                   name=f"wv{c}")
                nc.gpsimd.dma_start(
                    out=wv_t[c],
                    in_=WvR[:, :, c * VCH * 128:(c + 1) * VCH * 128])

            load_pair_weights(0)
            load_vchunk_weights(0)
            kx_sb = [pkx.tile([128, S], BF16, tag=f"kx{d}", name=f"kx{d}")
                     for d in range(DB)]
            vx_sb = [pvx.tile([128, S], BF16, tag=f"vx{d}", name=f"vx{d}")
                     for d in range(DB)]
            qx_sb = [pqx.tile([128, T], BF16, tag=f"qx{d}", name=f"qx{d}")
                     for d in range(DB)]
            # Input loads are column-chunk-major so the pair-0 projections
            # can start on the first chunk instead of the full tensor.
            for ci in range(S // 512):
                for d in range(DB):
                    nc.sync.dma_start(
                        out=kx_sb[d][:, ci * 512:(ci + 1) * 512],
                        in_=kT[d * 128:(d + 1) * 128, ci * 512:(ci + 1) * 512])
            for ci in range(S // 512):
                for d in range(DB):
                    nc.scalar.dma_start(
                        out=vx_sb[d][:, ci * 512:(ci + 1) * 512],
                        in_=vT[d * 128:(d + 1) * 128, ci * 512:(ci + 1) * 512])
            for ci in range(T // 512):
                for d in range(DB):
                    nc.gpsimd.dma_start(
                        out=qx_sb[d][:, ci * 512:(ci + 1) * 512],
                        in_=qT[d * 128:(d + 1) * 128, ci * 512:(ci + 1) * 512])

            # ---- thunk builders (emit one group of work each) ---------
            def k_thunk(j, ci):
                def f():
                    if j not in kpT_t:
                        kpT_t[j] = pkpt.tile([128, S], BF16, tag="kpT",
                                             name=f"kpT{j}")
                    ps = pfil.tile([128, 512], F32, tag="fil", name="kps")
                    for d in range(DB):
                        nc.tensor.matmul(
                            ps, lhsT=wk_t[j][:, d, :],
                            rhs=kx_sb[d][:, ci * 512:(ci + 1) * 512],
                            start=(d == 0), stop=(d == DB - 1))
                    nc.vector.tensor_scalar(
                        out=kpT_t[j][:, ci * 512:(ci + 1) * 512], in0=ps,
                        scalar1=bkT[:, j:j + 1], scalar2=None, op0=ALU.add)
                return f

            def v_thunk(c, s):
                def f():
                    ps = pfil.tile([128, 512], F32, tag="fil", name="vps")
                    psv = ps[:, 0:VCH * 128]
                    for d in range(DB):
                        nc.tensor.matmul(
                            psv, lhsT=vx_sb[d][:, s * 128:(s + 1) * 128],
                            rhs=wv_t[c][:, d, :],
                            start=(d == 0), stop=(d == DB - 1))
                    vr = vp_sb[s].rearrange("p (h c) -> p h c", c=VW)
                    nc.vector.tensor_copy(
                        out=vr[:, 2 * VCH * c:2 * VCH * (c + 1), 0:64],
                        in_=psv.rearrange("p (h c) -> p h c", c=64))
                return f

            def qp_thunk(j, t):
                def f():
                    if j not in qpT_t:
                        qpT_t[j] = pqpt.tile([128, T], BF16, tag="qpT",
                                             name=f"qpT{j}")
                    ps = pfil.tile([128, 512], F32, tag="fil", name="qps")
                    psq = ps[:, 0:128]
                    for d in range(DB):
                        nc.tensor.matmul(
                            psq, lhsT=qx_sb[d][:, t * 128:(t + 1) * 128],
                            rhs=wq_t[j][:, d, :],
                            start=(d == 0), stop=(d == DB - 1))
                    stg = pstg.tile([128, 128], F32, tag="qpn", name="qpn")
                    nc.vector.tensor_add(out=stg, in0=psq, in1=bq_t[j])
                    nc.sync.dma_start(
                        out=qp_dram[t * 128:(t + 1) * 128,
                                    j * 128:(j + 1) * 128],
                        in_=stg)
                    trp = pfil.tile([128, 512], F32, tag="fil", name="trp")
                    nc.tensor.transpose(trp[:, 0:128], stg, ident)
                    nc.vector.tensor_copy(
                        out=qpT_t[j][:, t * 128:(t + 1) * 128],
                        in_=trp[:, 0:128])
                return f

            def interleave(*lists):
                res = []
                n = max(len(x) for x in lists)
                for i in range(n):
                    for x in lists:
                        if i < len(x):
                            res.append(x[i])
                return res

            def pair_fillers(j):
                """Work to interleave into pair j's attention stream."""
                nxt = j + 1
                ks, qs, vs, misc = [], [], [], []
                if nxt < PAIRS:
                    load_pair_weights(nxt)
                    ks = [k_thunk(nxt, ci) for ci in range(S // 512)]
                    qs = [qp_thunk(nxt, t) for t in range(TB)]
                # V chunk c (pairs 2c, 2c+1): half during pair 2c-2, half
                # during pair 2c-1, so filler load is spread evenly.
                c = j // 2 + 1
                if c < PAIRS // VCH:
                    if j % 2 == 0:
                        load_vchunk_weights(c)
                    half = SB // 2
                    s0 = (j % 2) * half
                    vs = [v_thunk(c, s) for s in range(s0, s0 + half)]
                if j == PAIRS - 2:
                    def load_wfc():
                        nc.sync.dma_start(out=wfc_sb, in_=WfcR)
                    misc = [load_wfc]
                return interleave(ks, qs, vs) + misc

            # ================= prologue: pair 0 compute ================
            # Interleaved by input chunk so each thunk starts as soon as
            # its DMA slice lands.
            for s in range(SB):
                vr = vp_sb[s].rearrange("p (h c) -> p h c", c=VW)
                nc.vector.memset(vr[:, :, 64:65], 1.0)
            for ci in range(S // 512):
                k_thunk(0, ci)()
                for s in range(4 * ci, 4 * ci + 4):
                    v_thunk(0, s)()
                for t in range(2 * ci, min(2 * ci + 2, TB)):
                    qp_thunk(0, t)()

            # ================= attention stream ========================
            for j in range(PAIRS):
                fillers = pair_fillers(j)
                rec_den = prec.tile([128, T], F32, tag="rec", name="rec")
                kpt = kpT_t.pop(j)
                qpt = qpT_t.pop(j)
                for th in range(NTH):
                    cx_e = pcx.tile([VW, 512], F32, tag="cx", name="cxe")
                    cx_o = pcx.tile([VW, 512], F32, tag="cx", name="cxo")
                    for kbp in range(KBP):
                        sc_e = psc.tile([128, 1024], F32, tag="sc", name="sce")
                        sc_o = psc.tile([128, 1024], F32, tag="sc", name="sco")
                        # High priority keeps the even/odd head score matmuls
                        # adjacent in the scheduled PE stream: they target
                        # disjoint PE row groups (contraction rows 0-63 vs
                        # 64-127), so the HW runs adjacent pairs concurrently.
                        with tc.high_priority():
                            for kk in range(2):
                                kb = 2 * kbp + kk
                                for pr0, sc in ((0, sc_e), (64, sc_o)):
                                    nc.tensor.matmul(
                                        sc[:, kk * 512:(kk + 1) * 512],
                                        lhsT=kpt[pr0:pr0 + 64,
                                                 kb * 128:(kb + 1) * 128],
                                        rhs=qpt[pr0:pr0 + 64,
                                                th * 512:(th + 1) * 512],
                                        start=True, stop=True)
                        at_e = patn.tile([128, 1024], BF16, tag="at",
                                         name="ate")
                        nc.scalar.activation(out=at_e, in_=sc_e, func=AF.Exp,
                                             scale=0.125)
                        at_o = patn.tile([128, 1024], BF16, tag="at",
                                         name="ato")
                        nc.scalar.activation(out=at_o, in_=sc_o, func=AF.Exp,
                                             scale=0.125)
                        st = (kbp == 0)
                        sp = (kbp == KBP - 1)
                        for kk in range(2):
                            kb = 2 * kbp + kk
                            vr = vp_sb[kb].rearrange("p (h c) -> p h c", c=VW)
                            nc.tensor.matmul(
                                cx_e, lhsT=vr[:, 2 * j, :],
                                rhs=at_e[:, kk * 512:(kk + 1) * 512],
                                start=(st and kk == 0), stop=(sp and kk == 1))
                            nc.tensor.matmul(
                                cx_o, lhsT=vr[:, 2 * j + 1, :],
                                rhs=at_o[:, kk * 512:(kk + 1) * 512],
                                start=(st and kk == 0), stop=(sp and kk == 1))
                        npop = 1 if (th == 0 and kbp < 4) else 3
                        for _ in range(npop):
                            if fillers:
                                fillers.pop(0)()
                    # ---- sweep epilogue: den + ctx evacuation ---------
                    # Denominators go out to DRAM and come back as a
                    # partition-broadcast read (baseline-proven path).
                    tcol = slice(th * 512, (th + 1) * 512)
                    rr = pstg.tile([65, 512], F32, tag="recrow", name="recrow")
                    nc.vector.tensor_copy(out=rr[64:65, :],
                                          in_=cx_e[64:65, :])
                    # den write + bcast read share the gpsimd queue so FIFO
                    # order guarantees the DRAM RAW dependency
                    nc.gpsimd.dma_start(out=den_dram[2 * j, tcol],
                                        in_=rr[64:65, :])
                    rr2 = pstg.tile([65, 512], F32, tag="recrow",
                                    name="recrow2")
                    nc.vector.tensor_copy(out=rr2[64:65, :],
                                          in_=cx_o[64:65, :])
                    nc.gpsimd.dma_start(out=den_dram[2 * j + 1, tcol],
                                        in_=rr2[64:65, :])
                    nc.vector.tensor_copy(out=ctxT_sb[j][0:64, tcol],
                                          in_=cx_e[0:64, :])
                    tmp = ptmp.tile([64, 512], BF16, tag="ctmp", name="ctmp")
                    nc.vector.tensor_copy(out=tmp, in_=cx_o[0:64, :])
                    nc.sync.dma_start(out=ctxT_sb[j][64:128, tcol], in_=tmp)
                    # ---- normalize + bias for this query half ---------
                    # (overlaps the next sweep; keeps the pair-boundary
                    # and attention->FC bubbles short)
                    nc.gpsimd.dma_start(
                        out=rec_den[0:64, tcol],
                        in_=bcast_ap(den_dram[2 * j:2 * j + 1, tcol], 64))
                    nc.gpsimd.dma_start(
                        out=rec_den[64:128, tcol],
                        in_=bcast_ap(den_dram[2 * j + 1:2 * j + 2, tcol], 64))
                    nc.vector.reciprocal_approx_fast(
                        out=rec_den[:, tcol], in_=rec_den[:, tcol])
                    nc.vector.tensor_mul(out=ctxT_sb[j][:, tcol],
                                         in0=ctxT_sb[j][:, tcol],
                                         in1=rec_den[:, tcol])
                    nc.vector.tensor_scalar(out=ctxT_sb[j][:, tcol],
                                            in0=ctxT_sb[j][:, tcol],
                                            scalar1=bvT[:, j:j + 1],
                                            scalar2=None, op0=ALU.add)
                while fillers:
                    fillers.pop(0)()

        # ================= FC + residual + layernorm ====================
        with tc.tile_pool(name="fcps", bufs=2, space="PSUM") as pfc, \
             tc.tile_pool(name="lnbc", bufs=1) as plnb, \
             tc.tile_pool(name="qpl", bufs=2) as pqp, \
             tc.tile_pool(name="xln", bufs=2) as px, \
             tc.tile_pool(name="stat", bufs=4) as pst:
            gamma_bc = plnb.tile([128, D], F32, tag="gamma_bc", name="gamma_bc")
            nc.gpsimd.dma_start(out=gamma_bc, in_=bcast_ap(gamma, 128))
            beta_bc = plnb.tile([128, D], F32, tag="beta_bc", name="beta_bc")
            nc.gpsimd.dma_start(out=beta_bc, in_=bcast_ap(beta, 128))
            bfc_bc = plnb.tile([128, D], F32, tag="bfc_bc", name="bfc_bc")
            nc.gpsimd.dma_start(out=bfc_bc, in_=bcast_ap(bfc, 128))

            for t in range(TB):
                qp_t = pqp.tile([128, D], F32, tag="qp_t", name="qp_t")
                nc.sync.dma_start(out=qp_t,
                                  in_=qp_dram[t * 128:(t + 1) * 128, :])
                # bfc-add is off the fc critical chain: runs as soon as the
                # readback lands, before fc is ready.
                nc.gpsimd.tensor_add(out=qp_t, in0=qp_t, in1=bfc_bc)
                fc = pfc.tile([128, D], F32, tag="fc", name="fc")
                for jj in range(PAIRS):
                    for n0 in range(0, D, 512):
                        nc.tensor.matmul(
                            fc[:, n0:n0 + 512],
                            lhsT=ctxT_sb[jj][:, t * 128:(t + 1) * 128],
                            rhs=wfc_sb[:, jj, n0:n0 + 512],
                            start=(jj == 0), stop=(jj == PAIRS - 1))
                x = px.tile([128, D], F32, tag="x", name="x")
                # fc is PSUM: GpSimd cannot read it, so this add stays on DVE
                nc.vector.tensor_add(out=x, in0=fc, in1=qp_t)
                ngr = max(D // 512, 1)
                gsz = min(D, 512)
                stats = pst.tile([128, ngr, 6], F32, tag="stats", name="stats")
                for g in range(ngr):
                    nc.vector.bn_stats(out=stats[:, g, :],
                                       in_=x[:, g * gsz:(g + 1) * gsz])
                mv = pst.tile([128, 2], F32, tag="mv", name="mv")
                nc.vector.bn_aggr(out=mv, in_=stats)
                rstd = pst.tile([128, 1], F32, tag="rstd", name="rstd")
                nc.scalar.activation(out=rstd, in_=mv[:, 1:2], func=AF.Sqrt,
                                     bias=eps_t, scale=1.0)
                nc.vector.reciprocal(out=rstd, in_=rstd)
                xn = px.tile([128, D], F32, tag="xn", name="xn")
                rows = slice(t * 128, (t + 1) * 128)
                if t == TB - 1:
                    # The last block's chain IS the kernel tail: run it as
                    # two parallel column-half chains on DVE || Pool and
                    # split the store across both DMA queues.
                    for cols, engh, oeng in (
                            (slice(0, D // 2), nc.vector, nc.sync),
                            (slice(D // 2, D), nc.gpsimd, nc.scalar)):
                        engh.tensor_scalar(out=xn[:, cols], in0=x[:, cols],
                                           scalar1=mv[:, 0:1], scalar2=rstd,
                                           op0=ALU.subtract, op1=ALU.mult)
                        engh.tensor_mul(out=xn[:, cols], in0=xn[:, cols],
                                        in1=gamma_bc[:, cols])
                        engh.tensor_add(out=xn[:, cols], in0=xn[:, cols],
                                        in1=beta_bc[:, cols])
                        oeng.dma_start(out=out[rows, cols], in_=xn[:, cols])
                else:
                    eng = nc.vector if t % 2 == 0 else nc.gpsimd
                    eng.tensor_scalar(out=xn, in0=x, scalar1=mv[:, 0:1],
                                      scalar2=rstd, op0=ALU.subtract,
                                      op1=ALU.mult)
                    nc.gpsimd.tensor_mul(out=xn, in0=xn, in1=gamma_bc)
                    nc.gpsimd.tensor_add(out=xn, in0=xn, in1=beta_bc)
                    out_eng = nc.sync if t % 2 == 0 else nc.scalar
                    out_eng.dma_start(out=out[rows, :], in_=xn)

    nc.compile()
    return nc


_B, _S, _D, _H, _DK = 4, 2048, 1024, 16, 64
_T = _S // 2
_NCORES = 8
_BF = ml_dtypes.bfloat16

_nc_cache = [None]


def _get_nc():
    if _nc_cache[0] is None:
        _nc_cache[0] = build(T=_T, S=_S, D=_D, H=_H, DK=_DK, n_cores=_NCORES)
    return _nc_cache[0]


def _execute(inputs, trace=False):
    from concourse.bass_utils import run_bass_kernel_spmd

    nc = _get_nc()
    q = np.asarray(inputs["q"], np.float32)
    k = np.asarray(inputs["k"], np.float32)
    v = np.asarray(inputs["v"], np.float32)
    Wq = np.asarray(inputs["Wq"], np.float32).astype(_BF)
    Wk = np.asarray(inputs["Wk"], np.float32).astype(_BF)
    Wv = np.asarray(inputs["Wv"], np.float32).astype(_BF)
    Wfc = np.asarray(inputs["Wfc"], np.float32).astype(_BF)
    fp = {n: np.asarray(inputs[n], np.float32)
          for n in ("bq", "bk", "bv", "bfc", "gamma", "beta")}

    in_maps = []
    for c in range(_NCORES):
        b, half = divmod(c, 2)
        t0 = half * _T
        in_maps.append({
            "qT": np.ascontiguousarray(q[b, t0:t0 + _T].T).astype(_BF),
            "kT": np.ascontiguousarray(k[b].T).astype(_BF),
            "vT": np.ascontiguousarray(v[b].T).astype(_BF),
            "Wq": Wq, "Wk": Wk, "Wv": Wv, "Wfc": Wfc, **fp,
        })

    res = run_bass_kernel_spmd(nc, in_maps, core_ids=list(range(_NCORES)),
                               trace=trace)
    out = np.empty((_B, _S, _D), np.float32)
    for c in range(_NCORES):
        b, half = divmod(c, 2)
        out[b, half * _T:(half + 1) * _T] = res.results[c]["out"]
    return out, res.exec_time_ns


def kernel(**inputs) -> np.ndarray:
    out, _ = _execute(inputs, trace=False)
    return out



# revision 71
# speedup vs baseline: 1.1760x; 1.0015x over previous
"""Trainium2 Bass kernel for nn_AttentionLayer (B=4, S=2048, D=1024, H=16).

Self-contained: builds and compiles an SPMD Bass/Tile program once, then
runs it across 8 NeuronCores via run_bass_kernel_spmd.

Sharding (no collectives): core c handles batch b = c // 2 and query-token
half c % 2 (T=1024 query tokens). Each core receives pre-transposed bf16
activations plus bf16 weights, computes its [1024, 1024] slice of the
final layernorm output in fp32, and the host reassembles.

v2 pipeline: one continuous exp-overlapped stream. All projection work
(K and Q per-pair, V per-2-pair-chunk) is emitted as filler thunks inside
the attention loop so TensorE slack under the ScalarE exp stream is
filled; only pair 0's projections run up front (column-chunked DMAs so
compute starts on the first chunk). Attention runs per head-pair with
query-half sweeps so PSUM fits:
  sc 2x[128,1024] (4 banks) + cx 2x[65,512] (2) + fill 2x[128,512] (2).
The two heads' score matmuls contract over 64 rows at partitions 0-63 /
64-127 (disjoint PE row groups) and are forced adjacent in the schedule
via high_priority, so the hardware runs each pair concurrently (~2x).
Softmax denominators come from a ones-column in the V projection and take
a DRAM roundtrip for the partition-broadcast (GpSimd cannot touch PSUM,
and partition_broadcast corrupts on HW). The residual q-projection is
computed natural (fp32, DRAM staging for the FC phase) and PE-transposed
into the attention layout, replacing a second full Q projection. FC
matmuls overlap the final attention pair; layernorm alternates DVE/Pool.
"""

import numpy as np
import ml_dtypes

from contextlib import ExitStack

import concourse.bass as bass
import concourse.tile as tile
import concourse.mybir as mybir
from concourse import bacc
from concourse import masks

F32 = mybir.dt.float32
BF16 = mybir.dt.bfloat16
AF = mybir.ActivationFunctionType
ALU = mybir.AluOpType


def bcast_ap(ap: bass.AP, parts: int) -> bass.AP:
    """Partition-broadcast a [1, N]-shaped DRAM AP to [parts, N]."""
    return bass.AP(tensor=ap.tensor, offset=ap.offset,
                   ap=[[0, parts]] + list(ap.ap[-1:]))


def build(T=1024, S=2048, D=1024, H=16, DK=64, n_cores=8, eps=1e-5,
          trn_type="TRN2"):
    assert DK == 64 and H % 2 == 0 and D == H * DK
    DB = D // 128      # contraction chunks over d
    PAIRS = H // 2     # head pairs == 128-row output blocks
    TB = T // 128
    SB = S // 128      # key blocks
    KBP = SB // 2      # key-block pairs per sweep
    NTH = T // 512     # query halves
    VW = 65            # per-head vp stripe: 64 v columns + 1 ones column
    VCH = 2            # pairs per V-projection chunk

    nc = bacc.Bacc(trn_type, target_bir_lowering=False, debug=False,
                   num_devices=n_cores)

    qT = nc.dram_tensor("qT", [D, T], BF16, kind="ExternalInput").ap()
    kT = nc.dram_tensor("kT", [D, S], BF16, kind="ExternalInput").ap()
    vT = nc.dram_tensor("vT", [D, S], BF16, kind="ExternalInput").ap()
    Wq = nc.dram_tensor("Wq", [D, D], BF16, kind="ExternalInput").ap()
    Wk = nc.dram_tensor("Wk", [D, D], BF16, kind="ExternalInput").ap()
    Wv = nc.dram_tensor("Wv", [D, D], BF16, kind="ExternalInput").ap()
    Wfc = nc.dram_tensor("Wfc", [D, D], BF16, kind="ExternalInput").ap()
    bq = nc.dram_tensor("bq", [D], F32, kind="ExternalInput").ap()
    bk = nc.dram_tensor("bk", [D], F32, kind="ExternalInput").ap()
    bv = nc.dram_tensor("bv", [D], F32, kind="ExternalInput").ap()
    bfc = nc.dram_tensor("bfc", [D], F32, kind="ExternalInput").ap()
    gamma = nc.dram_tensor("gamma", [D], F32, kind="ExternalInput").ap()
    beta = nc.dram_tensor("beta", [D], F32, kind="ExternalInput").ap()
    out = nc.dram_tensor("out", [T, D], F32, kind="ExternalOutput").ap()

    qp_dram = nc.dram_tensor("qp_scratch", [T, D], F32).ap()
    den_dram = nc.dram_tensor("den_scratch", [H, T], F32).ap()

    WkR = Wk.rearrange("(db p) n -> p db n", p=128)
    WqR = Wq.rearrange("(db p) n -> p db n", p=128)
    WvR = Wv.rearrange("(db p) n -> p db n", p=128)
    WfcR = Wfc.rearrange("(db p) n -> p db n", p=128)

    with tile.TileContext(nc) as tc, ExitStack() as ctx:
        pconst = ctx.enter_context(tc.tile_pool(name="const", bufs=1))
        ppers = ctx.enter_context(tc.tile_pool(name="persist", bufs=1))
        pkpt = ctx.enter_context(tc.tile_pool(name="kpt", bufs=2))
        pqpt = ctx.enter_context(tc.tile_pool(name="qpt", bufs=3))
        pwfc = ctx.enter_context(tc.tile_pool(name="wfc", bufs=1))

        # ---- tiny constants -------------------------------------------
        bkT = pconst.tile([128, PAIRS], F32, tag="bkT", name="bkT")
        nc.gpsimd.dma_start(out=bkT, in_=bk.rearrange("(e p) -> p e", p=128))
        bvT = pconst.tile([128, PAIRS], F32, tag="bvT", name="bvT")
        nc.gpsimd.dma_start(out=bvT, in_=bv.rearrange("(e p) -> p e", p=128))
        eps_t = pconst.tile([128, 1], F32, tag="eps", name="eps")
        nc.vector.memset(eps_t, eps)
        ident = pconst.tile([128, 128], F32, tag="ident", name="ident")
        masks.make_identity(nc, ident)

        # ---- persistent tiles -----------------------------------------
        vp_sb = [ppers.tile([128, H * VW], BF16, tag=f"vp{s}", name=f"vp{s}")
                 for s in range(SB)]
        ctxT_sb = [ppers.tile([128, T], BF16, tag=f"ctxT{e}", name=f"ctxT{e}")
                   for e in range(PAIRS)]
        wfc_sb = pwfc.tile([128, DB, D], BF16, tag="wfc", name="wfc")

        kpT_t = {}   # pair -> rotating kpT tile [128, S]
        qpT_t = {}   # pair -> rotating qpT tile [128, T]

        with ExitStack() as attn_ctx:
            pkx = attn_ctx.enter_context(tc.tile_pool(name="kx", bufs=1))
            pvx = attn_ctx.enter_context(tc.tile_pool(name="vx", bufs=1))
            pqx = attn_ctx.enter_context(tc.tile_pool(name="qx", bufs=1))
            pwk = attn_ctx.enter_context(tc.tile_pool(name="wk", bufs=2))
            pwq = attn_ctx.enter_context(tc.tile_pool(name="wq", bufs=2))
            pwv = attn_ctx.enter_context(tc.tile_pool(name="wv", bufs=2))
            pbq = attn_ctx.enter_context(tc.tile_pool(name="bq", bufs=2))
            patn = attn_ctx.enter_context(tc.tile_pool(name="attn", bufs=9))
            pstg = attn_ctx.enter_context(tc.tile_pool(name="stg", bufs=3))
            ptmp = attn_ctx.enter_context(tc.tile_pool(name="ctmp", bufs=2))
            prec = attn_ctx.enter_context(tc.tile_pool(name="rec", bufs=1))
            psc = attn_ctx.enter_context(
                tc.tile_pool(name="scps", bufs=2, space="PSUM"))
            pcx = attn_ctx.enter_context(
                tc.tile_pool(name="cxps", bufs=2, space="PSUM"))
            pfil = attn_ctx.enter_context(
                tc.tile_pool(name="filps", bufs=2, space="PSUM"))

            # ---- input staging ----------------------------------------
            # Queue split so pair-0 work is not gated behind bulk loads:
            # sync: wk + kT; scalar: vT; gpsimd: small weights + qT.
            wk_t = {}
            wq_t = {}
            wv_t = {}
            bq_t = {}

            def load_pair_weights(j):
                wk_t[j] = pwk.tile([128, DB, 128], BF16, tag="wk",
                                   name=f"wk{j}")
                nc.sync.dma_start(out=wk_t[j],
                                  in_=WkR[:, :, j * 128:(j + 1) * 128])
                wq_t[j] = pwq.tile([128, DB, 128], BF16, tag="wq",
                                   name=f"wq{j}")
                nc.gpsimd.dma_start(out=wq_t[j],
                                    in_=WqR[:, :, j * 128:(j + 1) * 128])
                bq_t[j] = pbq.tile([128, 128], F32, tag="bq", name=f"bq{j}")
                nc.gpsimd.dma_start(out=bq_t[j],
                                    in_=bcast_ap(bq[j * 128:(j + 1) * 128], 128))

            def load_vchunk_weights(c):
                wv_t[c] = pwv.tile([128, DB, VCH * 128], BF16, tag="wv",
                                   name=f"wv{c}")
                nc.gpsimd.dma_start(
                    out=wv_t[c],
                    in_=WvR[:, :, c * VCH * 128:(c + 1) * VCH * 128])

            load_pair_weights(0)
            load_vchunk_weights(0)
            kx_sb = [pkx.tile([128, S], BF16, tag=f"kx{d}", name=f"kx{d}")
                     for d in range(DB)]
            vx_sb = [pvx.tile([128, S], BF16, tag=f"vx{d}", name=f"vx{d}")
                     for d in range(DB)]
            qx_sb = [pqx.tile([128, T], BF16, tag=f"qx{d}", name=f"qx{d}")
                     for d in range(DB)]
            # Input loads are column-chunk-major so the pair-0 projections
            # can start on the first chunk instead of the full tensor.
            for ci in range(S // 512):
                for d in range(DB):
                    nc.sync.dma_start(
                        out=kx_sb[d][:, ci * 512:(ci + 1) * 512],
                        in_=kT[d * 128:(d + 1) * 128, ci * 512:(ci + 1) * 512])
            for ci in range(S // 512):
                for d in range(DB):
                    nc.scalar.dma_start(
                        out=vx_sb[d][:, ci * 512:(ci + 1) * 512],
                        in_=vT[d * 128:(d + 1) * 128, ci * 512:(ci + 1) * 512])
            for ci in range(T // 512):
                for d in range(DB):
                    nc.gpsimd.dma_start(
                        out=qx_sb[d][:, ci * 512:(ci + 1) * 512],
                        in_=qT[d * 128:(d + 1) * 128, ci * 512:(ci + 1) * 512])

            # ---- thunk builders (emit one group of work each) ---------
            def k_thunk(j, ci):
                def f():
                    if j not in kpT_t:
                        kpT_t[j] = pkpt.tile([128, S], BF16, tag="kpT",
                                             name=f"kpT{j}")
                    ps = pfil.tile([128, 512], F32, tag="fil", name="kps")
                    for d in range(DB):
                        nc.tensor.matmul(
                            ps, lhsT=wk_t[j][:, d, :],
                            rhs=kx_sb[d][:, ci * 512:(ci + 1) * 512],
                            start=(d == 0), stop=(d == DB - 1))
                    nc.vector.tensor_scalar(
                        out=kpT_t[j][:, ci * 512:(ci + 1) * 512], in0=ps,
                        scalar1=bkT[:, j:j + 1], scalar2=None, op0=ALU.add)
                return f

            def v_thunk(c, s):
                def f():
                    ps = pfil.tile([128, 512], F32, tag="fil", name="vps")
                    psv = ps[:, 0:VCH * 128]
                    for d in range(DB):
                        nc.tensor.matmul(
                            psv, lhsT=vx_sb[d][:, s * 128:(s + 1) * 128],
                            rhs=wv_t[c][:, d, :],
                            start=(d == 0), stop=(d == DB - 1))
                    vr = vp_sb[s].rearrange("p (h c) -> p h c", c=VW)
                    nc.vector.tensor_copy(
                        out=vr[:, 2 * VCH * c:2 * VCH * (c + 1), 0:64],
                        in_=psv.rearrange("p (h c) -> p h c", c=64))
                return f

            def qp_thunk(j, t):
                def f():
                    if j not in qpT_t:
                        qpT_t[j] = pqpt.tile([128, T], BF16, tag="qpT",
                                             name=f"qpT{j}")
                    ps = pfil.tile([128, 512], F32, tag="fil", name="qps")
                    psq = ps[:, 0:128]
                    for d in range(DB):
                        nc.tensor.matmul(
                            psq, lhsT=qx_sb[d][:, t * 128:(t + 1) * 128],
                            rhs=wq_t[j][:, d, :],
                            start=(d == 0), stop=(d == DB - 1))
                    stg = pstg.tile([128, 128], F32, tag="qpn", name="qpn")
                    nc.vector.tensor_add(out=stg, in0=psq, in1=bq_t[j])
                    nc.sync.dma_start(
                        out=qp_dram[t * 128:(t + 1) * 128,
                                    j * 128:(j + 1) * 128],
                        in_=stg)
                    trp = pfil.tile([128, 512], F32, tag="fil", name="trp")
                    nc.tensor.transpose(trp[:, 0:128], stg, ident)
                    nc.vector.tensor_copy(
                        out=qpT_t[j][:, t * 128:(t + 1) * 128],
                        in_=trp[:, 0:128])
                return f

            def interleave(*lists):
                res = []
                n = max(len(x) for x in lists)
                for i in range(n):
                    for x in lists:
                        if i < len(x):
                            res.append(x[i])
                return res

            def pair_fillers(j):
                """Work to interleave into pair j's attention stream."""
                nxt = j + 1
                ks, qs, vs, misc = [], [], [], []
                if nxt < PAIRS:
                    load_pair_weights(nxt)
                    ks = [k_thunk(nxt, ci) for ci in range(S // 512)]
                    qs = [qp_thunk(nxt, t) for t in range(TB)]
                # V chunk c (pairs 2c, 2c+1): half during pair 2c-2, half
                # during pair 2c-1, so filler load is spread evenly.
                c = j // 2 + 1
                if c < PAIRS // VCH:
                    if j % 2 == 0:
                        load_vchunk_weights(c)
                    half = SB // 2
                    s0 = (j % 2) * half
                    vs = [v_thunk(c, s) for s in range(s0, s0 + half)]
                if j == PAIRS - 2:
                    def load_wfc():
                        nc.sync.dma_start(out=wfc_sb, in_=WfcR)
                    misc = [load_wfc]
                return interleave(ks, qs, vs) + misc

            # ================= prologue: pair 0 compute ================
            # Interleaved by input chunk so each thunk starts as soon as
            # its DMA slice lands.
            for s in range(SB):
                vr = vp_sb[s].rearrange("p (h c) -> p h c", c=VW)
                nc.vector.memset(vr[:, :, 64:65], 1.0)
            for ci in range(S // 512):
                k_thunk(0, ci)()
                for s in range(4 * ci, 4 * ci + 4):
                    v_thunk(0, s)()
                for t in range(2 * ci, min(2 * ci + 2, TB)):
                    qp_thunk(0, t)()

            # ================= attention stream ========================
            for j in range(PAIRS):
                fillers = pair_fillers(j)
                rec_den = prec.tile([128, T], F32, tag="rec", name="rec")
                kpt = kpT_t.pop(j)
                qpt = qpT_t.pop(j)
                for th in range(NTH):
                    cx_e = pcx.tile([VW, 512], F32, tag="cx", name="cxe")
                    cx_o = pcx.tile([VW, 512], F32, tag="cx", name="cxo")
                    for kbp in range(KBP):
                        sc_e = psc.tile([128, 1024], F32, tag="sc", name="sce")
                        sc_o = psc.tile([128, 1024], F32, tag="sc", name="sco")
                        # High priority keeps the even/odd head score matmuls
                        # adjacent in the scheduled PE stream: they target
                        # disjoint PE row groups (contraction rows 0-63 vs
                        # 64-127), so the HW runs adjacent pairs concurrently.
                        with tc.high_priority():
                            for kk in range(2):
                                kb = 2 * kbp + kk
                                for pr0, sc in ((0, sc_e), (64, sc_o)):
                                    nc.tensor.matmul(
                                        sc[:, kk * 512:(kk + 1) * 512],
                                        lhsT=kpt[pr0:pr0 + 64,
                                                 kb * 128:(kb + 1) * 128],
                                        rhs=qpt[pr0:pr0 + 64,
                                                th * 512:(th + 1) * 512],
                                        start=True, stop=True)
                        at_e = patn.tile([128, 1024], BF16, tag="at",
                                         name="ate")
                        nc.scalar.activation(out=at_e, in_=sc_e, func=AF.Exp,
                                             scale=0.125)
                        at_o = patn.tile([128, 1024], BF16, tag="at",
                                         name="ato")
                        nc.scalar.activation(out=at_o, in_=sc_o, func=AF.Exp,
                                             scale=0.125)
                        st = (kbp == 0)
                        sp = (kbp == KBP - 1)
                        for kk in range(2):
                            kb = 2 * kbp + kk
                            vr = vp_sb[kb].rearrange("p (h c) -> p h c", c=VW)
                            nc.tensor.matmul(
                                cx_e, lhsT=vr[:, 2 * j, :],
                                rhs=at_e[:, kk * 512:(kk + 1) * 512],
                                start=(st and kk == 0), stop=(sp and kk == 1))
                            nc.tensor.matmul(
                                cx_o, lhsT=vr[:, 2 * j + 1, :],
                                rhs=at_o[:, kk * 512:(kk + 1) * 512],
                                start=(st and kk == 0), stop=(sp and kk == 1))
                        npop = 1 if (th == 0 and kbp < 4) else 3
                        for _ in range(npop):
                            if fillers:
                                fillers.pop(0)()
                    # ---- sweep epilogue: den + ctx evacuation ---------
                    # Denominators go out to DRAM and come back as a
                    # partition-broadcast read (baseline-proven path).
                    tcol = slice(th * 512, (th + 1) * 512)
                    rr = pstg.tile([65, 512], F32, tag="recrow", name="recrow")
                    nc.vector.tensor_copy(out=rr[64:65, :],
                                          in_=cx_e[64:65, :])
                    # den write + bcast read share the gpsimd queue so FIFO
                    # order guarantees the DRAM RAW dependency
                    nc.gpsimd.dma_start(out=den_dram[2 * j, tcol],
                                        in_=rr[64:65, :])
                    rr2 = pstg.tile([65, 512], F32, tag="recrow",
                                    name="recrow2")
                    nc.vector.tensor_copy(out=rr2[64:65, :],
                                          in_=cx_o[64:65, :])
                    nc.gpsimd.dma_start(out=den_dram[2 * j + 1, tcol],
                                        in_=rr2[64:65, :])
                    nc.vector.tensor_copy(out=ctxT_sb[j][0:64, tcol],
                                          in_=cx_e[0:64, :])
                    tmp = ptmp.tile([64, 512], BF16, tag="ctmp", name="ctmp")
                    nc.vector.tensor_copy(out=tmp, in_=cx_o[0:64, :])
                    nc.sync.dma_start(out=ctxT_sb[j][64:128, tcol], in_=tmp)
                    # ---- normalize + bias for this query half ---------
                    # (overlaps the next sweep; keeps the pair-boundary
                    # and attention->FC bubbles short)
                    nc.gpsimd.dma_start(
                        out=rec_den[0:64, tcol],
                        in_=bcast_ap(den_dram[2 * j:2 * j + 1, tcol], 64))
                    nc.gpsimd.dma_start(
                        out=rec_den[64:128, tcol],
                        in_=bcast_ap(den_dram[2 * j + 1:2 * j + 2, tcol], 64))
                    nc.vector.reciprocal_approx_fast(
                        out=rec_den[:, tcol], in_=rec_den[:, tcol])
                    nc.vector.tensor_mul(out=ctxT_sb[j][:, tcol],
                                         in0=ctxT_sb[j][:, tcol],
                                         in1=rec_den[:, tcol])
                    nc.vector.tensor_scalar(out=ctxT_sb[j][:, tcol],
                                            in0=ctxT_sb[j][:, tcol],
                                            scalar1=bvT[:, j:j + 1],
                                            scalar2=None, op0=ALU.add)
                while fillers:
                    fillers.pop(0)()

        # ================= FC + residual + layernorm ====================
        with tc.tile_pool(name="fcps", bufs=2, space="PSUM") as pfc, \
             tc.tile_pool(name="lnbc", bufs=1) as plnb, \
             tc.tile_pool(name="qpl", bufs=2) as pqp, \
             tc.tile_pool(name="xln", bufs=2) as px, \
             tc.tile_pool(name="stat", bufs=4) as pst:
            gamma_bc = plnb.tile([128, D], F32, tag="gamma_bc", name="gamma_bc")
            nc.gpsimd.dma_start(out=gamma_bc, in_=bcast_ap(gamma, 128))
            beta_bc = plnb.tile([128, D], F32, tag="beta_bc", name="beta_bc")
            nc.gpsimd.dma_start(out=beta_bc, in_=bcast_ap(beta, 128))
            bfc_bc = plnb.tile([128, D], F32, tag="bfc_bc", name="bfc_bc")
            nc.gpsimd.dma_start(out=bfc_bc, in_=bcast_ap(bfc, 128))

            for t in range(TB):
                qp_t = pqp.tile([128, D], F32, tag="qp_t", name="qp_t")
                nc.sync.dma_start(out=qp_t,
                                  in_=qp_dram[t * 128:(t + 1) * 128, :])
                # bfc-add is off the fc critical chain: runs as soon as the
                # readback lands, before fc is ready.
                nc.gpsimd.tensor_add(out=qp_t, in0=qp_t, in1=bfc_bc)
                fc = pfc.tile([128, D], F32, tag="fc", name="fc")
                for jj in range(PAIRS):
                    for n0 in range(0, D, 512):
                        nc.tensor.matmul(
                            fc[:, n0:n0 + 512],
                            lhsT=ctxT_sb[jj][:, t * 128:(t + 1) * 128],
                            rhs=wfc_sb[:, jj, n0:n0 + 512],
                            start=(jj == 0), stop=(jj == PAIRS - 1))
                x = px.tile([128, D], F32, tag="x", name="x")
                # fc is PSUM: GpSimd cannot read it, so this add stays on DVE
                nc.vector.tensor_add(out=x, in0=fc, in1=qp_t)
                ngr = max(D // 512, 1)
                gsz = min(D, 512)
                stats = pst.tile([128, ngr, 6], F32, tag="stats", name="stats")
                for g in range(ngr):
                    nc.vector.bn_stats(out=stats[:, g, :],
                                       in_=x[:, g * gsz:(g + 1) * gsz])
                mv = pst.tile([128, 2], F32, tag="mv", name="mv")
                nc.vector.bn_aggr(out=mv, in_=stats)
                rstd = pst.tile([128, 1], F32, tag="rstd", name="rstd")
                nc.scalar.activation(out=rstd, in_=mv[:, 1:2], func=AF.Sqrt,
                                     bias=eps_t, scale=1.0)
                nc.vector.reciprocal(out=rstd, in_=rstd)
                xn = px.tile([128, D], F32, tag="xn", name="xn")
                rows = slice(t * 128, (t + 1) * 128)
                if t == TB - 1:
                    # The last block's chain IS the kernel tail: run it as
                    # two parallel column-half chains on DVE || Pool and
                    # split the store across both DMA queues.
                    for cols, engh, oeng in (
                            (slice(0, D // 2), nc.vector, nc.sync),
                            (slice(D // 2, D), nc.gpsimd, nc.scalar)):
                        engh.tensor_scalar(out=xn[:, cols], in0=x[:, cols],
                                           scalar1=mv[:, 0:1], scalar2=rstd,
                                           op0=ALU.subtract, op1=ALU.mult)
                        engh.tensor_mul(out=xn[:, cols], in0=xn[:, cols],
                                        in1=gamma_bc[:, cols])
                        engh.tensor_add(out=xn[:, cols], in0=xn[:, cols],
                                        in1=beta_bc[:, cols])
                        oeng.dma_start(out=out[rows, cols], in_=xn[:, cols])
                else:
                    eng = nc.vector if t % 2 == 0 else nc.gpsimd
                    eng.tensor_scalar(out=xn, in0=x, scalar1=mv[:, 0:1],
                                      scalar2=rstd, op0=ALU.subtract,
                                      op1=ALU.mult)
                    nc.gpsimd.tensor_mul(out=xn, in0=xn, in1=gamma_bc)
                    nc.gpsimd.tensor_add(out=xn, in0=xn, in1=beta_bc)
                    out_eng = nc.sync if t % 2 == 0 else nc.scalar
                    out_eng.dma_start(out=out[rows, :], in_=xn)

    nc.compile()
    return nc


_B, _S, _D, _H, _DK = 4, 2048, 1024, 16, 64
_T = _S // 2
_NCORES = 8
_BF = ml_dtypes.bfloat16

_nc_cache = [None]


def _get_nc():
    if _nc_cache[0] is None:
        _nc_cache[0] = build(T=_T, S=_S, D=_D, H=_H, DK=_DK, n_cores=_NCORES)
    return _nc_cache[0]


def _execute(inputs, trace=False):
    from concourse.bass_utils import run_bass_kernel_spmd

    nc = _get_nc()
    q = np.asarray(inputs["q"], np.float32)
    k = np.asarray(inputs["k"], np.float32)
    v = np.asarray(inputs["v"], np.float32)
    Wq = np.asarray(inputs["Wq"], np.float32).astype(_BF)
    Wk = np.asarray(inputs["Wk"], np.float32).astype(_BF)
    Wv = np.asarray(inputs["Wv"], np.float32).astype(_BF)
    Wfc = np.asarray(inputs["Wfc"], np.float32).astype(_BF)
    fp = {n: np.asarray(inputs[n], np.float32)
          for n in ("bq", "bk", "bv", "bfc", "gamma", "beta")}

    in_maps = []
    for c in range(_NCORES):
        b, half = divmod(c, 2)
        t0 = half * _T
        in_maps.append({
            "qT": np.ascontiguousarray(q[b, t0:t0 + _T].T).astype(_BF),
            "kT": np.ascontiguousarray(k[b].T).astype(_BF),
            "vT": np.ascontiguousarray(v[b].T).astype(_BF),
            "Wq": Wq, "Wk": Wk, "Wv": Wv, "Wfc": Wfc, **fp,
        })

    res = run_bass_kernel_spmd(nc, in_maps, core_ids=list(range(_NCORES)),
                               trace=trace)
    out = np.empty((_B, _S, _D), np.float32)
    for c in range(_NCORES):
        b, half = divmod(c, 2)
        out[b, half * _T:(half + 1) * _T] = res.results[c]["out"]
    return out, res.exec_time_ns


def kernel(**inputs) -> np.ndarray:
    out, _ = _execute(inputs, trace=False)
    return out



# revision 83
# speedup vs baseline: 1.1839x; 1.0068x over previous
"""Trainium2 Bass kernel for nn_AttentionLayer (B=4, S=2048, D=1024, H=16).

Self-contained: builds and compiles an SPMD Bass/Tile program once, then
runs it across 8 NeuronCores via run_bass_kernel_spmd.

Sharding (no collectives): core c handles batch b = c // 2 and query-token
half c % 2 (T=1024 query tokens). Each core receives pre-transposed bf16
activations plus bf16 weights, computes its [1024, 1024] slice of the
final layernorm output in fp32, and the host reassembles.

v2 pipeline: one continuous exp-overlapped stream. All projection work
(K and Q per-pair, V per-2-pair-chunk) is emitted as filler thunks inside
the attention loop so TensorE slack under the ScalarE exp stream is
filled; only pair 0's projections run up front (column-chunked DMAs so
compute starts on the first chunk). Attention runs per head-pair with
query-half sweeps so PSUM fits:
  sc 2x[128,1024] (4 banks) + cx 2x[65,512] (2) + fill 2x[128,512] (2).
The two heads' score matmuls contract over 64 rows at partitions 0-63 /
64-127 (disjoint PE row groups) and are forced adjacent in the schedule
via high_priority, so the hardware runs each pair concurrently (~2x).
Softmax denominators come from a ones-column in the V projection and take
a DRAM roundtrip for the partition-broadcast (GpSimd cannot touch PSUM,
and partition_broadcast corrupts on HW). The residual q-projection is
computed natural (fp32, DRAM staging for the FC phase) and PE-transposed
into the attention layout, replacing a second full Q projection. FC
matmuls overlap the final attention pair; layernorm alternates DVE/Pool.
"""

import numpy as np
import ml_dtypes

from contextlib import ExitStack

import concourse.bass as bass
import concourse.tile as tile
import concourse.mybir as mybir
from concourse import bacc
from concourse import masks

F32 = mybir.dt.float32
BF16 = mybir.dt.bfloat16
AF = mybir.ActivationFunctionType
ALU = mybir.AluOpType


def bcast_ap(ap: bass.AP, parts: int) -> bass.AP:
    """Partition-broadcast a [1, N]-shaped DRAM AP to [parts, N]."""
    return bass.AP(tensor=ap.tensor, offset=ap.offset,
                   ap=[[0, parts]] + list(ap.ap[-1:]))


def build(T=1024, S=2048, D=1024, H=16, DK=64, n_cores=8, eps=1e-5,
          trn_type="TRN2"):
    assert DK == 64 and H % 2 == 0 and D == H * DK
    DB = D // 128      # contraction chunks over d
    PAIRS = H // 2     # head pairs == 128-row output blocks
    TB = T // 128
    SB = S // 128      # key blocks
    KBP = SB // 2      # key-block pairs per sweep
    NTH = T // 512     # query halves
    VW = 65            # per-head vp stripe: 64 v columns + 1 ones column
    VCH = 2            # pairs per V-projection chunk

    nc = bacc.Bacc(trn_type, target_bir_lowering=False, debug=False,
                   num_devices=n_cores)

    qT = nc.dram_tensor("qT", [D, T], BF16, kind="ExternalInput").ap()
    kT = nc.dram_tensor("kT", [D, S], BF16, kind="ExternalInput").ap()
    vT = nc.dram_tensor("vT", [D, S], BF16, kind="ExternalInput").ap()
    Wq = nc.dram_tensor("Wq", [D, D], BF16, kind="ExternalInput").ap()
    Wk = nc.dram_tensor("Wk", [D, D], BF16, kind="ExternalInput").ap()
    Wv = nc.dram_tensor("Wv", [D, D], BF16, kind="ExternalInput").ap()
    Wfc = nc.dram_tensor("Wfc", [D, D], BF16, kind="ExternalInput").ap()
    bq = nc.dram_tensor("bq", [D], F32, kind="ExternalInput").ap()
    bk = nc.dram_tensor("bk", [D], F32, kind="ExternalInput").ap()
    bv = nc.dram_tensor("bv", [D], F32, kind="ExternalInput").ap()
    bfc = nc.dram_tensor("bfc", [D], F32, kind="ExternalInput").ap()
    gamma = nc.dram_tensor("gamma", [D], F32, kind="ExternalInput").ap()
    beta = nc.dram_tensor("beta", [D], F32, kind="ExternalInput").ap()
    out = nc.dram_tensor("out", [T, D], F32, kind="ExternalOutput").ap()

    qp_dram = nc.dram_tensor("qp_scratch", [T, D], F32).ap()
    den_dram = nc.dram_tensor("den_scratch", [H, T], F32).ap()
    fcp_dram = nc.dram_tensor("fcp_scratch", [T, D], BF16).ap()
    FCP = TB // 4      # leading t-blocks whose FC runs inside pair 7

    WkR = Wk.rearrange("(db p) n -> p db n", p=128)
    WqR = Wq.rearrange("(db p) n -> p db n", p=128)
    WvR = Wv.rearrange("(db p) n -> p db n", p=128)
    WfcR = Wfc.rearrange("(db p) n -> p db n", p=128)

    with tile.TileContext(nc) as tc, ExitStack() as ctx:
        pconst = ctx.enter_context(tc.tile_pool(name="const", bufs=1))
        ppers = ctx.enter_context(tc.tile_pool(name="persist", bufs=1))
        pkpt = ctx.enter_context(tc.tile_pool(name="kpt", bufs=2))
        pqpt = ctx.enter_context(tc.tile_pool(name="qpt", bufs=3))
        pwfc = ctx.enter_context(tc.tile_pool(name="wfc", bufs=1))

        # ---- tiny constants -------------------------------------------
        bkT = pconst.tile([128, PAIRS], F32, tag="bkT", name="bkT")
        nc.gpsimd.dma_start(out=bkT, in_=bk.rearrange("(e p) -> p e", p=128))
        bvT = pconst.tile([128, PAIRS], F32, tag="bvT", name="bvT")
        nc.gpsimd.dma_start(out=bvT, in_=bv.rearrange("(e p) -> p e", p=128))
        eps_t = pconst.tile([128, 1], F32, tag="eps", name="eps")
        nc.vector.memset(eps_t, eps)
        ident = pconst.tile([128, 128], F32, tag="ident", name="ident")
        masks.make_identity(nc, ident)

        # ---- persistent tiles -----------------------------------------
        vp_sb = [ppers.tile([128, H * VW], BF16, tag=f"vp{s}", name=f"vp{s}")
                 for s in range(SB)]
        ctxT_sb = [ppers.tile([128, T], BF16, tag=f"ctxT{e}", name=f"ctxT{e}")
                   for e in range(PAIRS)]
        wfc_sb = pwfc.tile([128, DB, D], BF16, tag="wfc", name="wfc")

        kpT_t = {}   # pair -> rotating kpT tile [128, S]
        qpT_t = {}   # pair -> rotating qpT tile [128, T]

        with ExitStack() as attn_ctx:
            pkx = attn_ctx.enter_context(tc.tile_pool(name="kx", bufs=1))
            pvx = attn_ctx.enter_context(tc.tile_pool(name="vx", bufs=1))
            pqx = attn_ctx.enter_context(tc.tile_pool(name="qx", bufs=1))
            pwk = attn_ctx.enter_context(tc.tile_pool(name="wk", bufs=2))
            pwq = attn_ctx.enter_context(tc.tile_pool(name="wq", bufs=2))
            pwv = attn_ctx.enter_context(tc.tile_pool(name="wv", bufs=2))
            pbq = attn_ctx.enter_context(tc.tile_pool(name="bq", bufs=2))
            patn = attn_ctx.enter_context(tc.tile_pool(name="attn", bufs=8))
            pfcp = attn_ctx.enter_context(tc.tile_pool(name="fcp", bufs=2))
            pstg = attn_ctx.enter_context(tc.tile_pool(name="stg", bufs=3))
            ptmp = attn_ctx.enter_context(tc.tile_pool(name="ctmp", bufs=2))
            prec = attn_ctx.enter_context(tc.tile_pool(name="rec", bufs=1))
            psc = attn_ctx.enter_context(
                tc.tile_pool(name="scps", bufs=2, space="PSUM"))
            pcx = attn_ctx.enter_context(
                tc.tile_pool(name="cxps", bufs=2, space="PSUM"))
            pfil = attn_ctx.enter_context(
                tc.tile_pool(name="filps", bufs=2, space="PSUM"))

            # ---- input staging ----------------------------------------
            # Queue split so pair-0 work is not gated behind bulk loads:
            # sync: wk + kT; scalar: vT; gpsimd: small weights + qT.
            wk_t = {}
            wq_t = {}
            wv_t = {}
            bq_t = {}

            def load_pair_weights(j):
                wk_t[j] = pwk.tile([128, DB, 128], BF16, tag="wk",
                                   name=f"wk{j}")
                nc.sync.dma_start(out=wk_t[j],
                                  in_=WkR[:, :, j * 128:(j + 1) * 128])
                wq_t[j] = pwq.tile([128, DB, 128], BF16, tag="wq",
                                   name=f"wq{j}")
                nc.gpsimd.dma_start(out=wq_t[j],
                                    in_=WqR[:, :, j * 128:(j + 1) * 128])
                bq_t[j] = pbq.tile([128, 128], F32, tag="bq", name=f"bq{j}")
                nc.gpsimd.dma_start(out=bq_t[j],
                                    in_=bcast_ap(bq[j * 128:(j + 1) * 128], 128))

            def load_vchunk_weights(c):
                wv_t[c] = pwv.tile([128, DB, VCH * 128], BF16, tag="wv",
                                   name=f"wv{c}")
                nc.gpsimd.dma_start(
                    out=wv_t[c],
                    in_=WvR[:, :, c * VCH * 128:(c + 1) * VCH * 128])

            load_pair_weights(0)
            load_vchunk_weights(0)
            kx_sb = [pkx.tile([128, S], BF16, tag=f"kx{d}", name=f"kx{d}")
                     for d in range(DB)]
            vx_sb = [pvx.tile([128, S], BF16, tag=f"vx{d}", name=f"vx{d}")
                     for d in range(DB)]
            qx_sb = [pqx.tile([128, T], BF16, tag=f"qx{d}", name=f"qx{d}")
                     for d in range(DB)]
            # Input loads are column-chunk-major so the pair-0 projections
            # can start on the first chunk instead of the full tensor.
            for ci in range(S // 512):
                for d in range(DB):
                    nc.sync.dma_start(
                        out=kx_sb[d][:, ci * 512:(ci + 1) * 512],
                        in_=kT[d * 128:(d + 1) * 128, ci * 512:(ci + 1) * 512])
            for ci in range(S // 512):
                for d in range(DB):
                    nc.scalar.dma_start(
                        out=vx_sb[d][:, ci * 512:(ci + 1) * 512],
                        in_=vT[d * 128:(d + 1) * 128, ci * 512:(ci + 1) * 512])
            for ci in range(T // 512):
                for d in range(DB):
                    nc.gpsimd.dma_start(
                        out=qx_sb[d][:, ci * 512:(ci + 1) * 512],
                        in_=qT[d * 128:(d + 1) * 128, ci * 512:(ci + 1) * 512])

            # ---- thunk builders (emit one group of work each) ---------
            def k_thunk(j, ci):
                def f():
                    if j not in kpT_t:
                        kpT_t[j] = pkpt.tile([128, S], BF16, tag="kpT",
                                             name=f"kpT{j}")
                    ps = pfil.tile([128, 512], F32, tag="fil", name="kps")
                    for d in range(DB):
                        nc.tensor.matmul(
                            ps, lhsT=wk_t[j][:, d, :],
                            rhs=kx_sb[d][:, ci * 512:(ci + 1) * 512],
                            start=(d == 0), stop=(d == DB - 1))
                    nc.vector.tensor_scalar(
                        out=kpT_t[j][:, ci * 512:(ci + 1) * 512], in0=ps,
                        scalar1=bkT[:, j:j + 1], scalar2=None, op0=ALU.add)
                return f

            def v_thunk(c, s):
                def f():
                    ps = pfil.tile([128, 512], F32, tag="fil", name="vps")
                    psv = ps[:, 0:VCH * 128]
                    for d in range(DB):
                        nc.tensor.matmul(
                            psv, lhsT=vx_sb[d][:, s * 128:(s + 1) * 128],
                            rhs=wv_t[c][:, d, :],
                            start=(d == 0), stop=(d == DB - 1))
                    vr = vp_sb[s].rearrange("p (h c) -> p h c", c=VW)
                    nc.vector.tensor_copy(
                        out=vr[:, 2 * VCH * c:2 * VCH * (c + 1), 0:64],
                        in_=psv.rearrange("p (h c) -> p h c", c=64))
                return f

            def qp_thunk(j, t):
                def f():
                    if j not in qpT_t:
                        qpT_t[j] = pqpt.tile([128, T], BF16, tag="qpT",
                                             name=f"qpT{j}")
                    ps = pfil.tile([128, 512], F32, tag="fil", name="qps")
                    psq = ps[:, 0:128]
                    for d in range(DB):
                        nc.tensor.matmul(
                            psq, lhsT=qx_sb[d][:, t * 128:(t + 1) * 128],
                            rhs=wq_t[j][:, d, :],
                            start=(d == 0), stop=(d == DB - 1))
                    stg = pstg.tile([128, 128], F32, tag="qpn", name="qpn")
                    nc.vector.tensor_add(out=stg, in0=psq, in1=bq_t[j])
                    nc.sync.dma_start(
                        out=qp_dram[t * 128:(t + 1) * 128,
                                    j * 128:(j + 1) * 128],
                        in_=stg)
                    trp = pfil.tile([128, 512], F32, tag="fil", name="trp")
                    nc.tensor.transpose(trp[:, 0:128], stg, ident)
                    nc.vector.tensor_copy(
                        out=qpT_t[j][:, t * 128:(t + 1) * 128],
                        in_=trp[:, 0:128])
                return f

            def fcp_thunk(t, half):
                # FC for a leading t-block (touches only the already-
                # normalized first query-half of every ctxT): runs inside
                # pair 7's slack, staged to DRAM in bf16.
                def f():
                    ps = pfil.tile([128, 512], F32, tag="fil", name="fcp")
                    for jj in range(PAIRS):
                        nc.tensor.matmul(
                            ps, lhsT=ctxT_sb[jj][:, t * 128:(t + 1) * 128],
                            rhs=wfc_sb[:, jj, half * 512:(half + 1) * 512],
                            start=(jj == 0), stop=(jj == PAIRS - 1))
                    stg = pfcp.tile([128, 512], BF16, tag="fcp", name="fcps")
                    nc.vector.tensor_copy(out=stg, in_=ps)
                    nc.sync.dma_start(
                        out=fcp_dram[t * 128:(t + 1) * 128,
                                     half * 512:(half + 1) * 512],
                        in_=stg)
                return f

            def interleave(*lists):
                res = []
                n = max(len(x) for x in lists)
                for i in range(n):
                    for x in lists:
                        if i < len(x):
                            res.append(x[i])
                return res

            def pair_fillers(j):
                """Work to interleave into pair j's attention stream."""
                nxt = j + 1
                ks, qs, vs, misc = [], [], [], []
                if nxt < PAIRS:
                    load_pair_weights(nxt)
                    ks = [k_thunk(nxt, ci) for ci in range(S // 512)]
                    qs = [qp_thunk(nxt, t) for t in range(TB)]
                # V chunk c (pairs 2c, 2c+1): half during pair 2c-2, half
                # during pair 2c-1, so filler load is spread evenly.
                c = j // 2 + 1
                if c < PAIRS // VCH:
                    if j % 2 == 0:
                        load_vchunk_weights(c)
                    half = SB // 2
                    s0 = (j % 2) * half
                    vs = [v_thunk(c, s) for s in range(s0, s0 + half)]
                if j == PAIRS - 2:
                    def load_wfc():
                        nc.sync.dma_start(out=wfc_sb, in_=WfcR)
                    misc = [load_wfc]
                return interleave(ks, qs, vs) + misc

            # ================= prologue: pair 0 compute ================
            # Interleaved by input chunk so each thunk starts as soon as
            # its DMA slice lands.
            for s in range(SB):
                vr = vp_sb[s].rearrange("p (h c) -> p h c", c=VW)
                nc.vector.memset(vr[:, :, 64:65], 1.0)
            for ci in range(S // 512):
                k_thunk(0, ci)()
                for s in range(4 * ci, 4 * ci + 4):
                    v_thunk(0, s)()
                for t in range(2 * ci, min(2 * ci + 2, TB)):
                    qp_thunk(0, t)()

            # ================= attention stream ========================
            for j in range(PAIRS):
                fillers = pair_fillers(j)
                rec_den = prec.tile([128, T], F32, tag="rec", name="rec")
                kpt = kpT_t.pop(j)
                qpt = qpT_t.pop(j)
                for th in range(NTH):
                    cx_e = pcx.tile([VW, 512], F32, tag="cx", name="cxe")
                    cx_o = pcx.tile([VW, 512], F32, tag="cx", name="cxo")
                    for kbp in range(KBP):
                        sc_e = psc.tile([128, 1024], F32, tag="sc", name="sce")
                        sc_o = psc.tile([128, 1024], F32, tag="sc", name="sco")
                        # High priority keeps the even/odd head score matmuls
                        # adjacent in the scheduled PE stream: they target
                        # disjoint PE row groups (contraction rows 0-63 vs
                        # 64-127), so the HW runs adjacent pairs concurrently.
                        with tc.high_priority():
                            for kk in range(2):
                                kb = 2 * kbp + kk
                                for pr0, sc in ((0, sc_e), (64, sc_o)):
                                    nc.tensor.matmul(
                                        sc[:, kk * 512:(kk + 1) * 512],
                                        lhsT=kpt[pr0:pr0 + 64,
                                                 kb * 128:(kb + 1) * 128],
                                        rhs=qpt[pr0:pr0 + 64,
                                                th * 512:(th + 1) * 512],
                                        start=True, stop=True)
                        at_e = patn.tile([128, 1024], BF16, tag="at",
                                         name="ate")
                        nc.scalar.activation(out=at_e, in_=sc_e, func=AF.Exp,
                                             scale=0.125)
                        at_o = patn.tile([128, 1024], BF16, tag="at",
                                         name="ato")
                        nc.scalar.activation(out=at_o, in_=sc_o, func=AF.Exp,
                                             scale=0.125)
                        st = (kbp == 0)
                        sp = (kbp == KBP - 1)
                        for kk in range(2):
                            kb = 2 * kbp + kk
                            vr = vp_sb[kb].rearrange("p (h c) -> p h c", c=VW)
                            nc.tensor.matmul(
                                cx_e, lhsT=vr[:, 2 * j, :],
                                rhs=at_e[:, kk * 512:(kk + 1) * 512],
                                start=(st and kk == 0), stop=(sp and kk == 1))
                            nc.tensor.matmul(
                                cx_o, lhsT=vr[:, 2 * j + 1, :],
                                rhs=at_o[:, kk * 512:(kk + 1) * 512],
                                start=(st and kk == 0), stop=(sp and kk == 1))
                        npop = 1 if (th == 0 and kbp < 4) else 3
                        for _ in range(npop):
                            if fillers:
                                fillers.pop(0)()
                    # ---- sweep epilogue: den + ctx evacuation ---------
                    # Denominators go out to DRAM and come back as a
                    # partition-broadcast read (baseline-proven path).
                    tcol = slice(th * 512, (th + 1) * 512)
                    rr = pstg.tile([65, 512], F32, tag="recrow", name="recrow")
                    nc.vector.tensor_copy(out=rr[64:65, :],
                                          in_=cx_e[64:65, :])
                    # den write + bcast read share the gpsimd queue so FIFO
                    # order guarantees the DRAM RAW dependency
                    nc.gpsimd.dma_start(out=den_dram[2 * j, tcol],
                                        in_=rr[64:65, :])
                    rr2 = pstg.tile([65, 512], F32, tag="recrow",
                                    name="recrow2")
                    nc.vector.tensor_copy(out=rr2[64:65, :],
                                          in_=cx_o[64:65, :])
                    nc.gpsimd.dma_start(out=den_dram[2 * j + 1, tcol],
                                        in_=rr2[64:65, :])
                    nc.vector.tensor_copy(out=ctxT_sb[j][0:64, tcol],
                                          in_=cx_e[0:64, :])
                    tmp = ptmp.tile([64, 512], BF16, tag="ctmp", name="ctmp")
                    nc.vector.tensor_copy(out=tmp, in_=cx_o[0:64, :])
                    nc.sync.dma_start(out=ctxT_sb[j][64:128, tcol], in_=tmp)
                    # ---- normalize + bias for this query half ---------
                    # (overlaps the next sweep; keeps the pair-boundary
                    # and attention->FC bubbles short)
                    nc.gpsimd.dma_start(
                        out=rec_den[0:64, tcol],
                        in_=bcast_ap(den_dram[2 * j:2 * j + 1, tcol], 64))
                    nc.gpsimd.dma_start(
                        out=rec_den[64:128, tcol],
                        in_=bcast_ap(den_dram[2 * j + 1:2 * j + 2, tcol], 64))
                    nc.vector.reciprocal_approx_fast(
                        out=rec_den[:, tcol], in_=rec_den[:, tcol])
                    nc.vector.tensor_mul(out=ctxT_sb[j][:, tcol],
                                         in0=ctxT_sb[j][:, tcol],
                                         in1=rec_den[:, tcol])
                    nc.vector.tensor_scalar(out=ctxT_sb[j][:, tcol],
                                            in0=ctxT_sb[j][:, tcol],
                                            scalar1=bvT[:, j:j + 1],
                                            scalar2=None, op0=ALU.add)
                    # Pair 7, after the first query-half is normalized:
                    # queue the leading t-blocks' FC as fillers for the
                    # second sweep. (Emitted post-normalize, so program
                    # order carries the ctxT dependency.)
                    if j == PAIRS - 1 and th == 0:
                        fillers.extend(fcp_thunk(t, half)
                                       for t in range(FCP)
                                       for half in range(2))
                while fillers:
                    fillers.pop(0)()

        # ================= FC + residual + layernorm ====================
        with tc.tile_pool(name="fcps", bufs=2, space="PSUM") as pfc, \
             tc.tile_pool(name="lnbc", bufs=1) as plnb, \
             tc.tile_pool(name="qpl", bufs=2) as pqp, \
             tc.tile_pool(name="xln", bufs=2) as px, \
             tc.tile_pool(name="stat", bufs=4) as pst:
            gamma_bc = plnb.tile([128, D], F32, tag="gamma_bc", name="gamma_bc")
            nc.gpsimd.dma_start(out=gamma_bc, in_=bcast_ap(gamma, 128))
            beta_bc = plnb.tile([128, D], F32, tag="beta_bc", name="beta_bc")
            nc.gpsimd.dma_start(out=beta_bc, in_=bcast_ap(beta, 128))
            bfc_bc = plnb.tile([128, D], F32, tag="bfc_bc", name="bfc_bc")
            nc.gpsimd.dma_start(out=bfc_bc, in_=bcast_ap(bfc, 128))

            for t in range(TB):
                qp_t = pqp.tile([128, D], F32, tag="qp_t", name="qp_t")
                nc.sync.dma_start(out=qp_t,
                                  in_=qp_dram[t * 128:(t + 1) * 128, :])
                # bfc-add is off the fc critical chain: runs as soon as the
                # readback lands, before fc is ready.
                nc.gpsimd.tensor_add(out=qp_t, in0=qp_t, in1=bfc_bc)
                x = px.tile([128, D], F32, tag="x", name="x")
                if t < FCP:
                    # fc for this block was computed inside pair 7 and
                    # staged to DRAM in bf16.
                    fcr = pqp.tile([128, D], BF16, tag="fcr", name="fcr")
                    nc.sync.dma_start(out=fcr,
                                      in_=fcp_dram[t * 128:(t + 1) * 128, :])
                    nc.vector.tensor_add(out=x, in0=fcr, in1=qp_t)
                else:
                    fc = pfc.tile([128, D], F32, tag="fc", name="fc")
                    for jj in range(PAIRS):
                        for n0 in range(0, D, 512):
                            nc.tensor.matmul(
                                fc[:, n0:n0 + 512],
                                lhsT=ctxT_sb[jj][:, t * 128:(t + 1) * 128],
                                rhs=wfc_sb[:, jj, n0:n0 + 512],
                                start=(jj == 0), stop=(jj == PAIRS - 1))
                    # fc is PSUM: GpSimd cannot read it -> DVE
                    nc.vector.tensor_add(out=x, in0=fc, in1=qp_t)
                ngr = max(D // 512, 1)
                gsz = min(D, 512)
                stats = pst.tile([128, ngr, 6], F32, tag="stats", name="stats")
                for g in range(ngr):
                    nc.vector.bn_stats(out=stats[:, g, :],
                                       in_=x[:, g * gsz:(g + 1) * gsz])
                mv = pst.tile([128, 2], F32, tag="mv", name="mv")
                nc.vector.bn_aggr(out=mv, in_=stats)
                rstd = pst.tile([128, 1], F32, tag="rstd", name="rstd")
                nc.scalar.activation(out=rstd, in_=mv[:, 1:2], func=AF.Sqrt,
                                     bias=eps_t, scale=1.0)
                nc.vector.reciprocal(out=rstd, in_=rstd)
                xn = px.tile([128, D], F32, tag="xn", name="xn")
                rows = slice(t * 128, (t + 1) * 128)
                if t == TB - 1:
                    # The last block's chain IS the kernel tail: run it as
                    # two parallel column-half chains on DVE || Pool and
                    # split the store across both DMA queues.
                    for cols, engh, oeng in (
                            (slice(0, D // 2), nc.vector, nc.sync),
                            (slice(D // 2, D), nc.gpsimd, nc.scalar)):
                        engh.tensor_scalar(out=xn[:, cols], in0=x[:, cols],
                                           scalar1=mv[:, 0:1], scalar2=rstd,
                                           op0=ALU.subtract, op1=ALU.mult)
                        engh.tensor_mul(out=xn[:, cols], in0=xn[:, cols],
                                        in1=gamma_bc[:, cols])
                        engh.tensor_add(out=xn[:, cols], in0=xn[:, cols],
                                        in1=beta_bc[:, cols])
                        oeng.dma_start(out=out[rows, cols], in_=xn[:, cols])
                else:
                    eng = nc.vector if t % 2 == 0 else nc.gpsimd
                    eng.tensor_scalar(out=xn, in0=x, scalar1=mv[:, 0:1],
                                      scalar2=rstd, op0=ALU.subtract,
                                      op1=ALU.mult)
                    nc.gpsimd.tensor_mul(out=xn, in0=xn, in1=gamma_bc)
                    nc.gpsimd.tensor_add(out=xn, in0=xn, in1=beta_bc)
                    out_eng = nc.sync if t % 2 == 0 else nc.scalar
                    out_eng.dma_start(out=out[rows, :], in_=xn)

    nc.compile()
    return nc


_B, _S, _D, _H, _DK = 4, 2048, 1024, 16, 64
_T = _S // 2
_NCORES = 8
_BF = ml_dtypes.bfloat16

_nc_cache = [None]


def _get_nc():
    if _nc_cache[0] is None:
        _nc_cache[0] = build(T=_T, S=_S, D=_D, H=_H, DK=_DK, n_cores=_NCORES)
    return _nc_cache[0]


def _execute(inputs, trace=False):
    from concourse.bass_utils import run_bass_kernel_spmd

    nc = _get_nc()
    q = np.asarray(inputs["q"], np.float32)
    k = np.asarray(inputs["k"], np.float32)
    v = np.asarray(inputs["v"], np.float32)
    Wq = np.asarray(inputs["Wq"], np.float32).astype(_BF)
    Wk = np.asarray(inputs["Wk"], np.float32).astype(_BF)
    Wv = np.asarray(inputs["Wv"], np.float32).astype(_BF)
    Wfc = np.asarray(inputs["Wfc"], np.float32).astype(_BF)
    fp = {n: np.asarray(inputs[n], np.float32)
          for n in ("bq", "bk", "bv", "bfc", "gamma", "beta")}

    in_maps = []
    for c in range(_NCORES):
        b, half = divmod(c, 2)
        t0 = half * _T
        in_maps.append({
            "qT": np.ascontiguousarray(q[b, t0:t0 + _T].T).astype(_BF),
            "kT": np.ascontiguousarray(k[b].T).astype(_BF),
            "vT": np.ascontiguousarray(v[b].T).astype(_BF),
            "Wq": Wq, "Wk": Wk, "Wv": Wv, "Wfc": Wfc, **fp,
        })

    res = run_bass_kernel_spmd(nc, in_maps, core_ids=list(range(_NCORES)),
                               trace=trace)
    out = np.empty((_B, _S, _D), np.float32)
    for c in range(_NCORES):
        b, half = divmod(c, 2)
        out[b, half * _T:(half + 1) * _T] = res.results[c]["out"]
    return out, res.exec_time_ns


def kernel(**inputs) -> np.ndarray:
    out, _ = _execute(inputs, trace=False)
    return out

